# revision 1
# baseline (speedup 1.0000x reference)
"""Hetero GNN encoder/decoder (SAGE x2 + BN + edge MLP decoder) on 8 trn2 cores.

Strategy (edge sharding by destination, node-range sharding):
  - Articles: core k owns rows [k*APC, (k+1)*APC); customers likewise (CPC).
  - Message edges partitioned by dst-owner core; mean-aggregation is computed
    fully locally (scale 1/cnt folded into the one-hot), via
    dma_gather (int16 per-src-block indices) + one-hot matmul (X^T @ P)
    scatter into PSUM windows of 128 nodes.  One-hot P tiles are built in
    batches of 8 tiles with wide tensor_tensor ops (DVE op overhead
    dominates, and GpSimd SWDGE descriptor-gen contends for the shared
    DVE/POOL SBUF port - fewer, wider DVE ops).
  - After each SAGE layer, node features are AllGathered (row-major fp16
    tables) so the next layer / decoder can gather from the full table.
  - BatchNorm: local per-channel partial sums + tiny AllReduce.
  - Decoder uses precomputed U_c = bn(z_c) @ Wd1[:128] + b_dec1 and
    U_a = bn(z_a) @ Wd1[128:] tables; per label y = w2 . relu(U_c[lc]+U_a[la])
    + b2 - no PE work in the decoder loop at all.

All structure (loop bounds, window emissions) is compile-time and identical
across cores; per-core variation lives in the data (padded to uniform sizes).
"""
import sys

sys.path.insert(0, "/opt/trn_rl_repo")

import numpy as np

import concourse.bacc as bacc
import concourse.bass as bass
import concourse.mybir as mybir
import concourse.tile as tile
from concourse.bass_utils import run_bass_kernel_spmd
from concourse.masks import make_identity

P = 128
NCORES = 8
GCH = 1024          # indices per dma_gather
TPC = GCH // P      # tiles per gather chunk (8)
WCH = 512           # W-stage column chunk
MAXW = 4            # max windows per tile (window-relative encoding)
BN_EPS = 1e-5


class Cfg:
    def __init__(self, n_c=300000, n_a=100000, e_lbl=1000000,
                 sbn=1280, srcb_c=30000, srcb_a=25000, f16=True):
        self.n_c, self.n_a, self.e_lbl = n_c, n_a, e_lbl
        self.cpc, self.apc = n_c // NCORES, n_a // NCORES
        assert self.cpc * NCORES == n_c and self.apc * NCORES == n_a
        self.chalf = self.cpc // 2
        assert self.chalf * 2 == self.cpc
        self.sbn = sbn
        self.srcb_c, self.srcb_a = srcb_c, srcb_a
        self.nblk_c = -(-n_c // srcb_c)
        self.nblk_a = -(-n_a // srcb_a)
        assert srcb_c < 32768 and srcb_a < 32768
        self.zc_sub = -(-self.cpc // 2)          # U_c local gather sub-block
        assert self.zc_sub < 32768
        self.f16 = f16
        self.dt = mybir.dt.float16 if f16 else mybir.dt.float32
        self.npdt = np.float16 if f16 else np.float32


def _ru(x, m):
    return (x + m - 1) // m * m


def _wrap_idx_chunk(flat):
    """[n] int16 -> [128, n/16] wrap (16-partition, replicated x8)."""
    n = flat.shape[0]
    w = flat.astype(np.int16).reshape(n // 16, 16).T
    return np.tile(w, (8, 1))


def _pack_pcol(a):
    """[n] -> [128, n/128]: element i -> partition i%128, col i//128."""
    return np.ascontiguousarray(a.reshape(-1, P).T)


# ---------------------------------------------------------------------------
# host-side structure + array prep for one aggregation pass
# ---------------------------------------------------------------------------

class AggPass:
    """Static structure (shared across cores) + per-core packed arrays."""

    def __init__(self, name, nloc, srcb, nsrc_blk, nsrc_rows, sbn):
        self.name = name
        self.nloc = nloc
        self.srcb = srcb
        self.nsrc_blk = nsrc_blk
        self.nsrc_rows = nsrc_rows
        self.sbn = sbn
        self.nsb = -(-nloc // sbn)
        self.sb_nodes = [min(sbn, nloc - s * sbn) for s in range(self.nsb)]
        self.run_L = None        # [nsb, nsrc_blk] uniform padded run lengths
        self.etot = 0
        self.emits = None        # per sb: list of (j, t, wlo, nwin) per tile
        self.uncovered = None    # per sb: list of never-touched windows
        self.idx = None          # per core [128, etot/16] int16
        self.dsc = None          # per core [128, 2*etot/128] f16 (dst|scl/run)


def prep_agg_pass(name, src, dst_loc, scale_e, core_e, nloc, srcb, nsrc_blk,
                  nsrc_rows, sbn):
    ap = AggPass(name, nloc, srcb, nsrc_blk, nsrc_rows, sbn)
    nsb = ap.nsb
    nruns = nsb * nsrc_blk

    per_core = []
    counts = np.zeros((NCORES, nruns), np.int64)
    for k in range(NCORES):
        m = core_e == k
        s, d, sc = src[m], dst_loc[m], scale_e[m]
        j = s // srcb
        sb = d // sbn
        order = np.lexsort((d, j, sb))
        s, d, sc, j, sb = s[order], d[order], sc[order], j[order], sb[order]
        rid = sb * nsrc_blk + j
        counts[k] = np.bincount(rid, minlength=nruns)
        per_core.append((s, d, sc, rid))

    run_L = _ru(counts.max(axis=0), P)          # uniform, 128-multiple
    offs = np.concatenate([[0], np.cumsum(run_L)]).astype(np.int64)
    etot = int(offs[-1])
    ap.run_L = run_L.reshape(nsb, nsrc_blk)
    ap.etot = etot

    # superblock-relative dst per core (for window structure)
    dstrel_all = np.full((NCORES, etot), -1.0e9, np.float64)
    pos_all = []
    for k in range(NCORES):
        s, d, sc, rid = per_core[k]
        run_start = np.concatenate([[0], np.cumsum(counts[k])])[:-1]
        pos = offs[rid] + (np.arange(len(s)) - run_start[rid])
        pos_all.append(pos)
        dstrel_all[k, pos] = (d - (d // sbn) * sbn).astype(np.float64)

    # per-tile window range (union over cores)
    T = etot // P
    Dw = dstrel_all.reshape(NCORES, T, P)
    valid_any = Dw.max(axis=2) >= 0              # [NCORES, T]
    wlo_c = np.where(Dw >= 0, Dw, np.inf).min(axis=2) // P
    whi_c = np.where(Dw >= 0, Dw, -np.inf).max(axis=2) // P
    wlo_t = np.where(valid_any, wlo_c, np.inf).min(axis=0)
    whi_t = np.where(valid_any, whi_c, -np.inf).max(axis=0)

    emits = []
    uncovered = []
    tile_wlo = np.zeros(T, np.int64)             # per global tile
    for s in range(nsb):
        nwin = -(-ap.sb_nodes[s] // P)
        covered = set()
        sb_emits = []
        for j in range(nsrc_blk):
            r = s * nsrc_blk + j
            o = int(offs[r])
            nt = int(ap.run_L[s, j] // P)
            for t in range(nt):
                g = o // P + t
                if np.isfinite(wlo_t[g]):
                    a = max(0, min(int(wlo_t[g]), nwin - 1))
                    b = max(a, min(int(whi_t[g]), nwin - 1))
                else:
                    a, b = 0, 0
                nw = b - a + 1
                assert nw <= MAXW, f"tile spans {nw} windows"
                covered.update(range(a, b + 1))
                tile_wlo[g] = a
                sb_emits.append((j, t, a, nw))
        emits.append(sb_emits)
        uncovered.append(sorted(set(range(nwin)) - covered))
    ap.emits = emits
    ap.uncovered = uncovered

    # per-core packed arrays (dst window-relative to tile_wlo)
    idxs, dscs = [], []
    for k in range(NCORES):
        s, d, sc, rid = per_core[k]
        pos = pos_all[k]
        idx16 = np.zeros(etot, np.int16)
        idx16[pos] = (s - (s // srcb) * srcb).astype(np.int16)
        dstrel = np.full(etot, -1000.0, np.float32)
        dstrel[pos] = (d - (d // sbn) * sbn).astype(np.float32)
        dstrel -= 128.0 * tile_wlo[np.arange(etot) // P]
        dstrel[dstrel < -1000.0] = -1000.0
        sclp = np.zeros(etot, np.float32)
        sclp[pos] = sc.astype(np.float32)

        wrapped = np.zeros((P, etot // 16), np.int16)
        dsc = np.zeros((P, 2 * (etot // P)), np.float16)
        for r in range(nruns):
            o = int(offs[r])
            L = int(run_L[r])
            if L == 0:
                continue
            wrapped[:, o // 16:(o + L) // 16] = _wrap_idx_chunk(idx16[o:o + L])
            nt = L // P
            c0 = 2 * (o // P)
            dsc[:, c0:c0 + nt] = _pack_pcol(dstrel[o:o + L]).astype(np.float16)
            dsc[:, c0 + nt:c0 + 2 * nt] = \
                _pack_pcol(sclp[o:o + L]).astype(np.float16)
        idxs.append(wrapped)
        dscs.append(dsc)
    ap.idx, ap.dsc = idxs, dscs
    return ap


# ---------------------------------------------------------------------------
# full host prep
# ---------------------------------------------------------------------------

def prep_all(cfg, inputs):
    i64 = lambda a: np.asarray(a).astype(np.int64)
    e_src = i64(inputs["edge_src_customer"])
    e_dst = i64(inputs["edge_dst_article"])
    l_c = i64(inputs["label_customer"])
    l_a = i64(inputs["label_article"])

    cnt_a = np.bincount(e_dst, minlength=cfg.n_a)
    cnt_c = np.bincount(e_src, minlength=cfg.n_c)
    scl_a = (1.0 / np.maximum(cnt_a, 1.0)).astype(np.float32)
    scl_c = (1.0 / np.maximum(cnt_c, 1.0)).astype(np.float32)

    pa = prep_agg_pass(
        "A", e_src, e_dst % cfg.apc, scl_a[e_dst], e_dst // cfg.apc,
        cfg.apc, cfg.srcb_c, cfg.nblk_c, cfg.n_c, cfg.sbn)
    dloc = e_src % cfg.cpc
    core_c = e_src // cfg.cpc
    half = (dloc >= cfg.chalf).astype(np.int64)
    pcs = []
    for h in range(2):
        m = half == h
        pcs.append(prep_agg_pass(
            f"C{h}", e_dst[m], dloc[m] - h * cfg.chalf, scl_c[e_src][m],
            core_c[m], cfg.chalf, cfg.srcb_a, cfg.nblk_a, cfg.n_a, cfg.sbn))

    # decoder labels: partition by customer core, group by (sub, ablk)
    core_l = l_c // cfg.cpc
    sub_l = (l_c % cfg.cpc) // cfg.zc_sub
    ablk_l = l_a // cfg.srcb_a
    gid = sub_l * cfg.nblk_a + ablk_l
    ngrp = 2 * cfg.nblk_a
    gcounts = np.zeros((NCORES, ngrp), np.int64)
    per_core_lbl = []
    for k in range(NCORES):
        m = core_l == k
        lc, la, g, orig = l_c[m], l_a[m], gid[m], np.nonzero(m)[0]
        order = np.argsort(g, kind="stable")
        lc, la, g, orig = lc[order], la[order], g[order], orig[order]
        gcounts[k] = np.bincount(g, minlength=ngrp)
        per_core_lbl.append((lc, la, g, orig))
    grp_L = _ru(gcounts.max(axis=0), P)
    goffs = np.concatenate([[0], np.cumsum(grp_L)]).astype(np.int64)
    ld_pad = int(goffs[-1])

    dec_idx_c, dec_idx_a, out_pos = [], [], []
    for k in range(NCORES):
        lc, la, g, orig = per_core_lbl[k]
        gstart = np.concatenate([[0], np.cumsum(gcounts[k])])[:-1]
        pos = goffs[g] + (np.arange(len(lc)) - gstart[g])
        ic = np.zeros(ld_pad, np.int16)
        ia = np.zeros(ld_pad, np.int16)
        po = np.full(ld_pad, -1, np.int64)
        lcl = lc % cfg.cpc
        ic[pos] = (lcl - (lcl // cfg.zc_sub) * cfg.zc_sub).astype(np.int16)
        ia[pos] = (la - (la // cfg.srcb_a) * cfg.srcb_a).astype(np.int16)
        po[pos] = orig
        wc = np.zeros((P, ld_pad // 16), np.int16)
        wa = np.zeros((P, ld_pad // 16), np.int16)
        for gi in range(ngrp):
            o = int(goffs[gi])
            L = int(grp_L[gi])
            if L:
                wc[:, o // 16:(o + L) // 16] = _wrap_idx_chunk(ic[o:o + L])
                wa[:, o // 16:(o + L) // 16] = _wrap_idx_chunk(ia[o:o + L])
        dec_idx_c.append(wc)
        dec_idx_a.append(wa)
        out_pos.append(po)

    dec = dict(grp_L=grp_L.reshape(2, cfg.nblk_a), goffs=goffs, ld_pad=ld_pad,
               idx_c=dec_idx_c, idx_a=dec_idx_a, out_pos=out_pos)
    return pa, pcs, dec


# ---------------------------------------------------------------------------
# kernel builder
# ---------------------------------------------------------------------------

F32 = mybir.dt.float32


def build_nc(cfg, pa, pcs, dec, dbg=False):
    DT = cfg.dt
    nc = bacc.Bacc("TRN2", target_bir_lowering=False, debug=False,
                   num_devices=NCORES, num_swdge_queues=4)
    qctr = [0]
    def next_q():
        qctr[0] = (qctr[0] + 1) % 4
        return qctr[0]

    ei = lambda n, s, d: nc.dram_tensor(n, s, d, kind="ExternalInput")
    xc = ei("xc", [cfg.n_c, P], DT)
    xa = ei("xa", [cfg.n_a, P], DT)
    xaT = ei("xaT", [P, cfg.apc], DT)
    xcT = ei("xcT", [P, cfg.cpc], DT)
    aggA_idx = ei("aggA_idx", [P, pa.etot // 16], mybir.dt.int16)
    aggA_dsc = ei("aggA_dsc", [P, 2 * pa.etot // P], DT)
    aggC_idx = [ei(f"aggC{h}_idx", [P, pcs[h].etot // 16], mybir.dt.int16)
                for h in range(2)]
    aggC_dsc = [ei(f"aggC{h}_dsc", [P, 2 * pcs[h].etot // P], DT)
                for h in range(2)]
    dec_idx_c = ei("dec_idx_c", [P, dec["ld_pad"] // 16], mybir.dt.int16)
    dec_idx_a = ei("dec_idx_a", [P, dec["ld_pad"] // 16], mybir.dt.int16)

    wnames = ["W_msg1_ca", "W_self1_a", "W_msg1_ac", "W_self1_c",
              "W_msg2_ca", "W_self2_a", "W_msg2_ac", "W_self2_c",
              "Wd1c", "Wd1a"]
    wts = {n: ei(n, [P, P], DT) for n in wnames}
    w2rep = ei("w2rep", [P, GCH], DT)       # W_dec2 repeated per 128-segment
    bnames = ["b1_a", "b1_c", "b2_a", "b2_c",
              "bn_gamma_c", "bn_beta_c", "bn_gamma_a", "bn_beta_a",
              "b_dec1", "b_dec2c"]
    bis = {n: ei(n, [P, 1], F32) for n in bnames}

    ldT = dec["ld_pad"] // P
    y_out = nc.dram_tensor("y", [P, ldT], F32, kind="ExternalOutput")
    dbg_outs = {}
    if dbg:
        dbg_outs = {
            "d_ha": nc.dram_tensor("d_ha", [cfg.n_a, P], DT,
                                   kind="ExternalOutput"),
            "d_hc": nc.dram_tensor("d_hc", [cfg.n_c, P], DT,
                                   kind="ExternalOutput"),
            "d_ua": nc.dram_tensor("d_ua", [cfg.n_a, P], DT,
                                   kind="ExternalOutput"),
            "d_uc": nc.dram_tensor("d_uc", [cfg.cpc, P], DT,
                                   kind="ExternalOutput"),
            "d_st": nc.dram_tensor("d_st", [P, 4], F32,
                                   kind="ExternalOutput"),
        }

    rg = [list(range(NCORES))]

    with tile.TileContext(nc) as tc:
        with (
            tc.tile_pool(name="dramp", bufs=1, space="DRAM") as dramp,
            tc.tile_pool(name="const", bufs=1) as cs,
        ):
            ha_own = dramp.tile([cfg.apc, P], DT)
            ha_full = dramp.tile([cfg.n_a, P], DT, addr_space="Shared")
            hc_own = dramp.tile([cfg.cpc, P], DT)
            hc_full = dramp.tile([cfg.n_c, P], DT, addr_space="Shared")
            ua_own = dramp.tile([cfg.apc, P], DT)
            ua_full = dramp.tile([cfg.n_a, P], DT, addr_space="Shared")
            uc_loc = dramp.tile([cfg.cpc, P], DT)
            haT_d = dramp.tile([P, cfg.apc], DT)
            hcT_d = dramp.tile([P, cfg.cpc], DT)
            zaT_d = dramp.tile([P, cfg.apc], DT)
            zcT_d = dramp.tile([P, cfg.cpc], DT)
            stats_in = dramp.tile([P, 4], F32)
            stats_out = dramp.tile([P, 4], F32, addr_space="Shared")

            # constants: batched iota tiles (values wi*128 + col%128)
            iota8 = []
            for wi in range(MAXW):
                ii = cs.tile([P, GCH], mybir.dt.int32, name=f"ioi{wi}")
                nc.gpsimd.iota(ii[:], pattern=[[0, TPC], [1, P]],
                               base=wi * P, channel_multiplier=0)
                ff = cs.tile([P, GCH], DT, name=f"iof{wi}")
                nc.vector.tensor_copy(ff[:], ii[:])
                iota8.append(ff)
            ident = cs.tile([P, P], DT)
            make_identity(nc, ident[:])
            w_sb = {n: cs.tile([P, P], DT, name=f"w_{n}") for n in wnames}
            for n in wnames:
                nc.sync.dma_start(out=w_sb[n][:], in_=wts[n][:])
            w2r_sb = cs.tile([P, GCH], DT)
            nc.sync.dma_start(out=w2r_sb[:], in_=w2rep[:])
            b_sb = {n: cs.tile([P, 1], F32, name=f"b_{n}") for n in bnames}
            for n in bnames:
                nc.sync.dma_start(out=b_sb[n][:], in_=bis[n][:])
            stats_sb = cs.tile([P, 4], F32)
            nc.vector.memset(stats_sb[:], 0.0)

            # ---------------- aggregation pass ----------------
            def agg_pass(ps, table, idx_d, dsc_d, meanT_sb):
                offs = np.concatenate(
                    [[0], np.cumsum(ps.run_L.reshape(-1))]).astype(np.int64)
                with (
                    tc.tile_pool(name=f"ag_{ps.name}", bufs=1) as sbp,
                    tc.tile_pool(name=f"agp_{ps.name}", bufs=2,
                                 space="PSUM") as psp,
                ):
                    for s in range(ps.nsb):
                        nodes = ps.sb_nodes[s]
                        nwin = -(-nodes // P)
                        psum = psp.tile([P, nwin * P], F32, tag="aggps",
                                        name="psum_agg", bufs=2)
                        # first/last touch per PSUM bank (4 windows/bank)
                        touch = {}
                        for (j, t, wlo, nw) in ps.emits[s]:
                            for wi in range(nw):
                                w = wlo + wi
                                touch.setdefault(w // 4, []).append((j, t, w))
                        firsts = {b: v[0] for b, v in touch.items()}
                        lasts = {b: v[-1] for b, v in touch.items()}

                        # group emits per run
                        by_run = {}
                        for e in ps.emits[s]:
                            by_run.setdefault(e[0], []).append(e)
                        for j in sorted(by_run):
                            r = s * ps.nsrc_blk + j
                            o = int(offs[r])
                            L = int(ps.run_L[s, j])
                            nt = L // P
                            blk_rows = min(ps.srcb,
                                           ps.nsrc_rows - j * ps.srcb)
                            idx_sb = sbp.tile([P, L // 16], mybir.dt.int16,
                                              tag="gidx", name="gidx",
                                              bufs=3)
                            nc.sync.dma_start(
                                out=idx_sb[:],
                                in_=idx_d[:, o // 16:(o + L) // 16])
                            dsc_sb = sbp.tile([P, 2 * nt], DT, tag="gdsc",
                                              name="gdsc", bufs=3)
                            nc.sync.dma_start(
                                out=dsc_sb[:],
                                in_=dsc_d[:, 2 * (o // P):2 * (o // P) + 2 * nt])
                            x_tiles = []
                            for c0 in range(0, L, GCH):
                                cl = min(GCH, L - c0)
                                x = sbp.tile([P, TPC, P], DT, tag="gx",
                                             name="gx", bufs=10)
                                nc.gpsimd.dma_gather(
                                    x[:, :cl // P, :],
                                    table[j * ps.srcb:
                                          j * ps.srcb + blk_rows, :],
                                    idx_sb[:, c0 // 16:(c0 + cl) // 16],
                                    cl, cl, P, queue_num=next_q())
                                x_tiles.append(x)
                            # per gather-chunk batched P builds
                            run_emits = by_run[j]
                            p8s = {}     # (chunk, wi) -> tile
                            for c in range(0, nt, TPC):
                                ctn = min(TPC, nt - c)
                                maxnw = max(e[3] for e in run_emits
                                            if c <= e[1] < c + ctn)
                                dstb = dsc_sb[:, c:c + ctn] \
                                    .to_broadcast([P, ctn, P])
                                sclb = dsc_sb[:, nt + c:nt + c + ctn] \
                                    .to_broadcast([P, ctn, P])
                                for wi in range(maxnw):
                                    p8 = sbp.tile([P, GCH], DT, tag="gp",
                                                  name="gp", bufs=6)
                                    p83 = p8[:, :ctn * P].rearrange(
                                        "p (t w) -> p t w", w=P)
                                    nc.vector.tensor_tensor(
                                        out=p83,
                                        in0=iota8[wi][:, :ctn * P].rearrange(
                                            "p (t w) -> p t w", w=P),
                                        in1=dstb,
                                        op=mybir.AluOpType.is_equal)
                                    nc.vector.tensor_tensor(
                                        out=p83, in0=p83, in1=sclb,
                                        op=mybir.AluOpType.mult)
                                    p8s[(c // TPC, wi)] = p8
                            for (j2, t, wlo, nw) in run_emits:
                                for wi in range(nw):
                                    w = wlo + wi
                                    p8 = p8s[(t // TPC, wi)]
                                    nc.tensor.matmul(
                                        psum[:, w * P:(w + 1) * P],
                                        lhsT=x_tiles[t // TPC]
                                            [:, t % TPC, :],
                                        rhs=p8[:, (t % TPC) * P:
                                               (t % TPC + 1) * P],
                                        start=(firsts[w // 4] == (j2, t, w)),
                                        stop=(lasts[w // 4] == (j2, t, w)),
                                        skip_group_check=True)
                        nc.vector.tensor_copy(
                            meanT_sb[:, s * ps.sbn:s * ps.sbn + nodes],
                            psum[:, :nodes])
                        for w in ps.uncovered[s]:
                            a = s * ps.sbn + w * P
                            b = min(a + P, s * ps.sbn + nodes)
                            nc.vector.memset(meanT_sb[:, a:b], 0.0)

            # ---------------- W stage ----------------
            def w_stage(nloc, meanT_sb, selfT_dram, self_off, wmsg, wself,
                        bias_col, relu, outT_dram, outT_off, rows_dram,
                        rows_off, stats_cols, sbp, psp):
                for c0 in range(0, nloc, WCH):
                    cw = min(WCH, nloc - c0)
                    sT = sbp.tile([P, WCH], DT, tag="wself", name="wselfT",
                                  bufs=3)
                    nc.sync.dma_start(
                        out=sT[:, :cw],
                        in_=selfT_dram[:, self_off + c0:self_off + c0 + cw])
                    psum = psp.tile([P, WCH], F32, tag="wps", name="wps",
                                    bufs=3)
                    nc.tensor.matmul(psum[:, :cw], lhsT=wmsg,
                                     rhs=meanT_sb[:, c0:c0 + cw],
                                     start=True, stop=False,
                                     skip_group_check=True)
                    nc.tensor.matmul(psum[:, :cw], lhsT=wself,
                                     rhs=sT[:, :cw],
                                     start=False, stop=True,
                                     skip_group_check=True)
                    oT = sbp.tile([P, WCH], DT, tag="woT", name="woT", bufs=3)
                    nc.scalar.activation(
                        oT[:, :cw], psum[:, :cw],
                        mybir.ActivationFunctionType.Relu if relu
                        else mybir.ActivationFunctionType.Identity,
                        bias=bias_col[:], scale=1.0)
                    nc.sync.dma_start(
                        out=outT_dram[:, outT_off + c0:outT_off + c0 + cw],
                        in_=oT[:, :cw])
                    if stats_cols is not None:
                        si, sj = stats_cols
                        part = sbp.tile([P, 1], F32, tag="wst1", name="wst1",
                                        bufs=2)
                        nc.vector.reduce_sum(part[:], oT[:, :cw],
                                             mybir.AxisListType.X)
                        nc.vector.tensor_add(stats_sb[:, si:si + 1],
                                             stats_sb[:, si:si + 1], part[:])
                        trash = sbp.tile([P, WCH], F32, tag="wtrash",
                                         name="wtrash", bufs=2)
                        part2 = sbp.tile([P, 1], F32, tag="wst2", name="wst2",
                                         bufs=2)
                        nc.scalar.activation(
                            trash[:, :cw], oT[:, :cw],
                            mybir.ActivationFunctionType.Square,
                            accum_out=part2[:])
                        nc.vector.tensor_add(stats_sb[:, sj:sj + 1],
                                             stats_sb[:, sj:sj + 1],
                                             part2[:])
                    if rows_dram is not None:
                        _emit_rows(oT, cw, rows_dram, rows_off + c0, sbp, psp)

            def _emit_rows(srcT_sb, cw, rows_dram, row_base, sbp, psp):
                for b0 in range(0, cw, P):
                    bw = min(P, cw - b0)
                    tp = psp.tile([P, P], DT, tag="tps", name="tps", bufs=2)
                    nc.tensor.transpose(tp[:bw, :], srcT_sb[:, b0:b0 + bw],
                                        ident[:])
                    rows = sbp.tile([P, P], DT, tag="rows", name="rows",
                                    bufs=3)
                    nc.scalar.copy(rows[:bw, :], tp[:bw, :])
                    nc.sync.dma_start(
                        out=rows_dram[row_base + b0:row_base + b0 + bw, :],
                        in_=rows[:bw, :])

            # ================= layer 1 =================
            with tc.tile_pool(name="meanA", bufs=1) as mp:
                meanT = mp.tile([P, pa.nsb * pa.sbn], DT, name="meanTA")
                agg_pass(pa, xc, aggA_idx, aggA_dsc, meanT)
                with (
                    tc.tile_pool(name="w1a", bufs=1) as sbp,
                    tc.tile_pool(name="w1ap", bufs=1, space="PSUM") as psp,
                ):
                    w_stage(cfg.apc, meanT, xaT, 0, w_sb["W_msg1_ca"][:],
                            w_sb["W_self1_a"][:], b_sb["b1_a"], True,
                            haT_d, 0, ha_own, 0, None, sbp, psp)
            nc.gpsimd.collective_compute(
                "AllGather", mybir.AluOpType.bypass, replica_groups=rg,
                ins=[ha_own[:]], outs=[ha_full[:]])

            for h in range(2):
                with tc.tile_pool(name=f"meanC{h}", bufs=1) as mp:
                    meanT = mp.tile([P, pcs[h].nsb * pcs[h].sbn], DT,
                                    name="meanTC")
                    agg_pass(pcs[h], xa, aggC_idx[h], aggC_dsc[h], meanT)
                    with (
                        tc.tile_pool(name=f"w1c{h}", bufs=1) as sbp,
                        tc.tile_pool(name=f"w1cp{h}", bufs=1,
                                     space="PSUM") as psp,
                    ):
                        w_stage(cfg.chalf, meanT, xcT, h * cfg.chalf,
                                w_sb["W_msg1_ac"][:], w_sb["W_self1_c"][:],
                                b_sb["b1_c"], True, hcT_d, h * cfg.chalf,
                                hc_own, h * cfg.chalf, None, sbp, psp)
            nc.gpsimd.collective_compute(
                "AllGather", mybir.AluOpType.bypass, replica_groups=rg,
                ins=[hc_own[:]], outs=[hc_full[:]])

            # ================= layer 2 =================
            with tc.tile_pool(name="meanA2", bufs=1) as mp:
                meanT = mp.tile([P, pa.nsb * pa.sbn], DT, name="meanTA2")
                agg_pass(pa, hc_full, aggA_idx, aggA_dsc, meanT)
                with (
                    tc.tile_pool(name="w2a", bufs=1) as sbp,
                    tc.tile_pool(name="w2ap", bufs=1, space="PSUM") as psp,
                ):
                    w_stage(cfg.apc, meanT, haT_d, 0, w_sb["W_msg2_ca"][:],
                            w_sb["W_self2_a"][:], b_sb["b2_a"], False,
                            zaT_d, 0, None, 0, (0, 1), sbp, psp)
            for h in range(2):
                with tc.tile_pool(name=f"meanC2{h}", bufs=1) as mp:
                    meanT = mp.tile([P, pcs[h].nsb * pcs[h].sbn], DT,
                                    name="meanTC2")
                    agg_pass(pcs[h], ha_full, aggC_idx[h], aggC_dsc[h],
                             meanT)
                    with (
                        tc.tile_pool(name=f"w2c{h}", bufs=1) as sbp,
                        tc.tile_pool(name=f"w2cp{h}", bufs=1,
                                     space="PSUM") as psp,
                    ):
                        w_stage(cfg.chalf, meanT, hcT_d, h * cfg.chalf,
                                w_sb["W_msg2_ac"][:], w_sb["W_self2_c"][:],
                                b_sb["b2_c"], False, zcT_d, h * cfg.chalf,
                                None, 0, (2, 3), sbp, psp)

            # ================= BN + U tables =================
            with (
                tc.tile_pool(name="bn", bufs=1) as sbp,
                tc.tile_pool(name="bnp", bufs=1, space="PSUM") as psp,
            ):
                nc.sync.dma_start(out=stats_in[:], in_=stats_sb[:])
                nc.gpsimd.collective_compute(
                    "AllReduce", mybir.AluOpType.add, replica_groups=rg,
                    ins=[stats_in[:]], outs=[stats_out[:]])
                st = sbp.tile([P, 4], F32)
                nc.sync.dma_start(out=st[:], in_=stats_out[:])

                def bn_coeff(si, sj, n, gamma, beta, tagp):
                    mu = sbp.tile([P, 1], F32, name=f"mu{tagp}")
                    nc.vector.tensor_scalar_mul(mu[:], st[:, si:si + 1],
                                                1.0 / n)
                    msq = sbp.tile([P, 1], F32, name=f"msq{tagp}")
                    nc.vector.tensor_scalar_mul(msq[:], st[:, sj:sj + 1],
                                                1.0 / n)
                    mu2 = sbp.tile([P, 1], F32, name=f"mu2{tagp}")
                    nc.vector.tensor_mul(mu2[:], mu[:], mu[:])
                    var = sbp.tile([P, 1], F32, name=f"var{tagp}")
                    nc.vector.tensor_sub(var[:], msq[:], mu2[:])
                    nc.vector.tensor_scalar_add(var[:], var[:], BN_EPS)
                    sd = sbp.tile([P, 1], F32, name=f"sd{tagp}")
                    nc.scalar.activation(sd[:], var[:],
                                         mybir.ActivationFunctionType.Sqrt)
                    rstd = sbp.tile([P, 1], F32, name=f"rstd{tagp}")
                    nc.vector.reciprocal(rstd[:], sd[:])
                    scl = sbp.tile([P, 1], F32, name=f"scl{tagp}")
                    nc.vector.tensor_mul(scl[:], b_sb[gamma][:], rstd[:])
                    mg = sbp.tile([P, 1], F32, name=f"mg{tagp}")
                    nc.vector.tensor_mul(mg[:], mu[:], scl[:])
                    bia = sbp.tile([P, 1], F32, name=f"bia{tagp}")
                    nc.vector.tensor_sub(bia[:], b_sb[beta][:], mg[:])
                    return scl, bia

                scl_a_c, bia_a_c = bn_coeff(0, 1, cfg.n_a, "bn_gamma_a",
                                            "bn_beta_a", "a")
                scl_c_c, bia_c_c = bn_coeff(2, 3, cfg.n_c, "bn_gamma_c",
                                            "bn_beta_c", "c")

                def bn_u(nloc, zT_dram, scl, bia, w1half, ubias, rows_dram):
                    """rows_dram <- rows of bn(z) @ w1half (+ubias)."""
                    for c0 in range(0, nloc, WCH):
                        cw = min(WCH, nloc - c0)
                        zT = sbp.tile([P, WCH], DT, tag="bnz", name="bnz",
                                      bufs=3)
                        nc.sync.dma_start(out=zT[:, :cw],
                                          in_=zT_dram[:, c0:c0 + cw])
                        bnT = sbp.tile([P, WCH], DT, tag="bnt", name="bnt",
                                       bufs=3)
                        nc.scalar.activation(
                            bnT[:, :cw], zT[:, :cw],
                            mybir.ActivationFunctionType.Identity,
                            bias=bia[:], scale=scl[:])
                        ups = psp.tile([P, WCH], F32, tag="ups", name="ups",
                                       bufs=2)
                        nc.tensor.matmul(ups[:, :cw], lhsT=w1half,
                                         rhs=bnT[:, :cw], start=True,
                                         stop=True, skip_group_check=True)
                        uT = sbp.tile([P, WCH], DT, tag="uT", name="uT",
                                      bufs=3)
                        nc.scalar.activation(
                            uT[:, :cw], ups[:, :cw],
                            mybir.ActivationFunctionType.Identity,
                            bias=ubias[:] if ubias is not None else 0.0,
                            scale=1.0)
                        _emit_rows(uT, cw, rows_dram, c0, sbp, psp)

                bn_u(cfg.apc, zaT_d, scl_a_c, bia_a_c, w_sb["Wd1a"][:],
                     None, ua_own)
                bn_u(cfg.cpc, zcT_d, scl_c_c, bia_c_c, w_sb["Wd1c"][:],
                     b_sb["b_dec1"], uc_loc)
            nc.gpsimd.collective_compute(
                "AllGather", mybir.AluOpType.bypass, replica_groups=rg,
                ins=[ua_own[:]], outs=[ua_full[:]])

            # ================= decoder =================
            grp_L = dec["grp_L"]
            goffs = dec["goffs"]
            with tc.tile_pool(name="dec", bufs=1) as sbp:
                ysb = sbp.tile([P, ldT], F32, name="ysb")
                for sub in range(2):
                    for ab in range(cfg.nblk_a):
                        gi = sub * cfg.nblk_a + ab
                        L = int(grp_L[sub, ab])
                        o = int(goffs[gi])
                        uc_rows = min(cfg.zc_sub, cfg.cpc - sub * cfg.zc_sub)
                        ua_rows = min(cfg.srcb_a, cfg.n_a - ab * cfg.srcb_a)
                        for c0 in range(0, L, GCH):
                            cl = min(GCH, L - c0)
                            oc = o + c0
                            ctn = cl // P
                            ixc = sbp.tile([P, GCH // 16], mybir.dt.int16,
                                           tag="dixc", name="dixc", bufs=4)
                            nc.sync.dma_start(
                                out=ixc[:, :cl // 16],
                                in_=dec_idx_c[:, oc // 16:(oc + cl) // 16])
                            ucg = sbp.tile([P, TPC, P], DT, tag="duc",
                                           name="duc", bufs=4)
                            nc.gpsimd.dma_gather(
                                ucg[:, :ctn, :],
                                uc_loc[sub * cfg.zc_sub:
                                       sub * cfg.zc_sub + uc_rows, :],
                                ixc[:, :cl // 16], cl, cl, P,
                                queue_num=next_q())
                            ixa = sbp.tile([P, GCH // 16], mybir.dt.int16,
                                           tag="dixa", name="dixa", bufs=4)
                            nc.sync.dma_start(
                                out=ixa[:, :cl // 16],
                                in_=dec_idx_a[:, oc // 16:(oc + cl) // 16])
                            uag = sbp.tile([P, TPC, P], DT, tag="dua",
                                           name="dua", bufs=4)
                            nc.gpsimd.dma_gather(
                                uag[:, :ctn, :],
                                ua_full[ab * cfg.srcb_a:
                                        ab * cfg.srcb_a + ua_rows, :],
                                ixa[:, :cl // 16], cl, cl, P,
                                queue_num=next_q())
                            ssum = sbp.tile([P, GCH], DT, tag="dsum",
                                            name="dsum", bufs=4)
                            ssum3 = ssum[:, :cl].rearrange(
                                "p (t w) -> p t w", w=P)
                            nc.vector.tensor_tensor(
                                out=ssum3,
                                in0=ucg[:, :ctn, :], in1=uag[:, :ctn, :],
                                op=mybir.AluOpType.add)
                            nc.vector.tensor_scalar(
                                out=ssum[:, :cl], in0=ssum[:, :cl],
                                scalar1=0.0, scalar2=None,
                                op0=mybir.AluOpType.max)
                            nc.vector.tensor_mul(
                                ssum[:, :cl], ssum[:, :cl], w2r_sb[:, :cl])
                            nc.vector.reduce_sum(
                                ysb[:, oc // P:oc // P + ctn],
                                ssum[:, :cl].rearrange(
                                    "p (t w) -> p t w", w=P),
                                mybir.AxisListType.X)
                nc.vector.tensor_scalar(
                    out=ysb[:], in0=ysb[:], scalar1=b_sb["b_dec2c"][:],
                    scalar2=None, op0=mybir.AluOpType.add)
                nc.sync.dma_start(out=y_out[:], in_=ysb[:])

            if dbg:
                nc.sync.dma_start(out=dbg_outs["d_ha"][:], in_=ha_full[:])
                nc.sync.dma_start(out=dbg_outs["d_hc"][:], in_=hc_full[:])
                nc.sync.dma_start(out=dbg_outs["d_ua"][:], in_=ua_full[:])
                nc.sync.dma_start(out=dbg_outs["d_uc"][:], in_=uc_loc[:])
                nc.sync.dma_start(out=dbg_outs["d_st"][:], in_=stats_out[:])

    nc.compile()
    return nc


# ---------------------------------------------------------------------------
# entry point
# ---------------------------------------------------------------------------

def make_in_maps(cfg, inputs, pa, pcs, dec):
    npdt = cfg.npdt
    f = lambda a: np.ascontiguousarray(np.asarray(a), dtype=np.float32)
    xc16 = f(inputs["x_customer"]).astype(npdt)
    xa16 = f(inputs["x_article"]).astype(npdt)
    wd1 = f(inputs["W_dec1"])
    w2 = f(inputs["W_dec2"]).reshape(-1)
    base = dict(
        xc=xc16, xa=xa16,
        W_msg1_ca=f(inputs["W_msg1_ca"]).astype(npdt),
        W_self1_a=f(inputs["W_self1_a"]).astype(npdt),
        W_msg1_ac=f(inputs["W_msg1_ac"]).astype(npdt),
        W_self1_c=f(inputs["W_self1_c"]).astype(npdt),
        W_msg2_ca=f(inputs["W_msg2_ca"]).astype(npdt),
        W_self2_a=f(inputs["W_self2_a"]).astype(npdt),
        W_msg2_ac=f(inputs["W_msg2_ac"]).astype(npdt),
        W_self2_c=f(inputs["W_self2_c"]).astype(npdt),
        Wd1c=wd1[:P].astype(npdt), Wd1a=wd1[P:].astype(npdt),
        w2rep=np.tile(w2.astype(npdt).reshape(1, P), (P, GCH // P)),
        b1_a=f(inputs["b1_a"]).reshape(P, 1),
        b1_c=f(inputs["b1_c"]).reshape(P, 1),
        b2_a=f(inputs["b2_a"]).reshape(P, 1),
        b2_c=f(inputs["b2_c"]).reshape(P, 1),
        bn_gamma_c=f(inputs["bn_gamma_c"]).reshape(P, 1),
        bn_beta_c=f(inputs["bn_beta_c"]).reshape(P, 1),
        bn_gamma_a=f(inputs["bn_gamma_a"]).reshape(P, 1),
        bn_beta_a=f(inputs["bn_beta_a"]).reshape(P, 1),
        b_dec1=f(inputs["b_dec1"]).reshape(P, 1),
        b_dec2c=np.full((P, 1), float(np.asarray(inputs["b_dec2"]).item()),
                        np.float32),
    )
    in_maps = []
    for k in range(NCORES):
        m = dict(base)
        m["xaT"] = np.ascontiguousarray(
            xa16[k * cfg.apc:(k + 1) * cfg.apc].T)
        m["xcT"] = np.ascontiguousarray(
            xc16[k * cfg.cpc:(k + 1) * cfg.cpc].T)
        m["aggA_idx"] = pa.idx[k]
        m["aggA_dsc"] = pa.dsc[k]
        for h in range(2):
            m[f"aggC{h}_idx"] = pcs[h].idx[k]
            m[f"aggC{h}_dsc"] = pcs[h].dsc[k]
        m["dec_idx_c"] = dec["idx_c"][k]
        m["dec_idx_a"] = dec["idx_a"][k]
        in_maps.append(m)
    return in_maps


def run(cfg, inputs, trace=False, dbg=False):
    pa, pcs, dec = prep_all(cfg, inputs)
    in_maps = make_in_maps(cfg, inputs, pa, pcs, dec)
    nc = build_nc(cfg, pa, pcs, dec, dbg=dbg)
    res = run_bass_kernel_spmd(nc, in_maps, core_ids=list(range(NCORES)),
                               trace=trace)
    y = np.empty(cfg.e_lbl, np.float32)
    for k in range(NCORES):
        yl = res.results[k]["y"].T.reshape(-1)
        po = dec["out_pos"][k]
        vm = po >= 0
        y[po[vm]] = yl[vm]
    return y, res


def kernel(**inputs):
    cfg = Cfg()
    y, _ = run(cfg, inputs, trace=False)
    return y



# revision 2
# speedup vs baseline: 1.1896x; 1.1896x over previous
"""Hetero GNN encoder/decoder v2 - restructured to minimize SWDGE descgen + DVE.

Key changes vs v1:
  - Pass order: A1 (dst=article-owner) -> AllGather comb[x_a|h_a] ->
    C-fused (C1+C2 share one 512B gather + one-hot P) -> A2
    (src=customer-owner, gathers LOCAL h_c rows, partial agg over all
    articles) -> ReduceScatter -> z_a -> AllGather z_a rows -> decoder.
  - One-hot P = is_equal only; 1/cnt scale applied at psum->SBUF copy
    via host-replicated per-column scale tables.
  - Decoder: transpose-gather of z_a (column-major), BN via per-partition
    scalar activation, Wd1a matmul + U_c window one-hot matmuls accumulate
    into one PSUM, w2 reduction via M=1 matmul.  No hc/ua AllGathers.
"""
import sys

sys.path.insert(0, "/opt/trn_rl_repo")

import numpy as np

import concourse.bacc as bacc
import concourse.bass as bass
import concourse.mybir as mybir
import concourse.tile as tile
from concourse.bass_utils import run_bass_kernel_spmd
from concourse.masks import make_identity

P = 128
NCORES = 8
MAXW = 4
MAXW_DEC = 8
BN_EPS = 1e-5
F32 = mybir.dt.float32
F16 = mybir.dt.float16
I16 = mybir.dt.int16


class Cfg:
    def __init__(self, small=False):
        if small:
            self.n_c, self.n_a, self.e_lbl = 6144, 1024, 8192
            self.sbn, self.sbn_cf = 128, 128
            self.srcb_c, self.srcb_a, self.srcb_h = 1024, 256, 384
            self.gch_a1, self.gch_cf, self.dch = 512, 256, 256
        else:
            self.n_c, self.n_a, self.e_lbl = 300000, 100000, 1000000
            self.sbn = 1250                 # superblock nodes (divides 12500)
            self.sbn_cf = 1024              # Cf superblock (2 psums, bank fit)
            self.srcb_c = 30000             # A1 src block (customers)
            self.srcb_a = 25000             # Cf src block (articles)
            self.srcb_h = 18750             # A2 src block (local customers)
            self.gch_a1 = 1024              # idx per gather, A1/A2 (256B rows)
            self.gch_cf = 1024              # idx per gather, Cf (512B rows)
            self.dch = 1024                 # decoder labels per chunk
        self.cpc, self.apc = self.n_c // NCORES, self.n_a // NCORES
        self.chalf = self.cpc // 2
        self.nblk_c = -(-self.n_c // self.srcb_c)
        self.nblk_a = -(-self.n_a // self.srcb_a)
        self.nblk_h = -(-self.cpc // self.srcb_h)
        self.nab = self.nblk_a
        assert self.apc % self.sbn == 0
        self.sb_per_blk = self.apc // self.sbn
        self.nwin_uc = -(-self.cpc // P)
        self.cpc_pad = self.nwin_uc * P


def _ru(x, m):
    return (x + m - 1) // m * m


def _wrap_idx(flat):
    """[n] int -> [128, n/16] wrap (16-partition layout, replicated x8)."""
    n = flat.shape[0]
    w = flat.astype(np.int16).reshape(n // 16, 16).T
    return np.tile(w, (8, 1))


def _pack_pcol(a):
    """[n] -> [128, n/128]: element i -> partition i%128, col i//128."""
    return np.ascontiguousarray(a.reshape(-1, P).T)


# ---------------------------------------------------------------------------
# host prep: one aggregation pass (uniform SPMD structure across cores)
# ---------------------------------------------------------------------------

class AggPass:
    def __init__(self, name, nloc, srcb, nblk, nsrc_rows, sbn):
        self.name, self.nloc, self.srcb = name, nloc, srcb
        self.nblk, self.nsrc_rows, self.sbn = nblk, nsrc_rows, sbn
        self.nsb = -(-nloc // sbn)
        self.sb_nodes = [min(sbn, nloc - s * sbn) for s in range(self.nsb)]


def prep_agg_pass(name, src, dst_loc, core_e, nloc, srcb, nblk, nsrc_rows, sbn):
    """Edges (src gathered, dst accumulated into nloc-range) per core.

    Returns AggPass with: run_L [nsb,nblk] uniform padded lengths, emits
    (per sb: list of (j, t, wlo, nw)), uncovered windows, per-core idx
    (wrapped int16) and dsc (dst-rel fp16, [128, etot/128])."""
    ap = AggPass(name, nloc, srcb, nblk, nsrc_rows, sbn)
    nsb = ap.nsb
    nruns = nsb * nblk

    per_core, counts = [], np.zeros((NCORES, nruns), np.int64)
    for k in range(NCORES):
        m = core_e == k
        s, d = src[m], dst_loc[m]
        j = s // srcb
        sb = d // sbn
        order = np.lexsort((d, j, sb))
        s, d, j, sb = s[order], d[order], j[order], sb[order]
        rid = sb * nblk + j
        counts[k] = np.bincount(rid, minlength=nruns)
        per_core.append((s, d, rid))

    run_L = _ru(counts.max(axis=0), P)
    offs = np.concatenate([[0], np.cumsum(run_L)]).astype(np.int64)
    etot = int(offs[-1])
    ap.run_L = run_L.reshape(nsb, nblk)
    ap.etot = etot

    dstrel_all = np.full((NCORES, etot), -1.0e9, np.float64)
    pos_all = []
    for k in range(NCORES):
        s, d, rid = per_core[k]
        run_start = np.concatenate([[0], np.cumsum(counts[k])])[:-1]
        pos = offs[rid] + (np.arange(len(s)) - run_start[rid])
        pos_all.append(pos)
        dstrel_all[k, pos] = (d - (d // sbn) * sbn).astype(np.float64)

    T = etot // P
    Dw = dstrel_all.reshape(NCORES, T, P)
    valid_any = Dw.max(axis=2) >= 0
    with np.errstate(invalid="ignore"):
        wlo_c = np.where(Dw >= 0, Dw, np.inf).min(axis=2) // P
        whi_c = np.where(Dw >= 0, Dw, -np.inf).max(axis=2) // P
    wlo_t = np.where(valid_any, wlo_c, np.inf).min(axis=0)
    whi_t = np.where(valid_any, whi_c, -np.inf).max(axis=0)

    emits, uncovered = [], []
    tile_wlo = np.zeros(T, np.int64)
    for s in range(nsb):
        nwin = -(-ap.sb_nodes[s] // P)
        covered, sb_emits = set(), []
        for j in range(nblk):
            r = s * nblk + j
            o = int(offs[r])
            nt = int(ap.run_L[s, j] // P)
            for t in range(nt):
                g = o // P + t
                if np.isfinite(wlo_t[g]):
                    a = max(0, min(int(wlo_t[g]), nwin - 1))
                    b = max(a, min(int(whi_t[g]), nwin - 1))
                else:
                    a, b = 0, 0
                nw = b - a + 1
                assert nw <= MAXW, f"{name}: tile spans {nw} windows"
                covered.update(range(a, b + 1))
                tile_wlo[g] = a
                sb_emits.append((j, t, a, nw))
        emits.append(sb_emits)
        uncovered.append(sorted(set(range(nwin)) - covered))
    ap.emits, ap.uncovered = emits, uncovered

    idxs, dscs = [], []
    for k in range(NCORES):
        s, d, rid = per_core[k]
        pos = pos_all[k]
        idx16 = np.zeros(etot, np.int16)
        idx16[pos] = (s - (s // srcb) * srcb).astype(np.int16)
        dstrel = np.full(etot, -1000.0, np.float32)
        dstrel[pos] = (d - (d // sbn) * sbn).astype(np.float32)
        dstrel -= 128.0 * tile_wlo[np.arange(etot) // P]
        dstrel[dstrel < -1000.0] = -1000.0
        idxs.append(_wrap_idx(idx16))
        dscs.append(_pack_pcol(dstrel).astype(np.float16))
    ap.idx, ap.dsc = idxs, dscs
    return ap


# ---------------------------------------------------------------------------
# host prep: decoder labels
# ---------------------------------------------------------------------------

def prep_decoder(cfg, l_c, l_a):
    """Labels partitioned by customer owner; per article-block (4 of 25000),
    sorted by customer.  Chunked into dch with uniform per-(core,ablk)
    padding.  Q emissions use tile_wlo + MAXW window-relative encoding."""
    nab = cfg.nab
    core_l = l_c // cfg.cpc
    ablk = l_a // cfg.srcb_a
    gcounts = np.zeros((NCORES, nab), np.int64)
    per_core = []
    for k in range(NCORES):
        m = core_l == k
        lc, la, ab, orig = l_c[m], l_a[m], ablk[m], np.nonzero(m)[0]
        order = np.lexsort((lc, ab))
        lc, la, ab, orig = lc[order], la[order], ab[order], orig[order]
        gcounts[k] = np.bincount(ab, minlength=nab)
        per_core.append((lc, la, ab, orig))
    grp_L = _ru(gcounts.max(axis=0), cfg.dch)
    goffs = np.concatenate([[0], np.cumsum(grp_L)]).astype(np.int64)
    ld_pad = int(goffs[-1])

    # window-relative structure (union over cores)
    win_all = np.full((NCORES, ld_pad), -1, np.int64)
    idx_a_list, crel_list, out_pos = [], [], []
    pos_all = []
    for k in range(NCORES):
        lc, la, ab, orig = per_core[k]
        gstart = np.concatenate([[0], np.cumsum(gcounts[k])])[:-1]
        pos = goffs[ab] + (np.arange(len(lc)) - gstart[ab])
        pos_all.append(pos)
        win_all[k, pos] = (lc % cfg.cpc) // P

    T = ld_pad // P
    Ww = win_all.reshape(NCORES, T, P)
    valid_any = Ww.max(axis=2) >= 0
    wlo_c = np.where(Ww >= 0, Ww, np.inf).min(axis=2)
    whi_c = np.where(Ww >= 0, Ww, -np.inf).max(axis=2)
    wlo_t = np.where(valid_any, wlo_c, np.inf).min(axis=0)
    whi_t = np.where(valid_any, whi_c, -np.inf).max(axis=0)
    tile_wlo = np.zeros(T, np.int64)
    tile_nw = np.ones(T, np.int64)
    for t in range(T):
        if np.isfinite(wlo_t[t]):
            a = min(int(wlo_t[t]), cfg.nwin_uc - 1)
            b = min(int(whi_t[t]), cfg.nwin_uc - 1)
            nw = b - a + 1
            assert nw <= MAXW_DEC, f"dec tile spans {nw} windows"
            tile_wlo[t], tile_nw[t] = a, nw

    for k in range(NCORES):
        lc, la, ab, orig = per_core[k]
        pos = pos_all[k]
        ia = np.zeros(ld_pad, np.int16)
        ia[pos] = (la - (la // cfg.srcb_a) * cfg.srcb_a).astype(np.int16)
        crel = np.full(ld_pad, -1000.0, np.float32)
        crel[pos] = ((lc % cfg.cpc) - tile_wlo[pos // P] * P).astype(np.float32)
        po = np.full(ld_pad, -1, np.int64)
        po[pos] = orig
        idx_a_list.append(_wrap_idx(ia))
        # replicated across partitions: [128, ld_pad] fp16
        crel_list.append(np.tile(crel.astype(np.float16)[None, :], (P, 1)))
        out_pos.append(po)

    return dict(grp_L=grp_L, goffs=goffs, ld_pad=ld_pad, tile_wlo=tile_wlo,
                tile_nw=tile_nw, idx_a=idx_a_list, crel=crel_list,
                out_pos=out_pos)


def prep_all(cfg, inputs):
    i64 = lambda a: np.asarray(a).astype(np.int64)
    e_src = i64(inputs["edge_src_customer"])
    e_dst = i64(inputs["edge_dst_article"])
    l_c = i64(inputs["label_customer"])
    l_a = i64(inputs["label_article"])

    cnt_a = np.bincount(e_dst, minlength=cfg.n_a)
    cnt_c = np.bincount(e_src, minlength=cfg.n_c)
    scl_a = (1.0 / np.maximum(cnt_a, 1.0)).astype(np.float32)
    scl_c = (1.0 / np.maximum(cnt_c, 1.0)).astype(np.float32)

    # A1: dst = local articles (owner core), src = global customers
    pa1 = prep_agg_pass("A1", e_src, e_dst % cfg.apc, e_dst // cfg.apc,
                        cfg.apc, cfg.srcb_c, cfg.nblk_c, cfg.n_c, cfg.sbn)
    # Cf halves: dst = local customers (owner core), src = global articles
    dloc = e_src % cfg.cpc
    core_c = e_src // cfg.cpc
    half = (dloc >= cfg.chalf).astype(np.int64)
    pcf = []
    for h in range(2):
        m = half == h
        pcf.append(prep_agg_pass(f"Cf{h}", e_dst[m], dloc[m] - h * cfg.chalf,
                                 core_c[m], cfg.chalf, cfg.srcb_a, cfg.nblk_a,
                                 cfg.n_a, cfg.sbn_cf))
    # A2: src = local customers (owner core), dst = ALL articles
    pa2 = prep_agg_pass("A2", e_src % cfg.cpc, e_dst, core_c,
                        cfg.n_a, cfg.srcb_h, cfg.nblk_h, cfg.cpc, cfg.sbn)
    dec = prep_decoder(cfg, l_c, l_a)
    return pa1, pcf, pa2, dec, scl_a, scl_c


# ---------------------------------------------------------------------------
# kernel builder
# ---------------------------------------------------------------------------

def build_nc(cfg, pa1, pcf, pa2, dec, dbg=False):
    import os
    stage = int(os.environ.get("K2_STAGE", "5"))
    nc = bacc.Bacc("TRN2", target_bir_lowering=False, debug=False,
                   num_devices=NCORES, num_swdge_queues=4)
    qctr = [0]
    def next_q():
        qctr[0] = (qctr[0] + 1) % 4
        return qctr[0]

    ei = lambda n, s, d: nc.dram_tensor(n, s, d, kind="ExternalInput")
    xc = ei("xc", [cfg.n_c, P], F16)              # full customer rows
    xa_own = ei("xa_own", [cfg.apc, P], F16)      # own article rows
    xaT = ei("xaT", [P, cfg.apc], F16)            # own articles colmajor
    xcT = ei("xcT", [P, cfg.cpc], F16)            # own customers colmajor
    scl_a_rep = ei("scl_a_rep", [P, cfg.apc], F16)
    scl_c_rep = ei("scl_c_rep", [P, cfg.cpc], F16)
    a1_idx = ei("a1_idx", [P, pa1.etot // 16], I16)
    a1_dsc = ei("a1_dsc", [P, pa1.etot // P], F16)
    cf_idx = [ei(f"cf{h}_idx", [P, pcf[h].etot // 16], I16) for h in range(2)]
    cf_dsc = [ei(f"cf{h}_dsc", [P, pcf[h].etot // P], F16) for h in range(2)]
    a2_idx = ei("a2_idx", [P, pa2.etot // 16], I16)
    a2_dsc = ei("a2_dsc", [P, pa2.etot // P], F16)
    dec_idx_a = ei("dec_idx_a", [P, dec["ld_pad"] // 16], I16)
    dec_crel = ei("dec_crel", [P, dec["ld_pad"]], F16)

    wnames = ["W_msg1_ca", "W_self1_a", "W_msg1_ac", "W_self1_c",
              "W_msg2_ca", "W_self2_a", "W_msg2_ac", "W_self2_c",
              "Wd1c", "Wd1a"]
    wts = {n: ei(n, [P, P], F16) for n in wnames}
    w2c = ei("w2c", [P, P], F16)
    bnames = ["b1_a", "b1_c", "b2_a", "b2_c",
              "bn_gamma_c", "bn_beta_c", "bn_gamma_a", "bn_beta_a",
              "b_dec1", "b_dec2c"]
    bis = {n: ei(n, [P, 1], F32) for n in bnames}

    y_out = nc.dram_tensor("y", [1, dec["ld_pad"]], F32, kind="ExternalOutput")
    dbg_outs = {}
    if dbg:
        for n, shp in [("d_comb", [cfg.n_a, 2 * P]), ("d_hcr", [cfg.cpc, P]),
                       ("d_zc", [P, cfg.cpc_pad]), ("d_za", [cfg.n_a, P]),
                       ("d_rs", [P, cfg.apc]), ("d_st", [P, 8])]:
            dbg_outs[n] = nc.dram_tensor(n, shp, F16 if n != "d_st" else F32,
                                         kind="ExternalOutput")
    rg = [list(range(NCORES))]

    with tile.TileContext(nc) as tc:
        with (
            tc.tile_pool(name="dramp", bufs=1, space="DRAM") as dramp,
            tc.tile_pool(name="const", bufs=1) as cs,
        ):
            comb_own = dramp.tile([cfg.apc, 2 * P], F16)
            comb_full = dramp.tile([cfg.n_a, 2 * P], F16, addr_space="Shared")
            haT_d = dramp.tile([P, cfg.apc], F16)
            hcrows_d = dramp.tile([cfg.cpc, P], F16)
            zcT_d = dramp.tile([P, cfg.cpc_pad], F16)
            partial_d = dramp.tile([NCORES, P, cfg.apc], F16)
            rs_out = dramp.tile([P, cfg.apc], F16)
            za_own = dramp.tile([cfg.apc, P], F16)
            za_full = dramp.tile([cfg.n_a, P], F16, addr_space="Shared")
            stc_in = dramp.tile([P, 2], F32)
            stc_out = dramp.tile([P, 2], F32, addr_space="Shared")
            sta_in = dramp.tile([P, 2], F32)
            sta_out = dramp.tile([P, 2], F32, addr_space="Shared")

            # constants
            iota8 = []
            ii = cs.tile([P, 2048], mybir.dt.int32, name="ioi")
            for wi in range(MAXW):
                nc.gpsimd.iota(ii[:], pattern=[[0, 16], [1, P]],
                               base=wi * P, channel_multiplier=0)
                ff = cs.tile([P, 2048], F16, name=f"iof{wi}")
                nc.vector.tensor_copy(ff[:], ii[:])
                iota8.append(ff)
            # decoder: const tiles with value p + wi*128 everywhere
            iotaP4 = []
            iop_i = cs.tile([P, cfg.dch], mybir.dt.int32, name="iopi")
            for wi in range(MAXW_DEC):
                nc.gpsimd.iota(iop_i[:], pattern=[[0, cfg.dch]], base=wi * P,
                               channel_multiplier=1)
                qf = cs.tile([P, cfg.dch], F16, name=f"iopf{wi}")
                nc.vector.tensor_copy(qf[:], iop_i[:])
                iotaP4.append(qf)
            ident = cs.tile([P, P], F16)
            make_identity(nc, ident[:])
            w_sb = {n: cs.tile([P, P], F16, name=f"w_{n}") for n in wnames}
            for n in wnames:
                nc.sync.dma_start(out=w_sb[n][:], in_=wts[n][:])
            w2_sb = cs.tile([P, P], F16)
            nc.sync.dma_start(out=w2_sb[:], in_=w2c[:])
            b_sb = {n: cs.tile([P, 1], F32, name=f"b_{n}") for n in bnames}
            for n in bnames:
                nc.sync.dma_start(out=b_sb[n][:], in_=bis[n][:])
            stc_sb = cs.tile([P, 2], F32)
            nc.vector.memset(stc_sb[:], 0.0)
            sta_sb = cs.tile([P, 2], F32)
            nc.vector.memset(sta_sb[:], 0.0)

            # copy own x_a rows into comb (h half filled by A1 W-stage)
            nc.sync.dma_start(out=comb_own[:, 0:P], in_=xa_own[:])

            # ----------------------------------------------------------------
            # generic aggregation superblock: gathers + one-hot matmuls
            # ----------------------------------------------------------------
            def agg_sb(ps, s, table, idx_d, dsc_d, psums, gch, esz, sbp, tag):
                """Accumulate superblock s of pass ps into psums (list of
                (psum_tile, lhsT_lo) pairs: lhsT slice [lo:lo+128] of the
                gathered row).  esz = row elements (128 or 256)."""
                offs = np.concatenate(
                    [[0], np.cumsum(ps.run_L.reshape(-1))]).astype(np.int64)
                touch = {}
                for (j, t, wlo, nw) in ps.emits[s]:
                    for wi in range(nw):
                        w = wlo + wi
                        touch.setdefault(w // 4, []).append((j, t, w))
                firsts = {b: v[0] for b, v in touch.items()}
                lasts = {b: v[-1] for b, v in touch.items()}
                by_run = {}
                for e in ps.emits[s]:
                    by_run.setdefault(e[0], []).append(e)
                tpc = gch // P
                for j in sorted(by_run):
                    r = s * ps.nblk + j
                    o = int(offs[r])
                    L = int(ps.run_L[s, j])
                    nt = L // P
                    blk_rows = min(ps.srcb, ps.nsrc_rows - j * ps.srcb)
                    idx_sb = sbp.tile([P, L // 16], I16, tag=f"{tag}gi",
                                      name=f"{tag}gi", bufs=3)
                    nc.scalar.dma_start(out=idx_sb[:],
                                        in_=idx_d[:, o // 16:(o + L) // 16])
                    dsc_sb = sbp.tile([P, nt], F16, tag=f"{tag}gd",
                                      name=f"{tag}gd", bufs=3)
                    nc.scalar.dma_start(out=dsc_sb[:],
                                        in_=dsc_d[:, o // P:o // P + nt])
                    run_emits = by_run[j]
                    for c0 in range(0, L, gch):
                        cl = min(gch, L - c0)
                        c = c0 // P          # first tile index of chunk
                        ctn = cl // P
                        x = sbp.tile([P, tpc, esz], F16, tag=f"{tag}gx",
                                     name=f"{tag}gx", bufs=6)
                        nc.gpsimd.dma_gather(
                            x[:, :ctn, :],
                            table[j * ps.srcb:j * ps.srcb + blk_rows, :],
                            idx_sb[:, c0 // 16:(c0 + cl) // 16],
                            cl, cl, esz, queue_num=next_q())
                        ch_emits = [e for e in run_emits
                                    if c <= e[1] < c + ctn]
                        maxnw = max(e[3] for e in ch_emits)
                        dstb = dsc_sb[:, c:c + ctn].to_broadcast([P, ctn, P])
                        p8s = {}
                        for wi in range(maxnw):
                            p8 = sbp.tile([P, gch], F16, tag=f"{tag}gp",
                                          name=f"{tag}gp", bufs=2 * MAXW)
                            p83 = p8[:, :ctn * P].rearrange(
                                "p (t w) -> p t w", w=P)
                            nc.vector.tensor_tensor(
                                out=p83,
                                in0=iota8[wi][:, :ctn * P].rearrange(
                                    "p (t w) -> p t w", w=P),
                                in1=dstb, op=mybir.AluOpType.is_equal)
                            p8s[wi] = p8
                        for (j2, t, wlo, nw) in ch_emits:
                            for wi in range(nw):
                                w = wlo + wi
                                p8 = p8s[wi]
                                first = firsts[w // 4] == (j2, t, w)
                                last = lasts[w // 4] == (j2, t, w)
                                for psum, lo in psums:
                                    nc.tensor.matmul(
                                        psum[:, w * P:(w + 1) * P],
                                        lhsT=x[:, t - c, lo:lo + P],
                                        rhs=p8[:, (t - c) * P:
                                               (t - c + 1) * P],
                                        start=first, stop=last,
                                        skip_group_check=True)

            def zero_uncovered(ps, s, dst_sb, nodes):
                for w in ps.uncovered[s]:
                    a, b = w * P, min(w * P + P, nodes)
                    nc.vector.memset(dst_sb[:, a:b], 0.0)

            def emit_rows(srcT_sb, cw, rows_dram, row_base, sbp, psp, tag,
                          col_lo=0, col_n=P):
                for b0 in range(0, cw, P):
                    bw = min(P, cw - b0)
                    tp = psp.tile([P, P], F16, tag=f"{tag}tp", name=f"{tag}tp",
                                  bufs=1)
                    nc.tensor.transpose(tp[:bw, :], srcT_sb[:, b0:b0 + bw],
                                        ident[:])
                    rows = sbp.tile([P, P], F16, tag=f"{tag}ro",
                                    name=f"{tag}ro", bufs=3)
                    nc.scalar.copy(rows[:bw, :], tp[:bw, :])
                    nc.sync.dma_start(
                        out=rows_dram[row_base + b0:row_base + b0 + bw,
                                      col_lo:col_lo + col_n],
                        in_=rows[:bw, :])

            # ================= A1: layer-1 articles =================
            with (
                tc.tile_pool(name="a1", bufs=1) as sbp,
                tc.tile_pool(name="a1p", bufs=1, space="PSUM") as psp,
            ):
                for s in range(pa1.nsb):
                    nodes = pa1.sb_nodes[s]
                    nwin = -(-nodes // P)
                    psum = psp.tile([P, nwin * P], F32, tag="a1ps",
                                    name="a1ps", bufs=1)
                    agg_sb(pa1, s, xc, a1_idx, a1_dsc, [(psum, 0)],
                           cfg.gch_a1, P, sbp, "a1")
                    # scaled copy psum -> meanT
                    mean_sb = sbp.tile([P, cfg.sbn], F16, tag="a1mn",
                                       name="a1mn", bufs=2)
                    scl_sb = sbp.tile([P, cfg.sbn], F16, tag="a1sc",
                                      name="a1sc", bufs=2)
                    nc.scalar.dma_start(
                        out=scl_sb[:, :nodes],
                        in_=scl_a_rep[:, s * cfg.sbn:s * cfg.sbn + nodes])
                    nc.vector.tensor_tensor(out=mean_sb[:, :nodes],
                                            in0=psum[:, :nodes],
                                            in1=scl_sb[:, :nodes],
                                            op=mybir.AluOpType.mult)
                    zero_uncovered(pa1, s, mean_sb, nodes)
                    # W-stage: haT = relu(Wmsg1^T meanT + Wself1^T xaT + b)
                    haT_sb = sbp.tile([P, cfg.sbn], F16, tag="a1h",
                                      name="a1h", bufs=2)
                    for c0 in range(0, nodes, 512):
                        cw = min(512, nodes - c0)
                        sT = sbp.tile([P, 512], F16, tag="a1sf", name="a1sf",
                                      bufs=3)
                        nc.scalar.dma_start(
                            out=sT[:, :cw],
                            in_=xaT[:, s * cfg.sbn + c0:s * cfg.sbn + c0 + cw])
                        wps = psp.tile([P, 512], F32, tag="a1wp", name="a1wp",
                                       bufs=2)
                        nc.tensor.matmul(wps[:, :cw], lhsT=w_sb["W_msg1_ca"][:],
                                         rhs=mean_sb[:, c0:c0 + cw],
                                         start=True, stop=False,
                                         skip_group_check=True)
                        nc.tensor.matmul(wps[:, :cw], lhsT=w_sb["W_self1_a"][:],
                                         rhs=sT[:, :cw], start=False, stop=True,
                                         skip_group_check=True)
                        nc.scalar.activation(
                            haT_sb[:, c0:c0 + cw], wps[:, :cw],
                            mybir.ActivationFunctionType.Relu,
                            bias=b_sb["b1_a"][:], scale=1.0)
                    nc.sync.dma_start(
                        out=haT_d[:, s * cfg.sbn:s * cfg.sbn + nodes],
                        in_=haT_sb[:, :nodes])
                    emit_rows(haT_sb, nodes, comb_own, s * cfg.sbn, sbp, psp,
                              "a1r", col_lo=P, col_n=P)

            # ================= AllGather comb =================
            nc.gpsimd.collective_compute(
                "AllGather", mybir.AluOpType.bypass, replica_groups=rg,
                ins=[comb_own[:]], outs=[comb_full[:]])

            # ================= C-fused: layers 1+2 customers =================
            for h in range(2 if stage >= 2 else 0):
                ps = pcf[h]
                with (
                    tc.tile_pool(name=f"cf{h}", bufs=1) as sbp,
                    tc.tile_pool(name=f"cf{h}p", bufs=1, space="PSUM") as psp,
                ):
                    for s in range(ps.nsb):
                        nodes = ps.sb_nodes[s]
                        nwin = -(-nodes // P)
                        base = h * cfg.chalf + s * cfg.sbn_cf
                        psum1 = psp.tile([P, nwin * P], F32, tag="cfp1",
                                         name="cfp1", bufs=1)
                        psum2 = psp.tile([P, nwin * P], F32, tag="cfp2",
                                         name="cfp2", bufs=1)
                        agg_sb(ps, s, comb_full, cf_idx[h], cf_dsc[h],
                               [(psum1, 0), (psum2, P)], cfg.gch_cf, 2 * P,
                               sbp, "cf")
                        scl_sb = sbp.tile([P, cfg.sbn_cf], F16, tag="cfsc",
                                          name="cfsc", bufs=2)
                        nc.scalar.dma_start(
                            out=scl_sb[:, :nodes],
                            in_=scl_c_rep[:, base:base + nodes])
                        mean1 = sbp.tile([P, cfg.sbn_cf], F16, tag="cfm1",
                                         name="cfm1", bufs=2)
                        nc.vector.tensor_tensor(out=mean1[:, :nodes],
                                                in0=psum1[:, :nodes],
                                                in1=scl_sb[:, :nodes],
                                                op=mybir.AluOpType.mult)
                        zero_uncovered(ps, s, mean1, nodes)
                        mean2 = sbp.tile([P, cfg.sbn_cf], F16, tag="cfm2",
                                         name="cfm2", bufs=2)
                        nc.vector.tensor_tensor(out=mean2[:, :nodes],
                                                in0=psum2[:, :nodes],
                                                in1=scl_sb[:, :nodes],
                                                op=mybir.AluOpType.mult)
                        zero_uncovered(ps, s, mean2, nodes)
                        hcT_sb = sbp.tile([P, cfg.sbn_cf], F16, tag="cfh",
                                          name="cfh", bufs=2)
                        zcT_sb = sbp.tile([P, cfg.sbn_cf], F16, tag="cfz",
                                          name="cfz", bufs=2)
                        for c0 in range(0, nodes, 512):
                            cw = min(512, nodes - c0)
                            sT = sbp.tile([P, 512], F16, tag="cfsf",
                                          name="cfsf", bufs=3)
                            nc.scalar.dma_start(
                                out=sT[:, :cw],
                                in_=xcT[:, base + c0:base + c0 + cw])
                            wps = psp.tile([P, 512], F32, tag="cfwp",
                                           name="cfwp", bufs=1)
                            nc.tensor.matmul(wps[:, :cw],
                                             lhsT=w_sb["W_msg1_ac"][:],
                                             rhs=mean1[:, c0:c0 + cw],
                                             start=True, stop=False,
                                             skip_group_check=True)
                            nc.tensor.matmul(wps[:, :cw],
                                             lhsT=w_sb["W_self1_c"][:],
                                             rhs=sT[:, :cw],
                                             start=False, stop=True,
                                             skip_group_check=True)
                            nc.scalar.activation(
                                hcT_sb[:, c0:c0 + cw], wps[:, :cw],
                                mybir.ActivationFunctionType.Relu,
                                bias=b_sb["b1_c"][:], scale=1.0)
                            wps2 = psp.tile([P, 512], F32, tag="cfw2",
                                            name="cfw2", bufs=1)
                            nc.tensor.matmul(wps2[:, :cw],
                                             lhsT=w_sb["W_msg2_ac"][:],
                                             rhs=mean2[:, c0:c0 + cw],
                                             start=True, stop=False,
                                             skip_group_check=True)
                            nc.tensor.matmul(wps2[:, :cw],
                                             lhsT=w_sb["W_self2_c"][:],
                                             rhs=hcT_sb[:, c0:c0 + cw],
                                             start=False, stop=True,
                                             skip_group_check=True)
                            nc.scalar.activation(
                                zcT_sb[:, c0:c0 + cw], wps2[:, :cw],
                                mybir.ActivationFunctionType.Identity,
                                bias=b_sb["b2_c"][:], scale=1.0)
                            # BN-c stats
                            part = sbp.tile([P, 1], F32, tag="cfs1",
                                            name="cfs1", bufs=2)
                            nc.vector.reduce_sum(part[:], zcT_sb[:, c0:c0 + cw],
                                                 mybir.AxisListType.X)
                            nc.vector.tensor_add(stc_sb[:, 0:1],
                                                 stc_sb[:, 0:1], part[:])
                            trash = sbp.tile([P, 512], F32, tag="cftr",
                                             name="cftr", bufs=2)
                            part2 = sbp.tile([P, 1], F32, tag="cfs2",
                                             name="cfs2", bufs=2)
                            nc.scalar.activation(
                                trash[:, :cw], zcT_sb[:, c0:c0 + cw],
                                mybir.ActivationFunctionType.Square,
                                accum_out=part2[:])
                            nc.vector.tensor_add(stc_sb[:, 1:2],
                                                 stc_sb[:, 1:2], part2[:])
                        nc.sync.dma_start(out=zcT_d[:, base:base + nodes],
                                          in_=zcT_sb[:, :nodes])
                        emit_rows(hcT_sb, nodes, hcrows_d, base, sbp, psp,
                                  "cfr")

            # zero zcT_d pad tail
            if stage >= 2 and cfg.cpc_pad > cfg.cpc:
                with tc.tile_pool(name="ztp", bufs=1) as sbp:
                    zt = sbp.tile([P, cfg.cpc_pad - cfg.cpc], F16, name="ztt")
                    nc.vector.memset(zt[:], 0.0)
                    nc.sync.dma_start(out=zcT_d[:, cfg.cpc:], in_=zt[:])

            # BN-c stats AllReduce (A2 overlaps with it)
            if stage >= 2:
                nc.sync.dma_start(out=stc_in[:], in_=stc_sb[:])
                nc.gpsimd.collective_compute(
                    "AllReduce", mybir.AluOpType.add, replica_groups=rg,
                    ins=[stc_in[:]], outs=[stc_out[:]])

            # ================= A2: partial article aggregates =================
            with (
                tc.tile_pool(name="a2", bufs=1) as sbp,
                tc.tile_pool(name="a2p", bufs=1, space="PSUM") as psp,
            ):
                for s in range(pa2.nsb if stage >= 3 else 0):
                    nodes = pa2.sb_nodes[s]
                    nwin = -(-nodes // P)
                    psum = psp.tile([P, nwin * P], F32, tag="a2ps",
                                    name="a2ps", bufs=2)
                    agg_sb(pa2, s, hcrows_d, a2_idx, a2_dsc, [(psum, 0)],
                           cfg.gch_a1, P, sbp, "a2")
                    stg = sbp.tile([P, cfg.sbn], F16, tag="a2st",
                                   name="a2st", bufs=3)
                    nc.vector.tensor_copy(stg[:, :nodes], psum[:, :nodes])
                    zero_uncovered(pa2, s, stg, nodes)
                    blk = s // cfg.sb_per_blk
                    col = (s % cfg.sb_per_blk) * cfg.sbn
                    nc.sync.dma_start(
                        out=partial_d[blk, :, col:col + nodes],
                        in_=stg[:, :nodes])

            # ================= ReduceScatter =================
            if stage >= 3:
                nc.gpsimd.collective_compute(
                    "ReduceScatter", mybir.AluOpType.add, replica_groups=rg,
                    ins=[partial_d[:]], outs=[rs_out[:]])

            # ---------------- BN coeff helper ----------------
            def bn_coeff(st_sb, n, gamma, beta, tagp, sbp):
                mu = sbp.tile([P, 1], F32, name=f"mu{tagp}")
                nc.vector.tensor_scalar_mul(mu[:], st_sb[:, 0:1], 1.0 / n)
                msq = sbp.tile([P, 1], F32, name=f"ms{tagp}")
                nc.vector.tensor_scalar_mul(msq[:], st_sb[:, 1:2], 1.0 / n)
                mu2 = sbp.tile([P, 1], F32, name=f"m2{tagp}")
                nc.vector.tensor_mul(mu2[:], mu[:], mu[:])
                var = sbp.tile([P, 1], F32, name=f"va{tagp}")
                nc.vector.tensor_sub(var[:], msq[:], mu2[:])
                nc.vector.tensor_scalar_add(var[:], var[:], BN_EPS)
                sd = sbp.tile([P, 1], F32, name=f"sd{tagp}")
                nc.scalar.activation(sd[:], var[:],
                                     mybir.ActivationFunctionType.Sqrt)
                rstd = sbp.tile([P, 1], F32, name=f"rs{tagp}")
                nc.vector.reciprocal(rstd[:], sd[:])
                scl = sbp.tile([P, 1], F32, name=f"sc{tagp}")
                nc.vector.tensor_mul(scl[:], b_sb[gamma][:], rstd[:])
                mg = sbp.tile([P, 1], F32, name=f"mg{tagp}")
                nc.vector.tensor_mul(mg[:], mu[:], scl[:])
                bia = sbp.tile([P, 1], F32, name=f"bi{tagp}")
                nc.vector.tensor_sub(bia[:], b_sb[beta][:], mg[:])
                return scl, bia

            with tc.tile_pool(name="tail", bufs=1) as keep:
                ucrows = keep.tile([P, cfg.cpc_pad], F16, name="ucrows")

                # ============ U_c build (overlaps RS wait) ============
                with (
                    tc.tile_pool(name="uc", bufs=1) as sbp,
                    tc.tile_pool(name="ucp", bufs=1, space="PSUM") as psp,
                ):
                  if stage >= 4:
                    st = sbp.tile([P, 2], F32, name="ucst")
                    nc.scalar.dma_start(out=st[:], in_=stc_out[:])
                    scl_c_col, bia_c_col = bn_coeff(
                        st, cfg.n_c, "bn_gamma_c", "bn_beta_c", "c", sbp)
                    for c0 in range(0, cfg.cpc_pad, 512):
                        cw = min(512, cfg.cpc_pad - c0)
                        zT = sbp.tile([P, 512], F16, tag="ucz", name="ucz",
                                      bufs=3)
                        nc.scalar.dma_start(out=zT[:, :cw],
                                            in_=zcT_d[:, c0:c0 + cw])
                        bnT = sbp.tile([P, 512], F16, tag="ucb", name="ucb",
                                       bufs=3)
                        nc.scalar.activation(
                            bnT[:, :cw], zT[:, :cw],
                            mybir.ActivationFunctionType.Identity,
                            bias=bia_c_col[:], scale=scl_c_col[:])
                        ups = psp.tile([P, 512], F32, tag="ucp", name="ucp",
                                       bufs=2)
                        nc.tensor.matmul(ups[:, :cw], lhsT=w_sb["Wd1c"][:],
                                         rhs=bnT[:, :cw], start=True,
                                         stop=True, skip_group_check=True)
                        uT = sbp.tile([P, 512], F16, tag="ucu", name="ucu",
                                      bufs=3)
                        nc.scalar.activation(
                            uT[:, :cw], ups[:, :cw],
                            mybir.ActivationFunctionType.Identity,
                            bias=b_sb["b_dec1"][:], scale=1.0)
                        for b0 in range(0, cw, P):
                            tp = psp.tile([P, P], F16, tag="uctp",
                                          name="uctp", bufs=2)
                            nc.tensor.transpose(tp[:], uT[:, b0:b0 + P],
                                                ident[:])
                            nc.scalar.copy(ucrows[:, c0 + b0:c0 + b0 + P],
                                           tp[:])

                # ============== z_a stage (after RS) ==============
                with (
                    tc.tile_pool(name="za", bufs=1) as sbp,
                    tc.tile_pool(name="zap", bufs=1, space="PSUM") as psp,
                ):
                    for c0 in range(0, cfg.apc if stage >= 4 else 0, 512):
                        cw = min(512, cfg.apc - c0)
                        rsT = sbp.tile([P, 512], F16, tag="zar", name="zar",
                                       bufs=3)
                        nc.scalar.dma_start(out=rsT[:, :cw],
                                            in_=rs_out[:, c0:c0 + cw])
                        sclT = sbp.tile([P, 512], F16, tag="zas", name="zas",
                                        bufs=3)
                        nc.scalar.dma_start(out=sclT[:, :cw],
                                            in_=scl_a_rep[:, c0:c0 + cw])
                        m2 = sbp.tile([P, 512], F16, tag="zam", name="zam",
                                      bufs=3)
                        nc.vector.tensor_tensor(out=m2[:, :cw],
                                                in0=rsT[:, :cw],
                                                in1=sclT[:, :cw],
                                                op=mybir.AluOpType.mult)
                        hT = sbp.tile([P, 512], F16, tag="zah", name="zah",
                                      bufs=3)
                        nc.scalar.dma_start(out=hT[:, :cw],
                                            in_=haT_d[:, c0:c0 + cw])
                        wps = psp.tile([P, 512], F32, tag="zap", name="zap",
                                       bufs=2)
                        nc.tensor.matmul(wps[:, :cw],
                                         lhsT=w_sb["W_msg2_ca"][:],
                                         rhs=m2[:, :cw], start=True,
                                         stop=False, skip_group_check=True)
                        nc.tensor.matmul(wps[:, :cw],
                                         lhsT=w_sb["W_self2_a"][:],
                                         rhs=hT[:, :cw], start=False,
                                         stop=True, skip_group_check=True)
                        zaT = sbp.tile([P, 512], F16, tag="zaz", name="zaz",
                                       bufs=3)
                        nc.scalar.activation(
                            zaT[:, :cw], wps[:, :cw],
                            mybir.ActivationFunctionType.Identity,
                            bias=b_sb["b2_a"][:], scale=1.0)
                        part = sbp.tile([P, 1], F32, tag="zs1", name="zs1",
                                        bufs=2)
                        nc.vector.reduce_sum(part[:], zaT[:, :cw],
                                             mybir.AxisListType.X)
                        nc.vector.tensor_add(sta_sb[:, 0:1], sta_sb[:, 0:1],
                                             part[:])
                        trash = sbp.tile([P, 512], F32, tag="ztr", name="ztr",
                                         bufs=2)
                        part2 = sbp.tile([P, 1], F32, tag="zs2", name="zs2",
                                         bufs=2)
                        nc.scalar.activation(
                            trash[:, :cw], zaT[:, :cw],
                            mybir.ActivationFunctionType.Square,
                            accum_out=part2[:])
                        nc.vector.tensor_add(sta_sb[:, 1:2], sta_sb[:, 1:2],
                                             part2[:])
                        emit_rows(zaT, cw, za_own, c0, sbp, psp, "zarw")

                if stage >= 4:
                    nc.sync.dma_start(out=sta_in[:], in_=sta_sb[:])
                    nc.gpsimd.collective_compute(
                        "AllReduce", mybir.AluOpType.add, replica_groups=rg,
                        ins=[sta_in[:]], outs=[sta_out[:]])
                    nc.gpsimd.collective_compute(
                        "AllGather", mybir.AluOpType.bypass, replica_groups=rg,
                        ins=[za_own[:]], outs=[za_full[:]])

                if dbg:
                    nc.sync.dma_start(out=dbg_outs["d_comb"][:],
                                      in_=comb_full[:])
                    nc.sync.dma_start(out=dbg_outs["d_hcr"][:],
                                      in_=hcrows_d[:])
                    nc.sync.dma_start(out=dbg_outs["d_zc"][:], in_=zcT_d[:])
                    nc.sync.dma_start(out=dbg_outs["d_za"][:], in_=za_full[:])
                    nc.sync.dma_start(out=dbg_outs["d_rs"][:], in_=rs_out[:])
                    nc.sync.dma_start(out=dbg_outs["d_st"][:, 0:2],
                                      in_=stc_out[:])
                    nc.sync.dma_start(out=dbg_outs["d_st"][:, 2:4],
                                      in_=sta_out[:])

                # ================= decoder =================
                with (
                    tc.tile_pool(name="dc", bufs=1) as sbp,
                    tc.tile_pool(name="dcp", bufs=1, space="PSUM") as psp,
                ):
                  if stage >= 5:
                    sta_sb2 = sbp.tile([P, 2], F32, name="dsta")
                    nc.scalar.dma_start(out=sta_sb2[:], in_=sta_out[:])
                    scl_a_col, bia_a_col = bn_coeff(
                        sta_sb2, cfg.n_a, "bn_gamma_a", "bn_beta_a", "a", sbp)
                    goffs, grp_L = dec["goffs"], dec["grp_L"]
                    tile_wlo, tile_nw = dec["tile_wlo"], dec["tile_nw"]
                    for ab in range(cfg.nab):
                        o0, L = int(goffs[ab]), int(grp_L[ab])
                        blk_rows = min(cfg.srcb_a, cfg.n_a - ab * cfg.srcb_a)
                        for c0 in range(o0, o0 + L, cfg.dch):
                            cl = min(cfg.dch, o0 + L - c0)
                            ctn = cl // P
                            ixa = sbp.tile([P, cfg.dch // 16], I16, tag="dia",
                                           name="dia", bufs=4)
                            nc.scalar.dma_start(
                                out=ixa[:, :cl // 16],
                                in_=dec_idx_a[:, c0 // 16:(c0 + cl) // 16])
                            zg = sbp.tile([P, 1, cfg.dch], F16, tag="dzg",
                                          name="dzg", bufs=4)
                            nc.gpsimd.dma_gather(
                                zg[:, :, :cl],
                                za_full[ab * cfg.srcb_a:
                                        ab * cfg.srcb_a + blk_rows, :],
                                ixa[:, :cl // 16], cl, cl, P,
                                transpose=True, queue_num=next_q())
                            bnz = sbp.tile([P, cfg.dch], F16, tag="dbn",
                                           name="dbn", bufs=4)
                            nc.scalar.activation(
                                bnz[:, :cl], zg[:, 0, :cl],
                                mybir.ActivationFunctionType.Identity,
                                bias=bia_a_col[:], scale=scl_a_col[:])
                            crel = sbp.tile([P, cfg.dch], F16, tag="dcr",
                                            name="dcr", bufs=4)
                            nc.scalar.dma_start(out=crel[:, :cl],
                                                in_=dec_crel[:, c0:c0 + cl])
                            spsum = psp.tile([P, cfg.dch], F32, tag="dsp",
                                             name="dsp", bufs=2)
                            emlist = []
                            for cc in range(0, cl, 512):
                                emlist.append(("w", cc, min(512, cl - cc)))
                            for t in range(ctn):
                                g = c0 // P + t
                                for wi in range(int(tile_nw[g])):
                                    emlist.append(("q", t, wi))
                            banks = {}
                            for em in emlist:
                                if em[0] == "w":
                                    bset = set(range(
                                        em[1] // 512,
                                        (em[1] + em[2] - 1) // 512 + 1))
                                else:
                                    bset = {em[1] * P // 512}
                                for b in bset:
                                    banks.setdefault(b, []).append(em)
                            firsts = {b: v[0] for b, v in banks.items()}
                            lasts = {b: v[-1] for b, v in banks.items()}
                            for em in emlist:
                                if em[0] == "w":
                                    _, cc, cww = em
                                    b = cc // 512
                                    nc.tensor.matmul(
                                        spsum[:, cc:cc + cww],
                                        lhsT=w_sb["Wd1a"][:],
                                        rhs=bnz[:, cc:cc + cww],
                                        start=firsts[b] == em,
                                        stop=lasts[b] == em,
                                        skip_group_check=True)
                            qts = {}
                            for t in range(ctn):
                                g = c0 // P + t
                                for wi in range(int(tile_nw[g])):
                                    if wi not in qts:
                                        q = sbp.tile([P, cfg.dch], F16,
                                                     tag=f"dq{wi}",
                                                     name=f"dq{wi}", bufs=2)
                                        nc.vector.tensor_tensor(
                                            out=q[:, :cl],
                                            in0=iotaP4[wi][:, :cl],
                                            in1=crel[:, :cl],
                                            op=mybir.AluOpType.is_equal)
                                        qts[wi] = q
                            for em in emlist:
                                if em[0] == "q":
                                    _, t, wi = em
                                    g = c0 // P + t
                                    w = int(tile_wlo[g]) + wi
                                    b = t * P // 512
                                    nc.tensor.matmul(
                                        spsum[:, t * P:(t + 1) * P],
                                        lhsT=ucrows[:, w * P:(w + 1) * P],
                                        rhs=qts[wi][:, t * P:(t + 1) * P],
                                        start=firsts[b] == em,
                                        stop=lasts[b] == em,
                                        skip_group_check=True)
                            relu_sb = sbp.tile([P, cfg.dch], F16, tag="drl",
                                               name="drl", bufs=3)
                            nc.scalar.activation(
                                relu_sb[:, :cl], spsum[:, :cl],
                                mybir.ActivationFunctionType.Relu)
                            yp = psp.tile([P, cfg.dch], F32, tag="dyp",
                                          name="dyp", bufs=1)
                            for cc in range(0, cl, 512):
                                cww = min(512, cl - cc)
                                nc.tensor.matmul(yp[0:1, cc:cc + cww],
                                                 lhsT=w2_sb[:],
                                                 rhs=relu_sb[:, cc:cc + cww],
                                                 start=True, stop=True,
                                                 skip_group_check=True)
                            ysb = sbp.tile([1, cfg.dch], F32, tag="dys",
                                           name="dys", bufs=3)
                            nc.scalar.copy(ysb[:, :cl], yp[0:1, :cl])
                            nc.sync.dma_start(out=y_out[:, c0:c0 + cl],
                                              in_=ysb[:, :cl])

    nc.compile()
    return nc


# ---------------------------------------------------------------------------
# entry point
# ---------------------------------------------------------------------------

def make_in_maps(cfg, inputs, pa1, pcf, pa2, dec, scl_a, scl_c):
    f = lambda a: np.ascontiguousarray(np.asarray(a), dtype=np.float32)
    xc16 = f(inputs["x_customer"]).astype(np.float16)
    xa16 = f(inputs["x_article"]).astype(np.float16)
    wd1 = f(inputs["W_dec1"])
    base = dict(
        xc=xc16,
        W_msg1_ca=f(inputs["W_msg1_ca"]).astype(np.float16),
        W_self1_a=f(inputs["W_self1_a"]).astype(np.float16),
        W_msg1_ac=f(inputs["W_msg1_ac"]).astype(np.float16),
        W_self1_c=f(inputs["W_self1_c"]).astype(np.float16),
        W_msg2_ca=f(inputs["W_msg2_ca"]).astype(np.float16),
        W_self2_a=f(inputs["W_self2_a"]).astype(np.float16),
        W_msg2_ac=f(inputs["W_msg2_ac"]).astype(np.float16),
        W_self2_c=f(inputs["W_self2_c"]).astype(np.float16),
        Wd1c=wd1[:P].astype(np.float16), Wd1a=wd1[P:].astype(np.float16),
        w2c=np.concatenate([f(inputs["W_dec2"]).reshape(P, 1),
                            np.zeros((P, P - 1), np.float32)],
                           axis=1).astype(np.float16),
        b1_a=f(inputs["b1_a"]).reshape(P, 1),
        b1_c=f(inputs["b1_c"]).reshape(P, 1),
        b2_a=f(inputs["b2_a"]).reshape(P, 1),
        b2_c=f(inputs["b2_c"]).reshape(P, 1),
        bn_gamma_c=f(inputs["bn_gamma_c"]).reshape(P, 1),
        bn_beta_c=f(inputs["bn_beta_c"]).reshape(P, 1),
        bn_gamma_a=f(inputs["bn_gamma_a"]).reshape(P, 1),
        bn_beta_a=f(inputs["bn_beta_a"]).reshape(P, 1),
        b_dec1=f(inputs["b_dec1"]).reshape(P, 1),
        b_dec2c=np.full((P, 1), float(np.asarray(inputs["b_dec2"]).item()),
                        np.float32),
    )
    in_maps = []
    for k in range(NCORES):
        m = dict(base)
        m["xa_own"] = np.ascontiguousarray(xa16[k * cfg.apc:(k + 1) * cfg.apc])
        m["xaT"] = np.ascontiguousarray(
            xa16[k * cfg.apc:(k + 1) * cfg.apc].T)
        m["xcT"] = np.ascontiguousarray(
            xc16[k * cfg.cpc:(k + 1) * cfg.cpc].T)
        m["scl_a_rep"] = np.tile(
            scl_a[k * cfg.apc:(k + 1) * cfg.apc].astype(np.float16)[None, :],
            (P, 1))
        m["scl_c_rep"] = np.tile(
            scl_c[k * cfg.cpc:(k + 1) * cfg.cpc].astype(np.float16)[None, :],
            (P, 1))
        m["a1_idx"], m["a1_dsc"] = pa1.idx[k], pa1.dsc[k]
        for h in range(2):
            m[f"cf{h}_idx"], m[f"cf{h}_dsc"] = pcf[h].idx[k], pcf[h].dsc[k]
        m["a2_idx"], m["a2_dsc"] = pa2.idx[k], pa2.dsc[k]
        m["dec_idx_a"], m["dec_crel"] = dec["idx_a"][k], dec["crel"][k]
        in_maps.append(m)
    return in_maps


def run(cfg, inputs, trace=False, dbg=False):
    pa1, pcf, pa2, dec, scl_a, scl_c = prep_all(cfg, inputs)
    in_maps = make_in_maps(cfg, inputs, pa1, pcf, pa2, dec, scl_a, scl_c)
    nc = build_nc(cfg, pa1, pcf, pa2, dec, dbg=dbg)
    res = run_bass_kernel_spmd(nc, in_maps, core_ids=list(range(NCORES)),
                               trace=trace)
    y = np.empty(cfg.e_lbl, np.float32)
    b2 = float(np.asarray(inputs["b_dec2"]).item())
    for k in range(NCORES):
        yl = res.results[k]["y"].reshape(-1) + b2
        po = dec["out_pos"][k]
        vm = po >= 0
        y[po[vm]] = yl[vm]
    return y, res


def kernel(**inputs):
    cfg = Cfg()
    y, _ = run(cfg, inputs, trace=False)
    return y


# revision 3
# speedup vs baseline: 1.2073x; 1.0148x over previous
"""Hetero GNN encoder/decoder v2 - restructured to minimize SWDGE descgen + DVE.

Key changes vs v1:
  - Pass order: A1 (dst=article-owner) -> AllGather comb[x_a|h_a] ->
    C-fused (C1+C2 share one 512B gather + one-hot P) -> A2
    (src=customer-owner, gathers LOCAL h_c rows, partial agg over all
    articles) -> ReduceScatter -> z_a -> AllGather z_a rows -> decoder.
  - One-hot P = is_equal only; 1/cnt scale applied at psum->SBUF copy
    via host-replicated per-column scale tables.
  - Decoder: transpose-gather of z_a (column-major), BN via per-partition
    scalar activation, Wd1a matmul + U_c window one-hot matmuls accumulate
    into one PSUM, w2 reduction via M=1 matmul.  No hc/ua AllGathers.
"""
import sys

sys.path.insert(0, "/opt/trn_rl_repo")

import numpy as np

import concourse.bacc as bacc
import concourse.bass as bass
import concourse.mybir as mybir
import concourse.tile as tile
from concourse.bass_utils import run_bass_kernel_spmd
from concourse.masks import make_identity

P = 128
NCORES = 8
MAXW = 4
MAXW_DEC = 8
BN_EPS = 1e-5
F32 = mybir.dt.float32
F16 = mybir.dt.float16
I16 = mybir.dt.int16


class Cfg:
    def __init__(self, small=False):
        if small:
            self.n_c, self.n_a, self.e_lbl = 6144, 1024, 8192
            self.sbn, self.sbn_cf = 128, 128
            self.srcb_c, self.srcb_a, self.srcb_h = 1024, 256, 384
            self.gch_a1, self.gch_cf, self.dch = 512, 256, 256
        else:
            self.n_c, self.n_a, self.e_lbl = 300000, 100000, 1000000
            self.sbn = 1250                 # superblock nodes (divides 12500)
            self.sbn_cf = 1024              # Cf superblock (2 psums, bank fit)
            self.srcb_c = 30000             # A1 src block (customers)
            self.srcb_a = 25000             # Cf src block (articles)
            self.srcb_h = 18750             # A2 src block (local customers)
            self.gch_a1 = 1024              # idx per gather, A1/A2 (256B rows)
            self.gch_cf = 1024              # idx per gather, Cf (512B rows)
            self.dch = 1024                 # decoder labels per chunk
        self.cpc, self.apc = self.n_c // NCORES, self.n_a // NCORES
        self.chalf = self.cpc // 2
        self.nblk_c = -(-self.n_c // self.srcb_c)
        self.nblk_a = -(-self.n_a // self.srcb_a)
        self.nblk_h = -(-self.cpc // self.srcb_h)
        self.nab = self.nblk_a
        assert self.apc % self.sbn == 0
        self.sb_per_blk = self.apc // self.sbn
        self.nwin_uc = -(-self.cpc // P)
        self.cpc_pad = self.nwin_uc * P


def _ru(x, m):
    return (x + m - 1) // m * m


def _wrap_idx(flat):
    """[n] int -> [128, n/16] wrap (16-partition layout, replicated x8)."""
    n = flat.shape[0]
    w = flat.astype(np.int16).reshape(n // 16, 16).T
    return np.tile(w, (8, 1))


def _pack_pcol(a):
    """[n] -> [128, n/128]: element i -> partition i%128, col i//128."""
    return np.ascontiguousarray(a.reshape(-1, P).T)


# ---------------------------------------------------------------------------
# host prep: one aggregation pass (uniform SPMD structure across cores)
# ---------------------------------------------------------------------------

class AggPass:
    def __init__(self, name, nloc, srcb, nblk, nsrc_rows, sbn):
        self.name, self.nloc, self.srcb = name, nloc, srcb
        self.nblk, self.nsrc_rows, self.sbn = nblk, nsrc_rows, sbn
        self.nsb = -(-nloc // sbn)
        self.sb_nodes = [min(sbn, nloc - s * sbn) for s in range(self.nsb)]


def prep_agg_pass(name, src, dst_loc, core_e, nloc, srcb, nblk, nsrc_rows, sbn):
    """Edges (src gathered, dst accumulated into nloc-range) per core.

    Returns AggPass with: run_L [nsb,nblk] uniform padded lengths, emits
    (per sb: list of (j, t, wlo, nw)), uncovered windows, per-core idx
    (wrapped int16) and dsc (dst-rel fp16, [128, etot/128])."""
    ap = AggPass(name, nloc, srcb, nblk, nsrc_rows, sbn)
    nsb = ap.nsb
    nruns = nsb * nblk

    per_core, counts = [], np.zeros((NCORES, nruns), np.int64)
    for k in range(NCORES):
        m = core_e == k
        s, d = src[m], dst_loc[m]
        j = s // srcb
        sb = d // sbn
        order = np.lexsort((d, j, sb))
        s, d, j, sb = s[order], d[order], j[order], sb[order]
        rid = sb * nblk + j
        counts[k] = np.bincount(rid, minlength=nruns)
        per_core.append((s, d, rid))

    run_L = _ru(counts.max(axis=0), P)
    offs = np.concatenate([[0], np.cumsum(run_L)]).astype(np.int64)
    etot = int(offs[-1])
    ap.run_L = run_L.reshape(nsb, nblk)
    ap.etot = etot

    dstrel_all = np.full((NCORES, etot), -1.0e9, np.float64)
    pos_all = []
    for k in range(NCORES):
        s, d, rid = per_core[k]
        run_start = np.concatenate([[0], np.cumsum(counts[k])])[:-1]
        pos = offs[rid] + (np.arange(len(s)) - run_start[rid])
        pos_all.append(pos)
        dstrel_all[k, pos] = (d - (d // sbn) * sbn).astype(np.float64)

    T = etot // P
    Dw = dstrel_all.reshape(NCORES, T, P)
    valid_any = Dw.max(axis=2) >= 0
    with np.errstate(invalid="ignore"):
        wlo_c = np.where(Dw >= 0, Dw, np.inf).min(axis=2) // P
        whi_c = np.where(Dw >= 0, Dw, -np.inf).max(axis=2) // P
    wlo_t = np.where(valid_any, wlo_c, np.inf).min(axis=0)
    whi_t = np.where(valid_any, whi_c, -np.inf).max(axis=0)

    emits, uncovered = [], []
    tile_wlo = np.zeros(T, np.int64)
    for s in range(nsb):
        nwin = -(-ap.sb_nodes[s] // P)
        covered, sb_emits = set(), []
        for j in range(nblk):
            r = s * nblk + j
            o = int(offs[r])
            nt = int(ap.run_L[s, j] // P)
            for t in range(nt):
                g = o // P + t
                if np.isfinite(wlo_t[g]):
                    a = max(0, min(int(wlo_t[g]), nwin - 1))
                    b = max(a, min(int(whi_t[g]), nwin - 1))
                else:
                    a, b = 0, 0
                nw = b - a + 1
                assert nw <= MAXW, f"{name}: tile spans {nw} windows"
                covered.update(range(a, b + 1))
                tile_wlo[g] = a
                sb_emits.append((j, t, a, nw))
        emits.append(sb_emits)
        uncovered.append(sorted(set(range(nwin)) - covered))
    ap.emits, ap.uncovered = emits, uncovered

    idxs, dscs = [], []
    for k in range(NCORES):
        s, d, rid = per_core[k]
        pos = pos_all[k]
        idx16 = np.zeros(etot, np.int16)
        idx16[pos] = (s - (s // srcb) * srcb).astype(np.int16)
        dstrel = np.full(etot, -1000.0, np.float32)
        dstrel[pos] = (d - (d // sbn) * sbn).astype(np.float32)
        dstrel -= 128.0 * tile_wlo[np.arange(etot) // P]
        dstrel[dstrel < -1000.0] = -1000.0
        idxs.append(_wrap_idx(idx16))
        dscs.append(_pack_pcol(dstrel).astype(np.float16))
    ap.idx, ap.dsc = idxs, dscs
    return ap


# ---------------------------------------------------------------------------
# host prep: decoder labels
# ---------------------------------------------------------------------------

def prep_decoder(cfg, l_c, l_a):
    """Labels partitioned by customer owner; per article-block (4 of 25000),
    sorted by customer.  Chunked into dch with uniform per-(core,ablk)
    padding.  Q emissions use tile_wlo + MAXW window-relative encoding."""
    nab = cfg.nab
    core_l = l_c // cfg.cpc
    ablk = l_a // cfg.srcb_a
    gcounts = np.zeros((NCORES, nab), np.int64)
    per_core = []
    for k in range(NCORES):
        m = core_l == k
        lc, la, ab, orig = l_c[m], l_a[m], ablk[m], np.nonzero(m)[0]
        order = np.lexsort((lc, ab))
        lc, la, ab, orig = lc[order], la[order], ab[order], orig[order]
        gcounts[k] = np.bincount(ab, minlength=nab)
        per_core.append((lc, la, ab, orig))
    grp_L = _ru(gcounts.max(axis=0), cfg.dch)
    goffs = np.concatenate([[0], np.cumsum(grp_L)]).astype(np.int64)
    ld_pad = int(goffs[-1])

    # window-relative structure (union over cores)
    win_all = np.full((NCORES, ld_pad), -1, np.int64)
    idx_a_list, crel_list, out_pos = [], [], []
    pos_all = []
    for k in range(NCORES):
        lc, la, ab, orig = per_core[k]
        gstart = np.concatenate([[0], np.cumsum(gcounts[k])])[:-1]
        pos = goffs[ab] + (np.arange(len(lc)) - gstart[ab])
        pos_all.append(pos)
        win_all[k, pos] = (lc % cfg.cpc) // P

    T = ld_pad // P
    Ww = win_all.reshape(NCORES, T, P)
    valid_any = Ww.max(axis=2) >= 0
    wlo_c = np.where(Ww >= 0, Ww, np.inf).min(axis=2)
    whi_c = np.where(Ww >= 0, Ww, -np.inf).max(axis=2)
    wlo_t = np.where(valid_any, wlo_c, np.inf).min(axis=0)
    whi_t = np.where(valid_any, whi_c, -np.inf).max(axis=0)
    tile_wlo = np.zeros(T, np.int64)
    tile_nw = np.ones(T, np.int64)
    for t in range(T):
        if np.isfinite(wlo_t[t]):
            a = min(int(wlo_t[t]), cfg.nwin_uc - 1)
            b = min(int(whi_t[t]), cfg.nwin_uc - 1)
            nw = b - a + 1
            assert nw <= MAXW_DEC, f"dec tile spans {nw} windows"
            tile_wlo[t], tile_nw[t] = a, nw

    for k in range(NCORES):
        lc, la, ab, orig = per_core[k]
        pos = pos_all[k]
        ia = np.zeros(ld_pad, np.int16)
        ia[pos] = (la - (la // cfg.srcb_a) * cfg.srcb_a).astype(np.int16)
        crel = np.full(ld_pad, -1000.0, np.float32)
        crel[pos] = ((lc % cfg.cpc) - tile_wlo[pos // P] * P).astype(np.float32)
        po = np.full(ld_pad, -1, np.int64)
        po[pos] = orig
        idx_a_list.append(_wrap_idx(ia))
        # replicated across partitions: [128, ld_pad] fp16
        crel_list.append(np.tile(crel.astype(np.float16)[None, :], (P, 1)))
        out_pos.append(po)

    return dict(grp_L=grp_L, goffs=goffs, ld_pad=ld_pad, tile_wlo=tile_wlo,
                tile_nw=tile_nw, idx_a=idx_a_list, crel=crel_list,
                out_pos=out_pos)


def prep_all(cfg, inputs):
    i64 = lambda a: np.asarray(a).astype(np.int64)
    e_src = i64(inputs["edge_src_customer"])
    e_dst = i64(inputs["edge_dst_article"])
    l_c = i64(inputs["label_customer"])
    l_a = i64(inputs["label_article"])

    cnt_a = np.bincount(e_dst, minlength=cfg.n_a)
    cnt_c = np.bincount(e_src, minlength=cfg.n_c)
    scl_a = (1.0 / np.maximum(cnt_a, 1.0)).astype(np.float32)
    scl_c = (1.0 / np.maximum(cnt_c, 1.0)).astype(np.float32)

    # A1: dst = local articles (owner core), src = global customers
    pa1 = prep_agg_pass("A1", e_src, e_dst % cfg.apc, e_dst // cfg.apc,
                        cfg.apc, cfg.srcb_c, cfg.nblk_c, cfg.n_c, cfg.sbn)
    # Cf halves: dst = local customers (owner core), src = global articles
    dloc = e_src % cfg.cpc
    core_c = e_src // cfg.cpc
    half = (dloc >= cfg.chalf).astype(np.int64)
    pcf = []
    for h in range(2):
        m = half == h
        pcf.append(prep_agg_pass(f"Cf{h}", e_dst[m], dloc[m] - h * cfg.chalf,
                                 core_c[m], cfg.chalf, cfg.srcb_a, cfg.nblk_a,
                                 cfg.n_a, cfg.sbn_cf))
    # A2: src = local customers (owner core), dst = ALL articles
    pa2 = prep_agg_pass("A2", e_src % cfg.cpc, e_dst, core_c,
                        cfg.n_a, cfg.srcb_h, cfg.nblk_h, cfg.cpc, cfg.sbn)
    dec = prep_decoder(cfg, l_c, l_a)
    return pa1, pcf, pa2, dec, scl_a, scl_c


# ---------------------------------------------------------------------------
# kernel builder
# ---------------------------------------------------------------------------

def build_nc(cfg, pa1, pcf, pa2, dec, dbg=False):
    import os
    stage = int(os.environ.get("K2_STAGE", "5"))
    nc = bacc.Bacc("TRN2", target_bir_lowering=False, debug=False,
                   num_devices=NCORES, num_swdge_queues=4)
    qctr = [0]
    def next_q():
        qctr[0] = (qctr[0] + 1) % 4
        return qctr[0]

    ei = lambda n, s, d: nc.dram_tensor(n, s, d, kind="ExternalInput")
    xc = ei("xc", [cfg.n_c, P], F16)              # full customer rows
    xa_own = ei("xa_own", [cfg.apc, P], F16)      # own article rows
    xaT = ei("xaT", [P, cfg.apc], F16)            # own articles colmajor
    xcT = ei("xcT", [P, cfg.cpc], F16)            # own customers colmajor
    scl_a_rep = ei("scl_a_rep", [P, cfg.apc], F16)
    scl_c_rep = ei("scl_c_rep", [P, cfg.cpc], F16)
    a1_idx = ei("a1_idx", [P, pa1.etot // 16], I16)
    a1_dsc = ei("a1_dsc", [P, pa1.etot // P], F16)
    cf_idx = [ei(f"cf{h}_idx", [P, pcf[h].etot // 16], I16) for h in range(2)]
    cf_dsc = [ei(f"cf{h}_dsc", [P, pcf[h].etot // P], F16) for h in range(2)]
    a2_idx = ei("a2_idx", [P, pa2.etot // 16], I16)
    a2_dsc = ei("a2_dsc", [P, pa2.etot // P], F16)
    dec_idx_a = ei("dec_idx_a", [P, dec["ld_pad"] // 16], I16)
    dec_crel = ei("dec_crel", [P, dec["ld_pad"]], F16)

    wnames = ["W_msg1_ca", "W_self1_a", "W_msg1_ac", "W_self1_c",
              "W_msg2_ca", "W_self2_a", "W_msg2_ac", "W_self2_c",
              "Wd1c", "Wd1a"]
    wts = {n: ei(n, [P, P], F16) for n in wnames}
    w2r = ei("w2r", [P, 1024], F16)
    bnames = ["b1_a", "b1_c", "b2_a", "b2_c",
              "bn_gamma_c", "bn_beta_c", "bn_gamma_a", "bn_beta_a",
              "b_dec1", "b_dec2c"]
    bis = {n: ei(n, [P, 1], F32) for n in bnames}

    y_out = nc.dram_tensor("y", [P, dec["ld_pad"] // P], F32,
                           kind="ExternalOutput")
    dbg_outs = {}
    if dbg:
        for n, shp in [("d_comb", [cfg.n_a, 2 * P]), ("d_hcr", [cfg.cpc, P]),
                       ("d_zc", [P, cfg.cpc_pad]), ("d_za", [cfg.n_a, P]),
                       ("d_rs", [P, cfg.apc]), ("d_st", [P, 8])]:
            dbg_outs[n] = nc.dram_tensor(n, shp, F16 if n != "d_st" else F32,
                                         kind="ExternalOutput")
    rg = [list(range(NCORES))]

    with tile.TileContext(nc) as tc:
        with (
            tc.tile_pool(name="dramp", bufs=1, space="DRAM") as dramp,
            tc.tile_pool(name="const", bufs=1) as cs,
        ):
            comb_own = dramp.tile([cfg.apc, 2 * P], F16)
            comb_full = dramp.tile([cfg.n_a, 2 * P], F16, addr_space="Shared")
            haT_d = dramp.tile([P, cfg.apc], F16)
            hcrows_d = dramp.tile([cfg.cpc, P], F16)
            zcT_d = dramp.tile([P, cfg.cpc_pad], F16)
            partial_d = dramp.tile([NCORES, P, cfg.apc], F16)
            rs_out = dramp.tile([P, cfg.apc], F16)
            zaT_d = dramp.tile([P, cfg.apc], F16)
            za_own = dramp.tile([cfg.apc, P], F16)
            za_full = dramp.tile([cfg.n_a, P], F16, addr_space="Shared")
            stc_in = dramp.tile([P, 2], F32)
            stc_out = dramp.tile([P, 2], F32, addr_space="Shared")
            sta_in = dramp.tile([P, 2], F32)
            sta_out = dramp.tile([P, 2], F32, addr_space="Shared")

            # constants
            iota8 = []
            ii = cs.tile([P, 2048], mybir.dt.int32, name="ioi")
            for wi in range(MAXW):
                nc.gpsimd.iota(ii[:], pattern=[[0, 16], [1, P]],
                               base=wi * P, channel_multiplier=0)
                ff = cs.tile([P, 2048], F16, name=f"iof{wi}")
                nc.vector.tensor_copy(ff[:], ii[:])
                iota8.append(ff)
            # decoder: const tiles with value p + wi*128 everywhere
            iotaP4 = []
            iop_i = cs.tile([P, cfg.dch], mybir.dt.int32, name="iopi")
            for wi in range(MAXW_DEC):
                nc.gpsimd.iota(iop_i[:], pattern=[[0, cfg.dch]], base=wi * P,
                               channel_multiplier=1)
                qf = cs.tile([P, cfg.dch], F16, name=f"iopf{wi}")
                nc.vector.tensor_copy(qf[:], iop_i[:])
                iotaP4.append(qf)
            ident = cs.tile([P, P], F16)
            make_identity(nc, ident[:])
            w_sb = {n: cs.tile([P, P], F16, name=f"w_{n}") for n in wnames}
            for n in wnames:
                nc.sync.dma_start(out=w_sb[n][:], in_=wts[n][:])
            w2r_sb = cs.tile([P, cfg.dch], F16)
            nc.sync.dma_start(out=w2r_sb[:], in_=w2r[:, :cfg.dch])
            b_sb = {n: cs.tile([P, 1], F32, name=f"b_{n}") for n in bnames}
            for n in bnames:
                nc.sync.dma_start(out=b_sb[n][:], in_=bis[n][:])
            stc_sb = cs.tile([P, 2], F32)
            nc.vector.memset(stc_sb[:], 0.0)
            sta_sb = cs.tile([P, 2], F32)
            nc.vector.memset(sta_sb[:], 0.0)

            # copy own x_a rows into comb (h half filled by A1 W-stage)
            nc.sync.dma_start(out=comb_own[:, 0:P], in_=xa_own[:])

            # ----------------------------------------------------------------
            # generic aggregation superblock: gathers + one-hot matmuls
            # ----------------------------------------------------------------
            def agg_sb(ps, s, table, idx_d, dsc_d, psums, gch, esz, sbp, tag):
                """Accumulate superblock s of pass ps into psums (list of
                (psum_tile, lhsT_lo) pairs: lhsT slice [lo:lo+128] of the
                gathered row).  esz = row elements (128 or 256)."""
                offs = np.concatenate(
                    [[0], np.cumsum(ps.run_L.reshape(-1))]).astype(np.int64)
                touch = {}
                for (j, t, wlo, nw) in ps.emits[s]:
                    for wi in range(nw):
                        w = wlo + wi
                        touch.setdefault(w // 4, []).append((j, t, w))
                firsts = {b: v[0] for b, v in touch.items()}
                lasts = {b: v[-1] for b, v in touch.items()}
                by_run = {}
                for e in ps.emits[s]:
                    by_run.setdefault(e[0], []).append(e)
                tpc = gch // P
                for j in sorted(by_run):
                    r = s * ps.nblk + j
                    o = int(offs[r])
                    L = int(ps.run_L[s, j])
                    nt = L // P
                    blk_rows = min(ps.srcb, ps.nsrc_rows - j * ps.srcb)
                    idx_sb = sbp.tile([P, L // 16], I16, tag=f"{tag}gi",
                                      name=f"{tag}gi", bufs=3)
                    nc.scalar.dma_start(out=idx_sb[:],
                                        in_=idx_d[:, o // 16:(o + L) // 16])
                    dsc_sb = sbp.tile([P, nt], F16, tag=f"{tag}gd",
                                      name=f"{tag}gd", bufs=3)
                    nc.scalar.dma_start(out=dsc_sb[:],
                                        in_=dsc_d[:, o // P:o // P + nt])
                    run_emits = by_run[j]
                    for c0 in range(0, L, gch):
                        cl = min(gch, L - c0)
                        c = c0 // P          # first tile index of chunk
                        ctn = cl // P
                        x = sbp.tile([P, tpc, esz], F16, tag=f"{tag}gx",
                                     name=f"{tag}gx", bufs=8)
                        nc.gpsimd.dma_gather(
                            x[:, :ctn, :],
                            table[j * ps.srcb:j * ps.srcb + blk_rows, :],
                            idx_sb[:, c0 // 16:(c0 + cl) // 16],
                            cl, cl, esz, queue_num=next_q())
                        ch_emits = [e for e in run_emits
                                    if c <= e[1] < c + ctn]
                        maxnw = max(e[3] for e in ch_emits)
                        dstb = dsc_sb[:, c:c + ctn].to_broadcast([P, ctn, P])
                        p8s = {}
                        for wi in range(maxnw):
                            p8 = sbp.tile([P, gch], F16, tag=f"{tag}gp",
                                          name=f"{tag}gp", bufs=2 * MAXW)
                            p83 = p8[:, :ctn * P].rearrange(
                                "p (t w) -> p t w", w=P)
                            nc.vector.tensor_tensor(
                                out=p83,
                                in0=iota8[wi][:, :ctn * P].rearrange(
                                    "p (t w) -> p t w", w=P),
                                in1=dstb, op=mybir.AluOpType.is_equal)
                            p8s[wi] = p8
                        for (j2, t, wlo, nw) in ch_emits:
                            for wi in range(nw):
                                w = wlo + wi
                                p8 = p8s[wi]
                                first = firsts[w // 4] == (j2, t, w)
                                last = lasts[w // 4] == (j2, t, w)
                                for psum, lo in psums:
                                    nc.tensor.matmul(
                                        psum[:, w * P:(w + 1) * P],
                                        lhsT=x[:, t - c, lo:lo + P],
                                        rhs=p8[:, (t - c) * P:
                                               (t - c + 1) * P],
                                        start=first, stop=last,
                                        skip_group_check=True)

            def zero_uncovered(ps, s, dst_sb, nodes):
                for w in ps.uncovered[s]:
                    a, b = w * P, min(w * P + P, nodes)
                    nc.vector.memset(dst_sb[:, a:b], 0.0)

            def emit_rows(srcT_sb, cw, rows_dram, row_base, sbp, psp, tag,
                          col_lo=0, col_n=P):
                for b0 in range(0, cw, P):
                    bw = min(P, cw - b0)
                    tp = psp.tile([P, P], F16, tag=f"{tag}tp", name=f"{tag}tp",
                                  bufs=1)
                    nc.tensor.transpose(tp[:bw, :], srcT_sb[:, b0:b0 + bw],
                                        ident[:])
                    rows = sbp.tile([P, P], F16, tag=f"{tag}ro",
                                    name=f"{tag}ro", bufs=3)
                    nc.scalar.copy(rows[:bw, :], tp[:bw, :])
                    nc.sync.dma_start(
                        out=rows_dram[row_base + b0:row_base + b0 + bw,
                                      col_lo:col_lo + col_n],
                        in_=rows[:bw, :])

            # ================= A1: layer-1 articles =================
            with (
                tc.tile_pool(name="a1", bufs=1) as sbp,
                tc.tile_pool(name="a1p", bufs=1, space="PSUM") as psp,
            ):
                for s in range(pa1.nsb):
                    nodes = pa1.sb_nodes[s]
                    nwin = -(-nodes // P)
                    psum = psp.tile([P, nwin * P], F32, tag="a1ps",
                                    name="a1ps", bufs=1)
                    agg_sb(pa1, s, xc, a1_idx, a1_dsc, [(psum, 0)],
                           cfg.gch_a1, P, sbp, "a1")
                    # scaled copy psum -> meanT
                    mean_sb = sbp.tile([P, cfg.sbn], F16, tag="a1mn",
                                       name="a1mn", bufs=2)
                    scl_sb = sbp.tile([P, cfg.sbn], F16, tag="a1sc",
                                      name="a1sc", bufs=2)
                    nc.scalar.dma_start(
                        out=scl_sb[:, :nodes],
                        in_=scl_a_rep[:, s * cfg.sbn:s * cfg.sbn + nodes])
                    nc.vector.tensor_tensor(out=mean_sb[:, :nodes],
                                            in0=psum[:, :nodes],
                                            in1=scl_sb[:, :nodes],
                                            op=mybir.AluOpType.mult)
                    zero_uncovered(pa1, s, mean_sb, nodes)
                    # W-stage: haT = relu(Wmsg1^T meanT + Wself1^T xaT + b)
                    haT_sb = sbp.tile([P, cfg.sbn], F16, tag="a1h",
                                      name="a1h", bufs=2)
                    for c0 in range(0, nodes, 512):
                        cw = min(512, nodes - c0)
                        sT = sbp.tile([P, 512], F16, tag="a1sf", name="a1sf",
                                      bufs=3)
                        nc.scalar.dma_start(
                            out=sT[:, :cw],
                            in_=xaT[:, s * cfg.sbn + c0:s * cfg.sbn + c0 + cw])
                        wps = psp.tile([P, 512], F32, tag="a1wp", name="a1wp",
                                       bufs=2)
                        nc.tensor.matmul(wps[:, :cw], lhsT=w_sb["W_msg1_ca"][:],
                                         rhs=mean_sb[:, c0:c0 + cw],
                                         start=True, stop=False,
                                         skip_group_check=True)
                        nc.tensor.matmul(wps[:, :cw], lhsT=w_sb["W_self1_a"][:],
                                         rhs=sT[:, :cw], start=False, stop=True,
                                         skip_group_check=True)
                        nc.scalar.activation(
                            haT_sb[:, c0:c0 + cw], wps[:, :cw],
                            mybir.ActivationFunctionType.Relu,
                            bias=b_sb["b1_a"][:], scale=1.0)
                    nc.sync.dma_start(
                        out=haT_d[:, s * cfg.sbn:s * cfg.sbn + nodes],
                        in_=haT_sb[:, :nodes])
                    emit_rows(haT_sb, nodes, comb_own, s * cfg.sbn, sbp, psp,
                              "a1r", col_lo=P, col_n=P)

            # ================= AllGather comb =================
            nc.gpsimd.collective_compute(
                "AllGather", mybir.AluOpType.bypass, replica_groups=rg,
                ins=[comb_own[:]], outs=[comb_full[:]])

            # ================= C-fused: layers 1+2 customers =================
            for h in range(2 if stage >= 2 else 0):
                ps = pcf[h]
                with (
                    tc.tile_pool(name=f"cf{h}", bufs=1) as sbp,
                    tc.tile_pool(name=f"cf{h}p", bufs=1, space="PSUM") as psp,
                ):
                    for s in range(ps.nsb):
                        nodes = ps.sb_nodes[s]
                        nwin = -(-nodes // P)
                        base = h * cfg.chalf + s * cfg.sbn_cf
                        psum1 = psp.tile([P, nwin * P], F32, tag="cfp1",
                                         name="cfp1", bufs=1)
                        psum2 = psp.tile([P, nwin * P], F32, tag="cfp2",
                                         name="cfp2", bufs=1)
                        agg_sb(ps, s, comb_full, cf_idx[h], cf_dsc[h],
                               [(psum1, 0), (psum2, P)], cfg.gch_cf, 2 * P,
                               sbp, "cf")
                        scl_sb = sbp.tile([P, cfg.sbn_cf], F16, tag="cfsc",
                                          name="cfsc", bufs=2)
                        nc.scalar.dma_start(
                            out=scl_sb[:, :nodes],
                            in_=scl_c_rep[:, base:base + nodes])
                        mean1 = sbp.tile([P, cfg.sbn_cf], F16, tag="cfm1",
                                         name="cfm1", bufs=2)
                        nc.vector.tensor_tensor(out=mean1[:, :nodes],
                                                in0=psum1[:, :nodes],
                                                in1=scl_sb[:, :nodes],
                                                op=mybir.AluOpType.mult)
                        zero_uncovered(ps, s, mean1, nodes)
                        mean2 = sbp.tile([P, cfg.sbn_cf], F16, tag="cfm2",
                                         name="cfm2", bufs=2)
                        nc.vector.tensor_tensor(out=mean2[:, :nodes],
                                                in0=psum2[:, :nodes],
                                                in1=scl_sb[:, :nodes],
                                                op=mybir.AluOpType.mult)
                        zero_uncovered(ps, s, mean2, nodes)
                        hcT_sb = sbp.tile([P, cfg.sbn_cf], F16, tag="cfh",
                                          name="cfh", bufs=2)
                        zcT_sb = sbp.tile([P, cfg.sbn_cf], F16, tag="cfz",
                                          name="cfz", bufs=2)
                        for c0 in range(0, nodes, 512):
                            cw = min(512, nodes - c0)
                            sT = sbp.tile([P, 512], F16, tag="cfsf",
                                          name="cfsf", bufs=3)
                            nc.scalar.dma_start(
                                out=sT[:, :cw],
                                in_=xcT[:, base + c0:base + c0 + cw])
                            wps = psp.tile([P, 512], F32, tag="cfwp",
                                           name="cfwp", bufs=1)
                            nc.tensor.matmul(wps[:, :cw],
                                             lhsT=w_sb["W_msg1_ac"][:],
                                             rhs=mean1[:, c0:c0 + cw],
                                             start=True, stop=False,
                                             skip_group_check=True)
                            nc.tensor.matmul(wps[:, :cw],
                                             lhsT=w_sb["W_self1_c"][:],
                                             rhs=sT[:, :cw],
                                             start=False, stop=True,
                                             skip_group_check=True)
                            nc.scalar.activation(
                                hcT_sb[:, c0:c0 + cw], wps[:, :cw],
                                mybir.ActivationFunctionType.Relu,
                                bias=b_sb["b1_c"][:], scale=1.0)
                            wps2 = psp.tile([P, 512], F32, tag="cfw2",
                                            name="cfw2", bufs=1)
                            nc.tensor.matmul(wps2[:, :cw],
                                             lhsT=w_sb["W_msg2_ac"][:],
                                             rhs=mean2[:, c0:c0 + cw],
                                             start=True, stop=False,
                                             skip_group_check=True)
                            nc.tensor.matmul(wps2[:, :cw],
                                             lhsT=w_sb["W_self2_c"][:],
                                             rhs=hcT_sb[:, c0:c0 + cw],
                                             start=False, stop=True,
                                             skip_group_check=True)
                            nc.scalar.activation(
                                zcT_sb[:, c0:c0 + cw], wps2[:, :cw],
                                mybir.ActivationFunctionType.Identity,
                                bias=b_sb["b2_c"][:], scale=1.0)
                            # BN-c stats
                            part = sbp.tile([P, 1], F32, tag="cfs1",
                                            name="cfs1", bufs=2)
                            nc.vector.reduce_sum(part[:], zcT_sb[:, c0:c0 + cw],
                                                 mybir.AxisListType.X)
                            nc.vector.tensor_add(stc_sb[:, 0:1],
                                                 stc_sb[:, 0:1], part[:])
                            trash = sbp.tile([P, 512], F32, tag="cftr",
                                             name="cftr", bufs=2)
                            part2 = sbp.tile([P, 1], F32, tag="cfs2",
                                             name="cfs2", bufs=2)
                            nc.scalar.activation(
                                trash[:, :cw], zcT_sb[:, c0:c0 + cw],
                                mybir.ActivationFunctionType.Square,
                                accum_out=part2[:])
                            nc.vector.tensor_add(stc_sb[:, 1:2],
                                                 stc_sb[:, 1:2], part2[:])
                        nc.sync.dma_start(out=zcT_d[:, base:base + nodes],
                                          in_=zcT_sb[:, :nodes])
                        emit_rows(hcT_sb, nodes, hcrows_d, base, sbp, psp,
                                  "cfr")

            # zero zcT_d pad tail
            if stage >= 2 and cfg.cpc_pad > cfg.cpc:
                with tc.tile_pool(name="ztp", bufs=1) as sbp:
                    zt = sbp.tile([P, cfg.cpc_pad - cfg.cpc], F16, name="ztt")
                    nc.vector.memset(zt[:], 0.0)
                    nc.sync.dma_start(out=zcT_d[:, cfg.cpc:], in_=zt[:])

            # BN-c stats AllReduce (A2 overlaps with it)
            if stage >= 2:
                nc.sync.dma_start(out=stc_in[:], in_=stc_sb[:])
                nc.gpsimd.collective_compute(
                    "AllReduce", mybir.AluOpType.add, replica_groups=rg,
                    ins=[stc_in[:]], outs=[stc_out[:]])

            # ================= A2: partial article aggregates =================
            with (
                tc.tile_pool(name="a2", bufs=1) as sbp,
                tc.tile_pool(name="a2p", bufs=1, space="PSUM") as psp,
            ):
                for s in range(pa2.nsb if stage >= 3 else 0):
                    nodes = pa2.sb_nodes[s]
                    nwin = -(-nodes // P)
                    psum = psp.tile([P, nwin * P], F32, tag="a2ps",
                                    name="a2ps", bufs=2)
                    agg_sb(pa2, s, hcrows_d, a2_idx, a2_dsc, [(psum, 0)],
                           cfg.gch_a1, P, sbp, "a2")
                    stg = sbp.tile([P, cfg.sbn], F16, tag="a2st",
                                   name="a2st", bufs=3)
                    nc.vector.tensor_copy(stg[:, :nodes], psum[:, :nodes])
                    zero_uncovered(pa2, s, stg, nodes)
                    blk = s // cfg.sb_per_blk
                    col = (s % cfg.sb_per_blk) * cfg.sbn
                    nc.sync.dma_start(
                        out=partial_d[blk, :, col:col + nodes],
                        in_=stg[:, :nodes])

            # ================= ReduceScatter =================
            if stage >= 3:
                nc.gpsimd.collective_compute(
                    "ReduceScatter", mybir.AluOpType.add, replica_groups=rg,
                    ins=[partial_d[:]], outs=[rs_out[:]])

            # ---------------- BN coeff helper ----------------
            def bn_coeff(st_sb, n, gamma, beta, tagp, sbp):
                mu = sbp.tile([P, 1], F32, name=f"mu{tagp}")
                nc.vector.tensor_scalar_mul(mu[:], st_sb[:, 0:1], 1.0 / n)
                msq = sbp.tile([P, 1], F32, name=f"ms{tagp}")
                nc.vector.tensor_scalar_mul(msq[:], st_sb[:, 1:2], 1.0 / n)
                mu2 = sbp.tile([P, 1], F32, name=f"m2{tagp}")
                nc.vector.tensor_mul(mu2[:], mu[:], mu[:])
                var = sbp.tile([P, 1], F32, name=f"va{tagp}")
                nc.vector.tensor_sub(var[:], msq[:], mu2[:])
                nc.vector.tensor_scalar_add(var[:], var[:], BN_EPS)
                sd = sbp.tile([P, 1], F32, name=f"sd{tagp}")
                nc.scalar.activation(sd[:], var[:],
                                     mybir.ActivationFunctionType.Sqrt)
                rstd = sbp.tile([P, 1], F32, name=f"rs{tagp}")
                nc.vector.reciprocal(rstd[:], sd[:])
                scl = sbp.tile([P, 1], F32, name=f"sc{tagp}")
                nc.vector.tensor_mul(scl[:], b_sb[gamma][:], rstd[:])
                mg = sbp.tile([P, 1], F32, name=f"mg{tagp}")
                nc.vector.tensor_mul(mg[:], mu[:], scl[:])
                bia = sbp.tile([P, 1], F32, name=f"bi{tagp}")
                nc.vector.tensor_sub(bia[:], b_sb[beta][:], mg[:])
                return scl, bia

            with tc.tile_pool(name="tail", bufs=1) as keep:
                ucrows = keep.tile([P, cfg.cpc_pad], F16, name="ucrows")

                # ============ U_c build (overlaps RS wait) ============
                with (
                    tc.tile_pool(name="uc", bufs=1) as sbp,
                    tc.tile_pool(name="ucp", bufs=1, space="PSUM") as psp,
                ):
                  if stage >= 4:
                    st = sbp.tile([P, 2], F32, name="ucst")
                    nc.scalar.dma_start(out=st[:], in_=stc_out[:])
                    scl_c_col, bia_c_col = bn_coeff(
                        st, cfg.n_c, "bn_gamma_c", "bn_beta_c", "c", sbp)
                    for c0 in range(0, cfg.cpc_pad, 512):
                        cw = min(512, cfg.cpc_pad - c0)
                        zT = sbp.tile([P, 512], F16, tag="ucz", name="ucz",
                                      bufs=3)
                        nc.scalar.dma_start(out=zT[:, :cw],
                                            in_=zcT_d[:, c0:c0 + cw])
                        bnT = sbp.tile([P, 512], F16, tag="ucb", name="ucb",
                                       bufs=3)
                        nc.scalar.activation(
                            bnT[:, :cw], zT[:, :cw],
                            mybir.ActivationFunctionType.Identity,
                            bias=bia_c_col[:], scale=scl_c_col[:])
                        ups = psp.tile([P, 512], F32, tag="ucp", name="ucp",
                                       bufs=2)
                        nc.tensor.matmul(ups[:, :cw], lhsT=w_sb["Wd1c"][:],
                                         rhs=bnT[:, :cw], start=True,
                                         stop=True, skip_group_check=True)
                        uT = sbp.tile([P, 512], F16, tag="ucu", name="ucu",
                                      bufs=3)
                        nc.scalar.activation(
                            uT[:, :cw], ups[:, :cw],
                            mybir.ActivationFunctionType.Identity,
                            bias=b_sb["b_dec1"][:], scale=1.0)
                        for b0 in range(0, cw, P):
                            tp = psp.tile([P, P], F16, tag="uctp",
                                          name="uctp", bufs=2)
                            nc.tensor.transpose(tp[:], uT[:, b0:b0 + P],
                                                ident[:])
                            nc.scalar.copy(ucrows[:, c0 + b0:c0 + b0 + P],
                                           tp[:])

                # ============== z_a stage (after RS) ==============
                with (
                    tc.tile_pool(name="za", bufs=1) as sbp,
                    tc.tile_pool(name="zap", bufs=1, space="PSUM") as psp,
                ):
                    for c0 in range(0, cfg.apc if stage >= 4 else 0, 512):
                        cw = min(512, cfg.apc - c0)
                        rsT = sbp.tile([P, 512], F16, tag="zar", name="zar",
                                       bufs=3)
                        nc.scalar.dma_start(out=rsT[:, :cw],
                                            in_=rs_out[:, c0:c0 + cw])
                        sclT = sbp.tile([P, 512], F16, tag="zas", name="zas",
                                        bufs=3)
                        nc.scalar.dma_start(out=sclT[:, :cw],
                                            in_=scl_a_rep[:, c0:c0 + cw])
                        m2 = sbp.tile([P, 512], F16, tag="zam", name="zam",
                                      bufs=3)
                        nc.vector.tensor_tensor(out=m2[:, :cw],
                                                in0=rsT[:, :cw],
                                                in1=sclT[:, :cw],
                                                op=mybir.AluOpType.mult)
                        hT = sbp.tile([P, 512], F16, tag="zah", name="zah",
                                      bufs=3)
                        nc.scalar.dma_start(out=hT[:, :cw],
                                            in_=haT_d[:, c0:c0 + cw])
                        wps = psp.tile([P, 512], F32, tag="zap", name="zap",
                                       bufs=2)
                        nc.tensor.matmul(wps[:, :cw],
                                         lhsT=w_sb["W_msg2_ca"][:],
                                         rhs=m2[:, :cw], start=True,
                                         stop=False, skip_group_check=True)
                        nc.tensor.matmul(wps[:, :cw],
                                         lhsT=w_sb["W_self2_a"][:],
                                         rhs=hT[:, :cw], start=False,
                                         stop=True, skip_group_check=True)
                        zaT = sbp.tile([P, 512], F16, tag="zaz", name="zaz",
                                       bufs=3)
                        nc.scalar.activation(
                            zaT[:, :cw], wps[:, :cw],
                            mybir.ActivationFunctionType.Identity,
                            bias=b_sb["b2_a"][:], scale=1.0)
                        part = sbp.tile([P, 1], F32, tag="zs1", name="zs1",
                                        bufs=2)
                        nc.vector.reduce_sum(part[:], zaT[:, :cw],
                                             mybir.AxisListType.X)
                        nc.vector.tensor_add(sta_sb[:, 0:1], sta_sb[:, 0:1],
                                             part[:])
                        trash = sbp.tile([P, 512], F32, tag="ztr", name="ztr",
                                         bufs=2)
                        part2 = sbp.tile([P, 1], F32, tag="zs2", name="zs2",
                                         bufs=2)
                        nc.scalar.activation(
                            trash[:, :cw], zaT[:, :cw],
                            mybir.ActivationFunctionType.Square,
                            accum_out=part2[:])
                        nc.vector.tensor_add(sta_sb[:, 1:2], sta_sb[:, 1:2],
                                             part2[:])
                        nc.sync.dma_start(out=zaT_d[:, c0:c0 + cw],
                                          in_=zaT[:, :cw])

                if stage >= 4:
                    nc.sync.dma_start(out=sta_in[:], in_=sta_sb[:])
                    nc.gpsimd.collective_compute(
                        "AllReduce", mybir.AluOpType.add, replica_groups=rg,
                        ins=[sta_in[:]], outs=[sta_out[:]])
                    # V_a = bn_a(z_a) @ Wd1a, as rows -> AllGather
                    with (
                        tc.tile_pool(name="va", bufs=1) as sbp,
                        tc.tile_pool(name="vap", bufs=1, space="PSUM") as psp,
                    ):
                        sta2 = sbp.tile([P, 2], F32, name="vast")
                        nc.scalar.dma_start(out=sta2[:], in_=sta_out[:])
                        scl_a_col, bia_a_col = bn_coeff(
                            sta2, cfg.n_a, "bn_gamma_a", "bn_beta_a", "a",
                            sbp)
                        for c0 in range(0, cfg.apc, 512):
                            cw = min(512, cfg.apc - c0)
                            zT = sbp.tile([P, 512], F16, tag="vaz",
                                          name="vaz", bufs=3)
                            nc.scalar.dma_start(out=zT[:, :cw],
                                                in_=zaT_d[:, c0:c0 + cw])
                            bnT = sbp.tile([P, 512], F16, tag="vab",
                                           name="vab", bufs=3)
                            nc.scalar.activation(
                                bnT[:, :cw], zT[:, :cw],
                                mybir.ActivationFunctionType.Identity,
                                bias=bia_a_col[:], scale=scl_a_col[:])
                            vps = psp.tile([P, 512], F32, tag="vap",
                                           name="vap", bufs=2)
                            nc.tensor.matmul(vps[:, :cw],
                                             lhsT=w_sb["Wd1a"][:],
                                             rhs=bnT[:, :cw], start=True,
                                             stop=True,
                                             skip_group_check=True)
                            vaT = sbp.tile([P, 512], F16, tag="vav",
                                           name="vav", bufs=3)
                            nc.scalar.copy(vaT[:, :cw], vps[:, :cw])
                            emit_rows(vaT, cw, za_own, c0, sbp, psp, "var")
                    nc.gpsimd.collective_compute(
                        "AllGather", mybir.AluOpType.bypass, replica_groups=rg,
                        ins=[za_own[:]], outs=[za_full[:]])

                if dbg:
                    nc.sync.dma_start(out=dbg_outs["d_comb"][:],
                                      in_=comb_full[:])
                    nc.sync.dma_start(out=dbg_outs["d_hcr"][:],
                                      in_=hcrows_d[:])
                    nc.sync.dma_start(out=dbg_outs["d_zc"][:], in_=zcT_d[:])
                    nc.sync.dma_start(out=dbg_outs["d_za"][:], in_=za_full[:])
                    nc.sync.dma_start(out=dbg_outs["d_rs"][:], in_=rs_out[:])
                    nc.sync.dma_start(out=dbg_outs["d_st"][:, 0:2],
                                      in_=stc_out[:])
                    nc.sync.dma_start(out=dbg_outs["d_st"][:, 2:4],
                                      in_=sta_out[:])

                # ================= decoder =================
                with (
                    tc.tile_pool(name="dc", bufs=1) as sbp,
                    tc.tile_pool(name="dcp", bufs=1, space="PSUM") as psp,
                ):
                  if stage >= 5:
                    sta_sb2 = sbp.tile([P, 2], F32, name="dsta")
                    nc.scalar.dma_start(out=sta_sb2[:], in_=sta_out[:])
                    scl_a_col, bia_a_col = bn_coeff(
                        sta_sb2, cfg.n_a, "bn_gamma_a", "bn_beta_a", "a", sbp)
                    goffs, grp_L = dec["goffs"], dec["grp_L"]
                    tile_wlo, tile_nw = dec["tile_wlo"], dec["tile_nw"]
                    for ab in range(cfg.nab):
                        o0, L = int(goffs[ab]), int(grp_L[ab])
                        blk_rows = min(cfg.srcb_a, cfg.n_a - ab * cfg.srcb_a)
                        for c0 in range(o0, o0 + L, cfg.dch):
                            cl = min(cfg.dch, o0 + L - c0)
                            ctn = cl // P
                            ixa = sbp.tile([P, cfg.dch // 16], I16, tag="dia",
                                           name="dia", bufs=4)
                            nc.scalar.dma_start(
                                out=ixa[:, :cl // 16],
                                in_=dec_idx_a[:, c0 // 16:(c0 + cl) // 16])
                            zg = sbp.tile([P, 1, cfg.dch], F16, tag="dzg",
                                          name="dzg", bufs=4)
                            nc.gpsimd.dma_gather(
                                zg[:, :, :cl],
                                za_full[ab * cfg.srcb_a:
                                        ab * cfg.srcb_a + blk_rows, :],
                                ixa[:, :cl // 16], cl, cl, P,
                                transpose=True, queue_num=next_q())
                            bnz = sbp.tile([P, cfg.dch], F16, tag="dbn",
                                           name="dbn", bufs=4)
                            nc.scalar.activation(
                                bnz[:, :cl], zg[:, 0, :cl],
                                mybir.ActivationFunctionType.Identity,
                                bias=bia_a_col[:], scale=scl_a_col[:])
                            crel = sbp.tile([P, cfg.dch], F16, tag="dcr",
                                            name="dcr", bufs=4)
                            nc.scalar.dma_start(out=crel[:, :cl],
                                                in_=dec_crel[:, c0:c0 + cl])
                            spsum = psp.tile([P, cfg.dch], F32, tag="dsp",
                                             name="dsp", bufs=2)
                            emlist = []
                            for cc in range(0, cl, 512):
                                emlist.append(("w", cc, min(512, cl - cc)))
                            for t in range(ctn):
                                g = c0 // P + t
                                for wi in range(int(tile_nw[g])):
                                    emlist.append(("q", t, wi))
                            banks = {}
                            for em in emlist:
                                if em[0] == "w":
                                    bset = set(range(
                                        em[1] // 512,
                                        (em[1] + em[2] - 1) // 512 + 1))
                                else:
                                    bset = {em[1] * P // 512}
                                for b in bset:
                                    banks.setdefault(b, []).append(em)
                            firsts = {b: v[0] for b, v in banks.items()}
                            lasts = {b: v[-1] for b, v in banks.items()}
                            for em in emlist:
                                if em[0] == "w":
                                    _, cc, cww = em
                                    b = cc // 512
                                    nc.tensor.matmul(
                                        spsum[:, cc:cc + cww],
                                        lhsT=w_sb["Wd1a"][:],
                                        rhs=bnz[:, cc:cc + cww],
                                        start=firsts[b] == em,
                                        stop=lasts[b] == em,
                                        skip_group_check=True)
                            qts = {}
                            for t in range(ctn):
                                g = c0 // P + t
                                for wi in range(int(tile_nw[g])):
                                    if wi not in qts:
                                        q = sbp.tile([P, cfg.dch], F16,
                                                     tag=f"dq{wi}",
                                                     name=f"dq{wi}", bufs=2)
                                        nc.vector.tensor_tensor(
                                            out=q[:, :cl],
                                            in0=iotaP4[wi][:, :cl],
                                            in1=crel[:, :cl],
                                            op=mybir.AluOpType.is_equal)
                                        qts[wi] = q
                            for em in emlist:
                                if em[0] == "q":
                                    _, t, wi = em
                                    g = c0 // P + t
                                    w = int(tile_wlo[g]) + wi
                                    b = t * P // 512
                                    nc.tensor.matmul(
                                        spsum[:, t * P:(t + 1) * P],
                                        lhsT=ucrows[:, w * P:(w + 1) * P],
                                        rhs=qts[wi][:, t * P:(t + 1) * P],
                                        start=firsts[b] == em,
                                        stop=lasts[b] == em,
                                        skip_group_check=True)
                            relu_sb = sbp.tile([P, cfg.dch], F16, tag="drl",
                                               name="drl", bufs=3)
                            nc.scalar.activation(
                                relu_sb[:, :cl], spsum[:, :cl],
                                mybir.ActivationFunctionType.Relu)
                            yp = psp.tile([P, cfg.dch], F32, tag="dyp",
                                          name="dyp", bufs=1)
                            for cc in range(0, cl, 512):
                                cww = min(512, cl - cc)
                                nc.tensor.matmul(yp[0:1, cc:cc + cww],
                                                 lhsT=w2_sb[:],
                                                 rhs=relu_sb[:, cc:cc + cww],
                                                 start=True, stop=True,
                                                 skip_group_check=True)
                            ysb = sbp.tile([1, cfg.dch], F32, tag="dys",
                                           name="dys", bufs=3)
                            nc.scalar.copy(ysb[:, :cl], yp[0:1, :cl])
                            nc.sync.dma_start(out=y_out[:, c0:c0 + cl],
                                              in_=ysb[:, :cl])

    nc.compile()
    return nc


# ---------------------------------------------------------------------------
# entry point
# ---------------------------------------------------------------------------

def make_in_maps(cfg, inputs, pa1, pcf, pa2, dec, scl_a, scl_c):
    f = lambda a: np.ascontiguousarray(np.asarray(a), dtype=np.float32)
    xc16 = f(inputs["x_customer"]).astype(np.float16)
    xa16 = f(inputs["x_article"]).astype(np.float16)
    wd1 = f(inputs["W_dec1"])
    base = dict(
        xc=xc16,
        W_msg1_ca=f(inputs["W_msg1_ca"]).astype(np.float16),
        W_self1_a=f(inputs["W_self1_a"]).astype(np.float16),
        W_msg1_ac=f(inputs["W_msg1_ac"]).astype(np.float16),
        W_self1_c=f(inputs["W_self1_c"]).astype(np.float16),
        W_msg2_ca=f(inputs["W_msg2_ca"]).astype(np.float16),
        W_self2_a=f(inputs["W_self2_a"]).astype(np.float16),
        W_msg2_ac=f(inputs["W_msg2_ac"]).astype(np.float16),
        W_self2_c=f(inputs["W_self2_c"]).astype(np.float16),
        Wd1c=wd1[:P].astype(np.float16), Wd1a=wd1[P:].astype(np.float16),
        w2r=np.tile(f(inputs["W_dec2"]).reshape(1, P),
                    (P, 8)).astype(np.float16),
        b1_a=f(inputs["b1_a"]).reshape(P, 1),
        b1_c=f(inputs["b1_c"]).reshape(P, 1),
        b2_a=f(inputs["b2_a"]).reshape(P, 1),
        b2_c=f(inputs["b2_c"]).reshape(P, 1),
        bn_gamma_c=f(inputs["bn_gamma_c"]).reshape(P, 1),
        bn_beta_c=f(inputs["bn_beta_c"]).reshape(P, 1),
        bn_gamma_a=f(inputs["bn_gamma_a"]).reshape(P, 1),
        bn_beta_a=f(inputs["bn_beta_a"]).reshape(P, 1),
        b_dec1=f(inputs["b_dec1"]).reshape(P, 1),
        b_dec2c=np.full((P, 1), float(np.asarray(inputs["b_dec2"]).item()),
                        np.float32),
    )
    in_maps = []
    for k in range(NCORES):
        m = dict(base)
        m["xa_own"] = np.ascontiguousarray(xa16[k * cfg.apc:(k + 1) * cfg.apc])
        m["xaT"] = np.ascontiguousarray(
            xa16[k * cfg.apc:(k + 1) * cfg.apc].T)
        m["xcT"] = np.ascontiguousarray(
            xc16[k * cfg.cpc:(k + 1) * cfg.cpc].T)
        m["scl_a_rep"] = np.tile(
            scl_a[k * cfg.apc:(k + 1) * cfg.apc].astype(np.float16)[None, :],
            (P, 1))
        m["scl_c_rep"] = np.tile(
            scl_c[k * cfg.cpc:(k + 1) * cfg.cpc].astype(np.float16)[None, :],
            (P, 1))
        m["a1_idx"], m["a1_dsc"] = pa1.idx[k], pa1.dsc[k]
        for h in range(2):
            m[f"cf{h}_idx"], m[f"cf{h}_dsc"] = pcf[h].idx[k], pcf[h].dsc[k]
        m["a2_idx"], m["a2_dsc"] = pa2.idx[k], pa2.dsc[k]
        m["dec_idx_a"], m["dec_crel"] = dec["idx_a"][k], dec["crel"][k]
        in_maps.append(m)
    return in_maps


def run(cfg, inputs, trace=False, dbg=False):
    pa1, pcf, pa2, dec, scl_a, scl_c = prep_all(cfg, inputs)
    in_maps = make_in_maps(cfg, inputs, pa1, pcf, pa2, dec, scl_a, scl_c)
    nc = build_nc(cfg, pa1, pcf, pa2, dec, dbg=dbg)
    res = run_bass_kernel_spmd(nc, in_maps, core_ids=list(range(NCORES)),
                               trace=trace)
    y = np.empty(cfg.e_lbl, np.float32)
    b2 = float(np.asarray(inputs["b_dec2"]).item())
    for k in range(NCORES):
        yl = res.results[k]["y"].T.reshape(-1) + b2
        po = dec["out_pos"][k]
        vm = po >= 0
        y[po[vm]] = yl[vm]
    return y, res


def kernel(**inputs):
    cfg = Cfg()
    y, _ = run(cfg, inputs, trace=False)
    return y


# revision 4
# speedup vs baseline: 1.4502x; 1.2013x over previous
"""Hetero GNN encoder/decoder v2 - restructured to minimize SWDGE descgen + DVE.

Key changes vs v1:
  - Pass order: A1 (dst=article-owner) -> AllGather comb[x_a|h_a] ->
    C-fused (C1+C2 share one 512B gather + one-hot P) -> A2
    (src=customer-owner, gathers LOCAL h_c rows, partial agg over all
    articles) -> ReduceScatter -> z_a -> AllGather z_a rows -> decoder.
  - One-hot P = is_equal only; 1/cnt scale applied at psum->SBUF copy
    via host-replicated per-column scale tables.
  - Decoder: transpose-gather of z_a (column-major), BN via per-partition
    scalar activation, Wd1a matmul + U_c window one-hot matmuls accumulate
    into one PSUM, w2 reduction via M=1 matmul.  No hc/ua AllGathers.
"""
import sys

sys.path.insert(0, "/opt/trn_rl_repo")

import numpy as np

import concourse.bacc as bacc
import concourse.bass as bass
import concourse.mybir as mybir
import concourse.tile as tile
from concourse.bass_utils import run_bass_kernel_spmd
from concourse.masks import make_identity

P = 128
NCORES = 8
MAXW = 4
MAXW_DEC = 8
BN_EPS = 1e-5
F32 = mybir.dt.float32
F16 = mybir.dt.float16
I16 = mybir.dt.int16


class Cfg:
    def __init__(self, small=False):
        if small:
            self.n_c, self.n_a, self.e_lbl = 6144, 1024, 8192
            self.sbn, self.sbn_cf = 128, 128
            self.srcb_c, self.srcb_a, self.srcb_h = 1024, 256, 384
            self.gch_a1, self.gch_cf, self.dch = 512, 256, 256
        else:
            self.n_c, self.n_a, self.e_lbl = 300000, 100000, 1000000
            self.sbn = 1250                 # superblock nodes (divides 12500)
            self.sbn_cf = 1024              # Cf superblock (2 psums, bank fit)
            self.srcb_c = 30000             # A1 src block (customers)
            self.srcb_a = 25000             # Cf src block (articles)
            self.srcb_h = 18750             # A2 src block (local customers)
            self.gch_a1 = 1024              # idx per gather, A1/A2 (256B rows)
            self.gch_cf = 1024              # idx per gather, Cf (512B rows)
            self.dch = 1024                 # decoder labels per chunk
        self.cpc, self.apc = self.n_c // NCORES, self.n_a // NCORES
        self.chalf = self.cpc // 2
        self.nblk_c = -(-self.n_c // self.srcb_c)
        self.nblk_a = -(-self.n_a // self.srcb_a)
        self.nblk_h = -(-self.cpc // self.srcb_h)
        self.nab = self.nblk_a
        assert self.apc % self.sbn == 0
        self.sb_per_blk = self.apc // self.sbn
        self.nwin_uc = -(-self.cpc // P)
        self.cpc_pad = self.nwin_uc * P


def _ru(x, m):
    return (x + m - 1) // m * m


def _wrap_idx(flat):
    """[n] int -> [128, n/16] wrap (16-partition layout, replicated x8)."""
    n = flat.shape[0]
    w = flat.astype(np.int16).reshape(n // 16, 16).T
    return np.tile(w, (8, 1))


def _pack_pcol(a):
    """[n] -> [128, n/128]: element i -> partition i%128, col i//128."""
    return np.ascontiguousarray(a.reshape(-1, P).T)


# ---------------------------------------------------------------------------
# host prep: one aggregation pass (uniform SPMD structure across cores)
# ---------------------------------------------------------------------------

class AggPass:
    def __init__(self, name, nloc, srcb, nblk, nsrc_rows, sbn):
        self.name, self.nloc, self.srcb = name, nloc, srcb
        self.nblk, self.nsrc_rows, self.sbn = nblk, nsrc_rows, sbn
        self.nsb = -(-nloc // sbn)
        self.sb_nodes = [min(sbn, nloc - s * sbn) for s in range(self.nsb)]


def prep_agg_pass(name, src, dst_loc, core_e, nloc, srcb, nblk, nsrc_rows, sbn):
    """Edges (src gathered, dst accumulated into nloc-range) per core.

    Returns AggPass with: run_L [nsb,nblk] uniform padded lengths, emits
    (per sb: list of (j, t, wlo, nw)), uncovered windows, per-core idx
    (wrapped int16) and dsc (dst-rel fp16, [128, etot/128])."""
    ap = AggPass(name, nloc, srcb, nblk, nsrc_rows, sbn)
    nsb = ap.nsb
    nruns = nsb * nblk

    per_core, counts = [], np.zeros((NCORES, nruns), np.int64)
    for k in range(NCORES):
        m = core_e == k
        s, d = src[m], dst_loc[m]
        j = s // srcb
        sb = d // sbn
        order = np.lexsort((d, j, sb))
        s, d, j, sb = s[order], d[order], j[order], sb[order]
        rid = sb * nblk + j
        counts[k] = np.bincount(rid, minlength=nruns)
        per_core.append((s, d, rid))

    run_L = _ru(counts.max(axis=0), P)
    offs = np.concatenate([[0], np.cumsum(run_L)]).astype(np.int64)
    etot = int(offs[-1])
    ap.run_L = run_L.reshape(nsb, nblk)
    ap.etot = etot

    dstrel_all = np.full((NCORES, etot), -1.0e9, np.float64)
    pos_all = []
    for k in range(NCORES):
        s, d, rid = per_core[k]
        run_start = np.concatenate([[0], np.cumsum(counts[k])])[:-1]
        pos = offs[rid] + (np.arange(len(s)) - run_start[rid])
        pos_all.append(pos)
        dstrel_all[k, pos] = (d - (d // sbn) * sbn).astype(np.float64)

    T = etot // P
    Dw = dstrel_all.reshape(NCORES, T, P)
    valid_any = Dw.max(axis=2) >= 0
    with np.errstate(invalid="ignore"):
        wlo_c = np.where(Dw >= 0, Dw, np.inf).min(axis=2) // P
        whi_c = np.where(Dw >= 0, Dw, -np.inf).max(axis=2) // P
    wlo_t = np.where(valid_any, wlo_c, np.inf).min(axis=0)
    whi_t = np.where(valid_any, whi_c, -np.inf).max(axis=0)

    emits, uncovered = [], []
    tile_wlo = np.zeros(T, np.int64)
    for s in range(nsb):
        nwin = -(-ap.sb_nodes[s] // P)
        covered, sb_emits = set(), []
        for j in range(nblk):
            r = s * nblk + j
            o = int(offs[r])
            nt = int(ap.run_L[s, j] // P)
            for t in range(nt):
                g = o // P + t
                if np.isfinite(wlo_t[g]):
                    a = max(0, min(int(wlo_t[g]), nwin - 1))
                    b = max(a, min(int(whi_t[g]), nwin - 1))
                else:
                    a, b = 0, 0
                nw = b - a + 1
                assert nw <= MAXW, f"{name}: tile spans {nw} windows"
                covered.update(range(a, b + 1))
                tile_wlo[g] = a
                sb_emits.append((j, t, a, nw))
        emits.append(sb_emits)
        uncovered.append(sorted(set(range(nwin)) - covered))
    ap.emits, ap.uncovered = emits, uncovered

    idxs, dscs = [], []
    for k in range(NCORES):
        s, d, rid = per_core[k]
        pos = pos_all[k]
        idx16 = np.zeros(etot, np.int16)
        idx16[pos] = (s - (s // srcb) * srcb).astype(np.int16)
        dstrel = np.full(etot, -1000.0, np.float32)
        dstrel[pos] = (d - (d // sbn) * sbn).astype(np.float32)
        dstrel -= 128.0 * tile_wlo[np.arange(etot) // P]
        dstrel[dstrel < -1000.0] = -1000.0
        idxs.append(_wrap_idx(idx16))
        dscs.append(_pack_pcol(dstrel).astype(np.float16))
    ap.idx, ap.dsc = idxs, dscs
    return ap


# ---------------------------------------------------------------------------
# host prep: decoder labels
# ---------------------------------------------------------------------------

def prep_decoder(cfg, l_c, l_a):
    """Labels partitioned by customer owner; per article-block (4 of 25000),
    sorted by customer.  Chunked into dch with uniform per-(core,ablk)
    padding.  Q emissions use tile_wlo + MAXW window-relative encoding."""
    nab = cfg.nab
    core_l = l_c // cfg.cpc
    ablk = l_a // cfg.srcb_a
    gcounts = np.zeros((NCORES, nab), np.int64)
    per_core = []
    for k in range(NCORES):
        m = core_l == k
        lc, la, ab, orig = l_c[m], l_a[m], ablk[m], np.nonzero(m)[0]
        order = np.lexsort((lc, ab))
        lc, la, ab, orig = lc[order], la[order], ab[order], orig[order]
        gcounts[k] = np.bincount(ab, minlength=nab)
        per_core.append((lc, la, ab, orig))
    grp_L = _ru(gcounts.max(axis=0), cfg.dch)
    goffs = np.concatenate([[0], np.cumsum(grp_L)]).astype(np.int64)
    ld_pad = int(goffs[-1])

    # window-relative structure (union over cores)
    win_all = np.full((NCORES, ld_pad), -1, np.int64)
    idx_a_list, crel_list, out_pos = [], [], []
    pos_all = []
    for k in range(NCORES):
        lc, la, ab, orig = per_core[k]
        gstart = np.concatenate([[0], np.cumsum(gcounts[k])])[:-1]
        pos = goffs[ab] + (np.arange(len(lc)) - gstart[ab])
        pos_all.append(pos)
        win_all[k, pos] = (lc % cfg.cpc) // P

    T = ld_pad // P
    Ww = win_all.reshape(NCORES, T, P)
    valid_any = Ww.max(axis=2) >= 0
    wlo_c = np.where(Ww >= 0, Ww, np.inf).min(axis=2)
    whi_c = np.where(Ww >= 0, Ww, -np.inf).max(axis=2)
    wlo_t = np.where(valid_any, wlo_c, np.inf).min(axis=0)
    whi_t = np.where(valid_any, whi_c, -np.inf).max(axis=0)
    tile_wlo = np.zeros(T, np.int64)
    tile_nw = np.ones(T, np.int64)
    for t in range(T):
        if np.isfinite(wlo_t[t]):
            a = min(int(wlo_t[t]), cfg.nwin_uc - 1)
            b = min(int(whi_t[t]), cfg.nwin_uc - 1)
            nw = b - a + 1
            assert nw <= MAXW_DEC, f"dec tile spans {nw} windows"
            tile_wlo[t], tile_nw[t] = a, nw

    for k in range(NCORES):
        lc, la, ab, orig = per_core[k]
        pos = pos_all[k]
        ia = np.zeros(ld_pad, np.int16)
        ia[pos] = (la - (la // cfg.srcb_a) * cfg.srcb_a).astype(np.int16)
        crel = np.full(ld_pad, -1000.0, np.float32)
        crel[pos] = ((lc % cfg.cpc) - tile_wlo[pos // P] * P).astype(np.float32)
        po = np.full(ld_pad, -1, np.int64)
        po[pos] = orig
        idx_a_list.append(_wrap_idx(ia))
        # replicated across partitions: [128, ld_pad] fp16
        crel_list.append(np.tile(crel.astype(np.float16)[None, :], (P, 1)))
        out_pos.append(po)

    return dict(grp_L=grp_L, goffs=goffs, ld_pad=ld_pad, tile_wlo=tile_wlo,
                tile_nw=tile_nw, idx_a=idx_a_list, crel=crel_list,
                out_pos=out_pos)


def prep_all(cfg, inputs):
    i64 = lambda a: np.asarray(a).astype(np.int64)
    e_src = i64(inputs["edge_src_customer"])
    e_dst = i64(inputs["edge_dst_article"])
    l_c = i64(inputs["label_customer"])
    l_a = i64(inputs["label_article"])

    cnt_a = np.bincount(e_dst, minlength=cfg.n_a)
    cnt_c = np.bincount(e_src, minlength=cfg.n_c)
    scl_a = (1.0 / np.maximum(cnt_a, 1.0)).astype(np.float32)
    scl_c = (1.0 / np.maximum(cnt_c, 1.0)).astype(np.float32)

    # A1: dst = local articles (owner core), src = global customers
    pa1 = prep_agg_pass("A1", e_src, e_dst % cfg.apc, e_dst // cfg.apc,
                        cfg.apc, cfg.srcb_c, cfg.nblk_c, cfg.n_c, cfg.sbn)
    # Cf halves: dst = local customers (owner core), src = global articles
    dloc = e_src % cfg.cpc
    core_c = e_src // cfg.cpc
    half = (dloc >= cfg.chalf).astype(np.int64)
    pcf = []
    for h in range(2):
        m = half == h
        pcf.append(prep_agg_pass(f"Cf{h}", e_dst[m], dloc[m] - h * cfg.chalf,
                                 core_c[m], cfg.chalf, cfg.srcb_a, cfg.nblk_a,
                                 cfg.n_a, cfg.sbn_cf))
    # A2: src = local customers (owner core), dst = ALL articles
    pa2 = prep_agg_pass("A2", e_src % cfg.cpc, e_dst, core_c,
                        cfg.n_a, cfg.srcb_h, cfg.nblk_h, cfg.cpc, cfg.sbn)
    dec = prep_decoder(cfg, l_c, l_a)
    return pa1, pcf, pa2, dec, scl_a, scl_c


# ---------------------------------------------------------------------------
# kernel builder
# ---------------------------------------------------------------------------

def build_nc(cfg, pa1, pcf, pa2, dec, dbg=False):
    import os
    stage = int(os.environ.get("K2_STAGE", "5"))
    nc = bacc.Bacc("TRN2", target_bir_lowering=False, debug=False,
                   num_devices=NCORES, num_swdge_queues=4)
    qctr = [0]
    def next_q():
        qctr[0] = (qctr[0] + 1) % 4
        return qctr[0]

    ei = lambda n, s, d: nc.dram_tensor(n, s, d, kind="ExternalInput")
    xc = ei("xc", [cfg.n_c, P], F16)              # full customer rows
    xa_own = ei("xa_own", [cfg.apc, P], F16)      # own article rows
    xaT = ei("xaT", [P, cfg.apc], F16)            # own articles colmajor
    xcT = ei("xcT", [P, cfg.cpc], F16)            # own customers colmajor
    scl_a_rep = ei("scl_a_rep", [P, cfg.apc], F16)
    scl_c_rep = ei("scl_c_rep", [P, cfg.cpc], F16)
    a1_idx = ei("a1_idx", [P, pa1.etot // 16], I16)
    a1_dsc = ei("a1_dsc", [P, pa1.etot // P], F16)
    cf_idx = [ei(f"cf{h}_idx", [P, pcf[h].etot // 16], I16) for h in range(2)]
    cf_dsc = [ei(f"cf{h}_dsc", [P, pcf[h].etot // P], F16) for h in range(2)]
    a2_idx = ei("a2_idx", [P, pa2.etot // 16], I16)
    a2_dsc = ei("a2_dsc", [P, pa2.etot // P], F16)
    dec_idx_a = ei("dec_idx_a", [P, dec["ld_pad"] // 16], I16)
    dec_crel = ei("dec_crel", [P, dec["ld_pad"]], F16)

    wnames = ["W_msg1_ca", "W_self1_a", "W_msg1_ac", "W_self1_c",
              "W_msg2_ca", "W_self2_a", "W_msg2_ac", "W_self2_c",
              "Wd1c", "Wd1a"]
    wts = {n: ei(n, [P, P], F16) for n in wnames}
    w2r = ei("w2r", [P, 1024], F16)
    bnames = ["b1_a", "b1_c", "b2_a", "b2_c",
              "bn_gamma_c", "bn_beta_c", "bn_gamma_a", "bn_beta_a",
              "b_dec1", "b_dec2c"]
    bis = {n: ei(n, [P, 1], F32) for n in bnames}

    y_out = nc.dram_tensor("y", [P, dec["ld_pad"] // P], F32,
                           kind="ExternalOutput")
    dbg_outs = {}
    if dbg:
        for n, shp in [("d_comb", [cfg.n_a, 2 * P]), ("d_hcr", [cfg.cpc, P]),
                       ("d_zc", [P, cfg.cpc_pad]), ("d_za", [cfg.n_a, P]),
                       ("d_rs", [P, cfg.apc]), ("d_st", [P, 8])]:
            dbg_outs[n] = nc.dram_tensor(n, shp, F16 if n != "d_st" else F32,
                                         kind="ExternalOutput")
    rg = [list(range(NCORES))]

    with tile.TileContext(nc) as tc:
        with (
            tc.tile_pool(name="dramp", bufs=1, space="DRAM") as dramp,
            tc.tile_pool(name="const", bufs=1) as cs,
        ):
            comb_own = dramp.tile([cfg.apc, 2 * P], F16)
            comb_full = dramp.tile([cfg.n_a, 2 * P], F16, addr_space="Shared")
            haT_d = dramp.tile([P, cfg.apc], F16)
            hcrows_d = dramp.tile([cfg.cpc, P], F16)
            zcT_d = dramp.tile([P, cfg.cpc_pad], F16)
            partial_d = dramp.tile([NCORES, P, cfg.apc], F16)
            rs_out = dramp.tile([P, cfg.apc], F16)
            zaT_d = dramp.tile([P, cfg.apc], F16)
            za_own = dramp.tile([cfg.apc, P], F16)
            za_full = dramp.tile([cfg.n_a, P], F16, addr_space="Shared")
            stc_in = dramp.tile([P, 2], F32)
            stc_out = dramp.tile([P, 2], F32, addr_space="Shared")
            sta_in = dramp.tile([P, 2], F32)
            sta_out = dramp.tile([P, 2], F32, addr_space="Shared")

            # constants
            iota8 = []
            ii = cs.tile([P, 2048], mybir.dt.int32, name="ioi")
            for wi in range(MAXW):
                nc.gpsimd.iota(ii[:], pattern=[[0, 16], [1, P]],
                               base=wi * P, channel_multiplier=0)
                ff = cs.tile([P, 2048], F16, name=f"iof{wi}")
                nc.vector.tensor_copy(ff[:], ii[:])
                iota8.append(ff)
            # decoder: const tiles with value p + wi*128 everywhere
            iotaP4 = []
            iop_i = cs.tile([P, cfg.dch], mybir.dt.int32, name="iopi")
            for wi in range(MAXW_DEC):
                nc.gpsimd.iota(iop_i[:], pattern=[[0, cfg.dch]], base=wi * P,
                               channel_multiplier=1)
                qf = cs.tile([P, cfg.dch], F16, name=f"iopf{wi}")
                nc.vector.tensor_copy(qf[:], iop_i[:])
                iotaP4.append(qf)
            ident = cs.tile([P, P], F16)
            make_identity(nc, ident[:])
            w_sb = {n: cs.tile([P, P], F16, name=f"w_{n}") for n in wnames}
            for n in wnames:
                nc.sync.dma_start(out=w_sb[n][:], in_=wts[n][:])
            w2r_sb = cs.tile([P, cfg.dch], F16)
            nc.sync.dma_start(out=w2r_sb[:], in_=w2r[:, :cfg.dch])
            b_sb = {n: cs.tile([P, 1], F32, name=f"b_{n}") for n in bnames}
            for n in bnames:
                nc.sync.dma_start(out=b_sb[n][:], in_=bis[n][:])
            stc_sb = cs.tile([P, 2], F32)
            nc.vector.memset(stc_sb[:], 0.0)
            sta_sb = cs.tile([P, 2], F32)
            nc.vector.memset(sta_sb[:], 0.0)

            # copy own x_a rows into comb (h half filled by A1 W-stage)
            nc.sync.dma_start(out=comb_own[:, 0:P], in_=xa_own[:])

            # ----------------------------------------------------------------
            # generic aggregation superblock: gathers + one-hot matmuls
            # ----------------------------------------------------------------
            def agg_sb(ps, s, table, idx_d, dsc_d, psums, gch, esz, sbp, tag):
                """Accumulate superblock s of pass ps into psums (list of
                (psum_tile, lhsT_lo) pairs: lhsT slice [lo:lo+128] of the
                gathered row).  esz = row elements (128 or 256)."""
                offs = np.concatenate(
                    [[0], np.cumsum(ps.run_L.reshape(-1))]).astype(np.int64)
                touch = {}
                for (j, t, wlo, nw) in ps.emits[s]:
                    for wi in range(nw):
                        w = wlo + wi
                        touch.setdefault(w // 4, []).append((j, t, w))
                firsts = {b: v[0] for b, v in touch.items()}
                lasts = {b: v[-1] for b, v in touch.items()}
                by_run = {}
                for e in ps.emits[s]:
                    by_run.setdefault(e[0], []).append(e)
                tpc = gch // P
                for j in sorted(by_run):
                    r = s * ps.nblk + j
                    o = int(offs[r])
                    L = int(ps.run_L[s, j])
                    nt = L // P
                    blk_rows = min(ps.srcb, ps.nsrc_rows - j * ps.srcb)
                    idx_sb = sbp.tile([P, L // 16], I16, tag=f"{tag}gi",
                                      name=f"{tag}gi", bufs=5)
                    nc.scalar.dma_start(out=idx_sb[:],
                                        in_=idx_d[:, o // 16:(o + L) // 16])
                    dsc_sb = sbp.tile([P, nt], F16, tag=f"{tag}gd",
                                      name=f"{tag}gd", bufs=5)
                    nc.scalar.dma_start(out=dsc_sb[:],
                                        in_=dsc_d[:, o // P:o // P + nt])
                    run_emits = by_run[j]
                    for c0 in range(0, L, gch):
                        cl = min(gch, L - c0)
                        c = c0 // P          # first tile index of chunk
                        ctn = cl // P
                        x = sbp.tile([P, tpc, esz], F16, tag=f"{tag}gx",
                                     name=f"{tag}gx", bufs=8)
                        nc.gpsimd.dma_gather(
                            x[:, :ctn, :],
                            table[j * ps.srcb:j * ps.srcb + blk_rows, :],
                            idx_sb[:, c0 // 16:(c0 + cl) // 16],
                            cl, cl, esz, queue_num=next_q())
                        ch_emits = [e for e in run_emits
                                    if c <= e[1] < c + ctn]
                        maxnw = max(e[3] for e in ch_emits)
                        dstb = dsc_sb[:, c:c + ctn].to_broadcast([P, ctn, P])
                        p8s = {}
                        for wi in range(maxnw):
                            p8 = sbp.tile([P, gch], F16, tag=f"{tag}gp",
                                          name=f"{tag}gp", bufs=2 * MAXW)
                            p83 = p8[:, :ctn * P].rearrange(
                                "p (t w) -> p t w", w=P)
                            nc.vector.tensor_tensor(
                                out=p83,
                                in0=iota8[wi][:, :ctn * P].rearrange(
                                    "p (t w) -> p t w", w=P),
                                in1=dstb, op=mybir.AluOpType.is_equal)
                            p8s[wi] = p8
                        for (j2, t, wlo, nw) in ch_emits:
                            for wi in range(nw):
                                w = wlo + wi
                                p8 = p8s[wi]
                                first = firsts[w // 4] == (j2, t, w)
                                last = lasts[w // 4] == (j2, t, w)
                                for psum, lo in psums:
                                    nc.tensor.matmul(
                                        psum[:, w * P:(w + 1) * P],
                                        lhsT=x[:, t - c, lo:lo + P],
                                        rhs=p8[:, (t - c) * P:
                                               (t - c + 1) * P],
                                        start=first, stop=last,
                                        skip_group_check=True)

            def zero_uncovered(ps, s, dst_sb, nodes):
                for w in ps.uncovered[s]:
                    a, b = w * P, min(w * P + P, nodes)
                    nc.vector.memset(dst_sb[:, a:b], 0.0)

            def emit_rows(srcT_sb, cw, rows_dram, row_base, sbp, psp, tag,
                          col_lo=0, col_n=P):
                for b0 in range(0, cw, P):
                    bw = min(P, cw - b0)
                    tp = psp.tile([P, P], F16, tag=f"{tag}tp", name=f"{tag}tp",
                                  bufs=1)
                    nc.tensor.transpose(tp[:bw, :], srcT_sb[:, b0:b0 + bw],
                                        ident[:])
                    rows = sbp.tile([P, P], F16, tag=f"{tag}ro",
                                    name=f"{tag}ro", bufs=3)
                    nc.scalar.copy(rows[:bw, :], tp[:bw, :])
                    nc.sync.dma_start(
                        out=rows_dram[row_base + b0:row_base + b0 + bw,
                                      col_lo:col_lo + col_n],
                        in_=rows[:bw, :])

            # ================= A1: layer-1 articles =================
            with (
                tc.tile_pool(name="a1", bufs=1) as sbp,
                tc.tile_pool(name="a1p", bufs=1, space="PSUM") as psp,
            ):
                for s in range(pa1.nsb):
                    nodes = pa1.sb_nodes[s]
                    nwin = -(-nodes // P)
                    psum = psp.tile([P, nwin * P], F32, tag="a1ps",
                                    name="a1ps", bufs=2)
                    agg_sb(pa1, s, xc, a1_idx, a1_dsc, [(psum, 0)],
                           cfg.gch_a1, P, sbp, "a1")
                    # scaled copy psum -> meanT
                    mean_sb = sbp.tile([P, cfg.sbn], F16, tag="a1mn",
                                       name="a1mn", bufs=2)
                    scl_sb = sbp.tile([P, cfg.sbn], F16, tag="a1sc",
                                      name="a1sc", bufs=2)
                    nc.scalar.dma_start(
                        out=scl_sb[:, :nodes],
                        in_=scl_a_rep[:, s * cfg.sbn:s * cfg.sbn + nodes])
                    nc.vector.tensor_tensor(out=mean_sb[:, :nodes],
                                            in0=psum[:, :nodes],
                                            in1=scl_sb[:, :nodes],
                                            op=mybir.AluOpType.mult)
                    zero_uncovered(pa1, s, mean_sb, nodes)
                    # W-stage: haT = relu(Wmsg1^T meanT + Wself1^T xaT + b)
                    haT_sb = sbp.tile([P, cfg.sbn], F16, tag="a1h",
                                      name="a1h", bufs=2)
                    for c0 in range(0, nodes, 512):
                        cw = min(512, nodes - c0)
                        sT = sbp.tile([P, 512], F16, tag="a1sf", name="a1sf",
                                      bufs=3)
                        nc.scalar.dma_start(
                            out=sT[:, :cw],
                            in_=xaT[:, s * cfg.sbn + c0:s * cfg.sbn + c0 + cw])
                        wps = psp.tile([P, 512], F32, tag="a1wp", name="a1wp",
                                       bufs=1)
                        nc.tensor.matmul(wps[:, :cw], lhsT=w_sb["W_msg1_ca"][:],
                                         rhs=mean_sb[:, c0:c0 + cw],
                                         start=True, stop=False,
                                         skip_group_check=True)
                        nc.tensor.matmul(wps[:, :cw], lhsT=w_sb["W_self1_a"][:],
                                         rhs=sT[:, :cw], start=False, stop=True,
                                         skip_group_check=True)
                        nc.scalar.activation(
                            haT_sb[:, c0:c0 + cw], wps[:, :cw],
                            mybir.ActivationFunctionType.Relu,
                            bias=b_sb["b1_a"][:], scale=1.0)
                    nc.sync.dma_start(
                        out=haT_d[:, s * cfg.sbn:s * cfg.sbn + nodes],
                        in_=haT_sb[:, :nodes])
                    emit_rows(haT_sb, nodes, comb_own, s * cfg.sbn, sbp, psp,
                              "a1r", col_lo=P, col_n=P)

            # ================= AllGather comb =================
            nc.gpsimd.collective_compute(
                "AllGather", mybir.AluOpType.bypass, replica_groups=rg,
                ins=[comb_own[:]], outs=[comb_full[:]])

            # ================= C-fused: layers 1+2 customers =================
            for h in range(2 if stage >= 2 else 0):
                ps = pcf[h]
                with (
                    tc.tile_pool(name=f"cf{h}", bufs=1) as sbp,
                    tc.tile_pool(name=f"cf{h}p", bufs=1, space="PSUM") as psp,
                ):
                    for s in range(ps.nsb):
                        nodes = ps.sb_nodes[s]
                        nwin = -(-nodes // P)
                        base = h * cfg.chalf + s * cfg.sbn_cf
                        psum1 = psp.tile([P, nwin * P], F32, tag="cfp1",
                                         name="cfp1", bufs=1)
                        psum2 = psp.tile([P, nwin * P], F32, tag="cfp2",
                                         name="cfp2", bufs=1)
                        agg_sb(ps, s, comb_full, cf_idx[h], cf_dsc[h],
                               [(psum1, 0), (psum2, P)], cfg.gch_cf, 2 * P,
                               sbp, "cf")
                        scl_sb = sbp.tile([P, cfg.sbn_cf], F16, tag="cfsc",
                                          name="cfsc", bufs=2)
                        nc.scalar.dma_start(
                            out=scl_sb[:, :nodes],
                            in_=scl_c_rep[:, base:base + nodes])
                        mean1 = sbp.tile([P, cfg.sbn_cf], F16, tag="cfm1",
                                         name="cfm1", bufs=2)
                        nc.vector.tensor_tensor(out=mean1[:, :nodes],
                                                in0=psum1[:, :nodes],
                                                in1=scl_sb[:, :nodes],
                                                op=mybir.AluOpType.mult)
                        zero_uncovered(ps, s, mean1, nodes)
                        mean2 = sbp.tile([P, cfg.sbn_cf], F16, tag="cfm2",
                                         name="cfm2", bufs=2)
                        nc.vector.tensor_tensor(out=mean2[:, :nodes],
                                                in0=psum2[:, :nodes],
                                                in1=scl_sb[:, :nodes],
                                                op=mybir.AluOpType.mult)
                        zero_uncovered(ps, s, mean2, nodes)
                        hcT_sb = sbp.tile([P, cfg.sbn_cf], F16, tag="cfh",
                                          name="cfh", bufs=2)
                        zcT_sb = sbp.tile([P, cfg.sbn_cf], F16, tag="cfz",
                                          name="cfz", bufs=2)
                        for c0 in range(0, nodes, 512):
                            cw = min(512, nodes - c0)
                            sT = sbp.tile([P, 512], F16, tag="cfsf",
                                          name="cfsf", bufs=3)
                            nc.scalar.dma_start(
                                out=sT[:, :cw],
                                in_=xcT[:, base + c0:base + c0 + cw])
                            wps = psp.tile([P, 512], F32, tag="cfwp",
                                           name="cfwp", bufs=1)
                            nc.tensor.matmul(wps[:, :cw],
                                             lhsT=w_sb["W_msg1_ac"][:],
                                             rhs=mean1[:, c0:c0 + cw],
                                             start=True, stop=False,
                                             skip_group_check=True)
                            nc.tensor.matmul(wps[:, :cw],
                                             lhsT=w_sb["W_self1_c"][:],
                                             rhs=sT[:, :cw],
                                             start=False, stop=True,
                                             skip_group_check=True)
                            nc.scalar.activation(
                                hcT_sb[:, c0:c0 + cw], wps[:, :cw],
                                mybir.ActivationFunctionType.Relu,
                                bias=b_sb["b1_c"][:], scale=1.0)
                            wps2 = psp.tile([P, 512], F32, tag="cfw2",
                                            name="cfw2", bufs=1)
                            nc.tensor.matmul(wps2[:, :cw],
                                             lhsT=w_sb["W_msg2_ac"][:],
                                             rhs=mean2[:, c0:c0 + cw],
                                             start=True, stop=False,
                                             skip_group_check=True)
                            nc.tensor.matmul(wps2[:, :cw],
                                             lhsT=w_sb["W_self2_c"][:],
                                             rhs=hcT_sb[:, c0:c0 + cw],
                                             start=False, stop=True,
                                             skip_group_check=True)
                            nc.scalar.activation(
                                zcT_sb[:, c0:c0 + cw], wps2[:, :cw],
                                mybir.ActivationFunctionType.Identity,
                                bias=b_sb["b2_c"][:], scale=1.0)
                            # BN-c stats
                            part = sbp.tile([P, 1], F32, tag="cfs1",
                                            name="cfs1", bufs=2)
                            nc.vector.reduce_sum(part[:], zcT_sb[:, c0:c0 + cw],
                                                 mybir.AxisListType.X)
                            nc.vector.tensor_add(stc_sb[:, 0:1],
                                                 stc_sb[:, 0:1], part[:])
                            trash = sbp.tile([P, 512], F32, tag="cftr",
                                             name="cftr", bufs=2)
                            part2 = sbp.tile([P, 1], F32, tag="cfs2",
                                             name="cfs2", bufs=2)
                            nc.scalar.activation(
                                trash[:, :cw], zcT_sb[:, c0:c0 + cw],
                                mybir.ActivationFunctionType.Square,
                                accum_out=part2[:])
                            nc.vector.tensor_add(stc_sb[:, 1:2],
                                                 stc_sb[:, 1:2], part2[:])
                        nc.sync.dma_start(out=zcT_d[:, base:base + nodes],
                                          in_=zcT_sb[:, :nodes])
                        emit_rows(hcT_sb, nodes, hcrows_d, base, sbp, psp,
                                  "cfr")

            # zero zcT_d pad tail
            if stage >= 2 and cfg.cpc_pad > cfg.cpc:
                with tc.tile_pool(name="ztp", bufs=1) as sbp:
                    zt = sbp.tile([P, cfg.cpc_pad - cfg.cpc], F16, name="ztt")
                    nc.vector.memset(zt[:], 0.0)
                    nc.sync.dma_start(out=zcT_d[:, cfg.cpc:], in_=zt[:])

            # BN-c stats AllReduce (A2 overlaps with it)
            if stage >= 2:
                nc.sync.dma_start(out=stc_in[:], in_=stc_sb[:])
                nc.gpsimd.collective_compute(
                    "AllReduce", mybir.AluOpType.add, replica_groups=rg,
                    ins=[stc_in[:]], outs=[stc_out[:]])

            # ================= A2: partial article aggregates =================
            with (
                tc.tile_pool(name="a2", bufs=1) as sbp,
                tc.tile_pool(name="a2p", bufs=1, space="PSUM") as psp,
            ):
                for s in range(pa2.nsb if stage >= 3 else 0):
                    nodes = pa2.sb_nodes[s]
                    nwin = -(-nodes // P)
                    psum = psp.tile([P, nwin * P], F32, tag="a2ps",
                                    name="a2ps", bufs=2)
                    agg_sb(pa2, s, hcrows_d, a2_idx, a2_dsc, [(psum, 0)],
                           cfg.gch_a1, P, sbp, "a2")
                    stg = sbp.tile([P, cfg.sbn], F16, tag="a2st",
                                   name="a2st", bufs=3)
                    nc.vector.tensor_copy(stg[:, :nodes], psum[:, :nodes])
                    zero_uncovered(pa2, s, stg, nodes)
                    blk = s // cfg.sb_per_blk
                    col = (s % cfg.sb_per_blk) * cfg.sbn
                    nc.sync.dma_start(
                        out=partial_d[blk, :, col:col + nodes],
                        in_=stg[:, :nodes])

            # ================= ReduceScatter =================
            if stage >= 3:
                nc.gpsimd.collective_compute(
                    "ReduceScatter", mybir.AluOpType.add, replica_groups=rg,
                    ins=[partial_d[:]], outs=[rs_out[:]])

            # ---------------- BN coeff helper ----------------
            def bn_coeff(st_sb, n, gamma, beta, tagp, sbp):
                mu = sbp.tile([P, 1], F32, name=f"mu{tagp}")
                nc.vector.tensor_scalar_mul(mu[:], st_sb[:, 0:1], 1.0 / n)
                msq = sbp.tile([P, 1], F32, name=f"ms{tagp}")
                nc.vector.tensor_scalar_mul(msq[:], st_sb[:, 1:2], 1.0 / n)
                mu2 = sbp.tile([P, 1], F32, name=f"m2{tagp}")
                nc.vector.tensor_mul(mu2[:], mu[:], mu[:])
                var = sbp.tile([P, 1], F32, name=f"va{tagp}")
                nc.vector.tensor_sub(var[:], msq[:], mu2[:])
                nc.vector.tensor_scalar_add(var[:], var[:], BN_EPS)
                sd = sbp.tile([P, 1], F32, name=f"sd{tagp}")
                nc.scalar.activation(sd[:], var[:],
                                     mybir.ActivationFunctionType.Sqrt)
                rstd = sbp.tile([P, 1], F32, name=f"rs{tagp}")
                nc.vector.reciprocal(rstd[:], sd[:])
                scl = sbp.tile([P, 1], F32, name=f"sc{tagp}")
                nc.vector.tensor_mul(scl[:], b_sb[gamma][:], rstd[:])
                mg = sbp.tile([P, 1], F32, name=f"mg{tagp}")
                nc.vector.tensor_mul(mg[:], mu[:], scl[:])
                bia = sbp.tile([P, 1], F32, name=f"bi{tagp}")
                nc.vector.tensor_sub(bia[:], b_sb[beta][:], mg[:])
                return scl, bia

            with tc.tile_pool(name="tail", bufs=1) as keep:
                ucrows = keep.tile([P, cfg.cpc_pad], F16, name="ucrows")

                # ============ U_c build (overlaps RS wait) ============
                with (
                    tc.tile_pool(name="uc", bufs=1) as sbp,
                    tc.tile_pool(name="ucp", bufs=1, space="PSUM") as psp,
                ):
                  if stage >= 4:
                    st = sbp.tile([P, 2], F32, name="ucst")
                    nc.scalar.dma_start(out=st[:], in_=stc_out[:])
                    scl_c_col, bia_c_col = bn_coeff(
                        st, cfg.n_c, "bn_gamma_c", "bn_beta_c", "c", sbp)
                    for c0 in range(0, cfg.cpc_pad, 512):
                        cw = min(512, cfg.cpc_pad - c0)
                        zT = sbp.tile([P, 512], F16, tag="ucz", name="ucz",
                                      bufs=3)
                        nc.scalar.dma_start(out=zT[:, :cw],
                                            in_=zcT_d[:, c0:c0 + cw])
                        bnT = sbp.tile([P, 512], F16, tag="ucb", name="ucb",
                                       bufs=3)
                        nc.scalar.activation(
                            bnT[:, :cw], zT[:, :cw],
                            mybir.ActivationFunctionType.Identity,
                            bias=bia_c_col[:], scale=scl_c_col[:])
                        ups = psp.tile([P, 512], F32, tag="ucp", name="ucp",
                                       bufs=2)
                        nc.tensor.matmul(ups[:, :cw], lhsT=w_sb["Wd1c"][:],
                                         rhs=bnT[:, :cw], start=True,
                                         stop=True, skip_group_check=True)
                        uT = sbp.tile([P, 512], F16, tag="ucu", name="ucu",
                                      bufs=3)
                        nc.scalar.activation(
                            uT[:, :cw], ups[:, :cw],
                            mybir.ActivationFunctionType.Identity,
                            bias=b_sb["b_dec1"][:], scale=1.0)
                        for b0 in range(0, cw, P):
                            tp = psp.tile([P, P], F16, tag="uctp",
                                          name="uctp", bufs=2)
                            nc.tensor.transpose(tp[:], uT[:, b0:b0 + P],
                                                ident[:])
                            nc.scalar.copy(ucrows[:, c0 + b0:c0 + b0 + P],
                                           tp[:])

                # ============== z_a stage (after RS) ==============
                with (
                    tc.tile_pool(name="za", bufs=1) as sbp,
                    tc.tile_pool(name="zap", bufs=1, space="PSUM") as psp,
                ):
                    for c0 in range(0, cfg.apc if stage >= 4 else 0, 512):
                        cw = min(512, cfg.apc - c0)
                        rsT = sbp.tile([P, 512], F16, tag="zar", name="zar",
                                       bufs=3)
                        nc.scalar.dma_start(out=rsT[:, :cw],
                                            in_=rs_out[:, c0:c0 + cw])
                        sclT = sbp.tile([P, 512], F16, tag="zas", name="zas",
                                        bufs=3)
                        nc.scalar.dma_start(out=sclT[:, :cw],
                                            in_=scl_a_rep[:, c0:c0 + cw])
                        m2 = sbp.tile([P, 512], F16, tag="zam", name="zam",
                                      bufs=3)
                        nc.vector.tensor_tensor(out=m2[:, :cw],
                                                in0=rsT[:, :cw],
                                                in1=sclT[:, :cw],
                                                op=mybir.AluOpType.mult)
                        hT = sbp.tile([P, 512], F16, tag="zah", name="zah",
                                      bufs=3)
                        nc.scalar.dma_start(out=hT[:, :cw],
                                            in_=haT_d[:, c0:c0 + cw])
                        wps = psp.tile([P, 512], F32, tag="zap", name="zap",
                                       bufs=2)
                        nc.tensor.matmul(wps[:, :cw],
                                         lhsT=w_sb["W_msg2_ca"][:],
                                         rhs=m2[:, :cw], start=True,
                                         stop=False, skip_group_check=True)
                        nc.tensor.matmul(wps[:, :cw],
                                         lhsT=w_sb["W_self2_a"][:],
                                         rhs=hT[:, :cw], start=False,
                                         stop=True, skip_group_check=True)
                        zaT = sbp.tile([P, 512], F16, tag="zaz", name="zaz",
                                       bufs=3)
                        nc.scalar.activation(
                            zaT[:, :cw], wps[:, :cw],
                            mybir.ActivationFunctionType.Identity,
                            bias=b_sb["b2_a"][:], scale=1.0)
                        part = sbp.tile([P, 1], F32, tag="zs1", name="zs1",
                                        bufs=2)
                        nc.vector.reduce_sum(part[:], zaT[:, :cw],
                                             mybir.AxisListType.X)
                        nc.vector.tensor_add(sta_sb[:, 0:1], sta_sb[:, 0:1],
                                             part[:])
                        trash = sbp.tile([P, 512], F32, tag="ztr", name="ztr",
                                         bufs=2)
                        part2 = sbp.tile([P, 1], F32, tag="zs2", name="zs2",
                                         bufs=2)
                        nc.scalar.activation(
                            trash[:, :cw], zaT[:, :cw],
                            mybir.ActivationFunctionType.Square,
                            accum_out=part2[:])
                        nc.vector.tensor_add(sta_sb[:, 1:2], sta_sb[:, 1:2],
                                             part2[:])
                        nc.sync.dma_start(out=zaT_d[:, c0:c0 + cw],
                                          in_=zaT[:, :cw])

                if stage >= 4:
                    nc.sync.dma_start(out=sta_in[:], in_=sta_sb[:])
                    nc.gpsimd.collective_compute(
                        "AllReduce", mybir.AluOpType.add, replica_groups=rg,
                        ins=[sta_in[:]], outs=[sta_out[:]])
                    # V_a = bn_a(z_a) @ Wd1a, as rows -> AllGather
                    with (
                        tc.tile_pool(name="va", bufs=1) as sbp,
                        tc.tile_pool(name="vap", bufs=1, space="PSUM") as psp,
                    ):
                        sta2 = sbp.tile([P, 2], F32, name="vast")
                        nc.scalar.dma_start(out=sta2[:], in_=sta_out[:])
                        scl_a_col, bia_a_col = bn_coeff(
                            sta2, cfg.n_a, "bn_gamma_a", "bn_beta_a", "a",
                            sbp)
                        for c0 in range(0, cfg.apc, 512):
                            cw = min(512, cfg.apc - c0)
                            zT = sbp.tile([P, 512], F16, tag="vaz",
                                          name="vaz", bufs=3)
                            nc.scalar.dma_start(out=zT[:, :cw],
                                                in_=zaT_d[:, c0:c0 + cw])
                            bnT = sbp.tile([P, 512], F16, tag="vab",
                                           name="vab", bufs=3)
                            nc.scalar.activation(
                                bnT[:, :cw], zT[:, :cw],
                                mybir.ActivationFunctionType.Identity,
                                bias=bia_a_col[:], scale=scl_a_col[:])
                            vps = psp.tile([P, 512], F32, tag="vap",
                                           name="vap", bufs=2)
                            nc.tensor.matmul(vps[:, :cw],
                                             lhsT=w_sb["Wd1a"][:],
                                             rhs=bnT[:, :cw], start=True,
                                             stop=True,
                                             skip_group_check=True)
                            vaT = sbp.tile([P, 512], F16, tag="vav",
                                           name="vav", bufs=3)
                            nc.scalar.copy(vaT[:, :cw], vps[:, :cw])
                            emit_rows(vaT, cw, za_own, c0, sbp, psp, "var")
                    nc.gpsimd.collective_compute(
                        "AllGather", mybir.AluOpType.bypass, replica_groups=rg,
                        ins=[za_own[:]], outs=[za_full[:]])

                if dbg:
                    nc.sync.dma_start(out=dbg_outs["d_comb"][:],
                                      in_=comb_full[:])
                    nc.sync.dma_start(out=dbg_outs["d_hcr"][:],
                                      in_=hcrows_d[:])
                    nc.sync.dma_start(out=dbg_outs["d_zc"][:], in_=zcT_d[:])
                    nc.sync.dma_start(out=dbg_outs["d_za"][:], in_=za_full[:])
                    nc.sync.dma_start(out=dbg_outs["d_rs"][:], in_=rs_out[:])
                    nc.sync.dma_start(out=dbg_outs["d_st"][:, 0:2],
                                      in_=stc_out[:])
                    nc.sync.dma_start(out=dbg_outs["d_st"][:, 2:4],
                                      in_=sta_out[:])

                # ================= decoder =================
                with (
                    tc.tile_pool(name="dc", bufs=1) as sbp,
                    tc.tile_pool(name="dcp", bufs=1, space="PSUM") as psp,
                ):
                  if stage >= 5:
                    sta_sb2 = sbp.tile([P, 2], F32, name="dsta")
                    nc.scalar.dma_start(out=sta_sb2[:], in_=sta_out[:])
                    scl_a_col, bia_a_col = bn_coeff(
                        sta_sb2, cfg.n_a, "bn_gamma_a", "bn_beta_a", "a", sbp)
                    goffs, grp_L = dec["goffs"], dec["grp_L"]
                    tile_wlo, tile_nw = dec["tile_wlo"], dec["tile_nw"]
                    for ab in range(cfg.nab):
                        o0, L = int(goffs[ab]), int(grp_L[ab])
                        blk_rows = min(cfg.srcb_a, cfg.n_a - ab * cfg.srcb_a)
                        for c0 in range(o0, o0 + L, cfg.dch):
                            cl = min(cfg.dch, o0 + L - c0)
                            ctn = cl // P
                            ixa = sbp.tile([P, cfg.dch // 16], I16, tag="dia",
                                           name="dia", bufs=4)
                            nc.scalar.dma_start(
                                out=ixa[:, :cl // 16],
                                in_=dec_idx_a[:, c0 // 16:(c0 + cl) // 16])
                            zg = sbp.tile([P, 1, cfg.dch], F16, tag="dzg",
                                          name="dzg", bufs=4)
                            nc.gpsimd.dma_gather(
                                zg[:, :, :cl],
                                za_full[ab * cfg.srcb_a:
                                        ab * cfg.srcb_a + blk_rows, :],
                                ixa[:, :cl // 16], cl, cl, P,
                                transpose=True, queue_num=next_q())
                            bnz = sbp.tile([P, cfg.dch], F16, tag="dbn",
                                           name="dbn", bufs=4)
                            nc.scalar.activation(
                                bnz[:, :cl], zg[:, 0, :cl],
                                mybir.ActivationFunctionType.Identity,
                                bias=bia_a_col[:], scale=scl_a_col[:])
                            crel = sbp.tile([P, cfg.dch], F16, tag="dcr",
                                            name="dcr", bufs=4)
                            nc.scalar.dma_start(out=crel[:, :cl],
                                                in_=dec_crel[:, c0:c0 + cl])
                            spsum = psp.tile([P, cfg.dch], F32, tag="dsp",
                                             name="dsp", bufs=2)
                            emlist = []
                            for cc in range(0, cl, 512):
                                emlist.append(("w", cc, min(512, cl - cc)))
                            for t in range(ctn):
                                g = c0 // P + t
                                for wi in range(int(tile_nw[g])):
                                    emlist.append(("q", t, wi))
                            banks = {}
                            for em in emlist:
                                if em[0] == "w":
                                    bset = set(range(
                                        em[1] // 512,
                                        (em[1] + em[2] - 1) // 512 + 1))
                                else:
                                    bset = {em[1] * P // 512}
                                for b in bset:
                                    banks.setdefault(b, []).append(em)
                            firsts = {b: v[0] for b, v in banks.items()}
                            lasts = {b: v[-1] for b, v in banks.items()}
                            for em in emlist:
                                if em[0] == "w":
                                    _, cc, cww = em
                                    b = cc // 512
                                    nc.tensor.matmul(
                                        spsum[:, cc:cc + cww],
                                        lhsT=w_sb["Wd1a"][:],
                                        rhs=bnz[:, cc:cc + cww],
                                        start=firsts[b] == em,
                                        stop=lasts[b] == em,
                                        skip_group_check=True)
                            qts = {}
                            for t in range(ctn):
                                g = c0 // P + t
                                for wi in range(int(tile_nw[g])):
                                    if wi not in qts:
                                        q = sbp.tile([P, cfg.dch], F16,
                                                     tag=f"dq{wi}",
                                                     name=f"dq{wi}", bufs=2)
                                        nc.vector.tensor_tensor(
                                            out=q[:, :cl],
                                            in0=iotaP4[wi][:, :cl],
                                            in1=crel[:, :cl],
                                            op=mybir.AluOpType.is_equal)
                                        qts[wi] = q
                            for em in emlist:
                                if em[0] == "q":
                                    _, t, wi = em
                                    g = c0 // P + t
                                    w = int(tile_wlo[g]) + wi
                                    b = t * P // 512
                                    nc.tensor.matmul(
                                        spsum[:, t * P:(t + 1) * P],
                                        lhsT=ucrows[:, w * P:(w + 1) * P],
                                        rhs=qts[wi][:, t * P:(t + 1) * P],
                                        start=firsts[b] == em,
                                        stop=lasts[b] == em,
                                        skip_group_check=True)
                            relu_sb = sbp.tile([P, cfg.dch], F16, tag="drl",
                                               name="drl", bufs=3)
                            nc.scalar.activation(
                                relu_sb[:, :cl], spsum[:, :cl],
                                mybir.ActivationFunctionType.Relu)
                            yp = psp.tile([P, cfg.dch], F32, tag="dyp",
                                          name="dyp", bufs=1)
                            for cc in range(0, cl, 512):
                                cww = min(512, cl - cc)
                                nc.tensor.matmul(yp[0:1, cc:cc + cww],
                                                 lhsT=w2_sb[:],
                                                 rhs=relu_sb[:, cc:cc + cww],
                                                 start=True, stop=True,
                                                 skip_group_check=True)
                            ysb = sbp.tile([1, cfg.dch], F32, tag="dys",
                                           name="dys", bufs=3)
                            nc.scalar.copy(ysb[:, :cl], yp[0:1, :cl])
                            nc.sync.dma_start(out=y_out[:, c0:c0 + cl],
                                              in_=ysb[:, :cl])

    nc.compile()
    return nc


# ---------------------------------------------------------------------------
# entry point
# ---------------------------------------------------------------------------

def make_in_maps(cfg, inputs, pa1, pcf, pa2, dec, scl_a, scl_c):
    f = lambda a: np.ascontiguousarray(np.asarray(a), dtype=np.float32)
    xc16 = f(inputs["x_customer"]).astype(np.float16)
    xa16 = f(inputs["x_article"]).astype(np.float16)
    wd1 = f(inputs["W_dec1"])
    base = dict(
        xc=xc16,
        W_msg1_ca=f(inputs["W_msg1_ca"]).astype(np.float16),
        W_self1_a=f(inputs["W_self1_a"]).astype(np.float16),
        W_msg1_ac=f(inputs["W_msg1_ac"]).astype(np.float16),
        W_self1_c=f(inputs["W_self1_c"]).astype(np.float16),
        W_msg2_ca=f(inputs["W_msg2_ca"]).astype(np.float16),
        W_self2_a=f(inputs["W_self2_a"]).astype(np.float16),
        W_msg2_ac=f(inputs["W_msg2_ac"]).astype(np.float16),
        W_self2_c=f(inputs["W_self2_c"]).astype(np.float16),
        Wd1c=wd1[:P].astype(np.float16), Wd1a=wd1[P:].astype(np.float16),
        w2r=np.tile(f(inputs["W_dec2"]).reshape(1, P),
                    (P, 8)).astype(np.float16),
        b1_a=f(inputs["b1_a"]).reshape(P, 1),
        b1_c=f(inputs["b1_c"]).reshape(P, 1),
        b2_a=f(inputs["b2_a"]).reshape(P, 1),
        b2_c=f(inputs["b2_c"]).reshape(P, 1),
        bn_gamma_c=f(inputs["bn_gamma_c"]).reshape(P, 1),
        bn_beta_c=f(inputs["bn_beta_c"]).reshape(P, 1),
        bn_gamma_a=f(inputs["bn_gamma_a"]).reshape(P, 1),
        bn_beta_a=f(inputs["bn_beta_a"]).reshape(P, 1),
        b_dec1=f(inputs["b_dec1"]).reshape(P, 1),
        b_dec2c=np.full((P, 1), float(np.asarray(inputs["b_dec2"]).item()),
                        np.float32),
    )
    in_maps = []
    for k in range(NCORES):
        m = dict(base)
        m["xa_own"] = np.ascontiguousarray(xa16[k * cfg.apc:(k + 1) * cfg.apc])
        m["xaT"] = np.ascontiguousarray(
            xa16[k * cfg.apc:(k + 1) * cfg.apc].T)
        m["xcT"] = np.ascontiguousarray(
            xc16[k * cfg.cpc:(k + 1) * cfg.cpc].T)
        m["scl_a_rep"] = np.tile(
            scl_a[k * cfg.apc:(k + 1) * cfg.apc].astype(np.float16)[None, :],
            (P, 1))
        m["scl_c_rep"] = np.tile(
            scl_c[k * cfg.cpc:(k + 1) * cfg.cpc].astype(np.float16)[None, :],
            (P, 1))
        m["a1_idx"], m["a1_dsc"] = pa1.idx[k], pa1.dsc[k]
        for h in range(2):
            m[f"cf{h}_idx"], m[f"cf{h}_dsc"] = pcf[h].idx[k], pcf[h].dsc[k]
        m["a2_idx"], m["a2_dsc"] = pa2.idx[k], pa2.dsc[k]
        m["dec_idx_a"], m["dec_crel"] = dec["idx_a"][k], dec["crel"][k]
        in_maps.append(m)
    return in_maps


def run(cfg, inputs, trace=False, dbg=False):
    pa1, pcf, pa2, dec, scl_a, scl_c = prep_all(cfg, inputs)
    in_maps = make_in_maps(cfg, inputs, pa1, pcf, pa2, dec, scl_a, scl_c)
    nc = build_nc(cfg, pa1, pcf, pa2, dec, dbg=dbg)
    res = run_bass_kernel_spmd(nc, in_maps, core_ids=list(range(NCORES)),
                               trace=trace)
    y = np.empty(cfg.e_lbl, np.float32)
    b2 = float(np.asarray(inputs["b_dec2"]).item())
    for k in range(NCORES):
        yl = res.results[k]["y"].T.reshape(-1) + b2
        po = dec["out_pos"][k]
        vm = po >= 0
        y[po[vm]] = yl[vm]
    return y, res


def kernel(**inputs):
    cfg = Cfg()
    y, _ = run(cfg, inputs, trace=False)
    return y


# revision 5
# speedup vs baseline: 1.4513x; 1.0008x over previous
"""Hetero GNN encoder/decoder v2 - restructured to minimize SWDGE descgen + DVE.

Key changes vs v1:
  - Pass order: A1 (dst=article-owner) -> AllGather comb[x_a|h_a] ->
    C-fused (C1+C2 share one 512B gather + one-hot P) -> A2
    (src=customer-owner, gathers LOCAL h_c rows, partial agg over all
    articles) -> ReduceScatter -> z_a -> AllGather z_a rows -> decoder.
  - One-hot P = is_equal only; 1/cnt scale applied at psum->SBUF copy
    via host-replicated per-column scale tables.
  - Decoder: transpose-gather of z_a (column-major), BN via per-partition
    scalar activation, Wd1a matmul + U_c window one-hot matmuls accumulate
    into one PSUM, w2 reduction via M=1 matmul.  No hc/ua AllGathers.
"""
import sys

sys.path.insert(0, "/opt/trn_rl_repo")

import numpy as np

import concourse.bacc as bacc
import concourse.bass as bass
import concourse.mybir as mybir
import concourse.tile as tile
from concourse.bass_utils import run_bass_kernel_spmd
from concourse.masks import make_identity

P = 128
NCORES = 8
MAXW = 4
MAXW_DEC = 8
BN_EPS = 1e-5
F32 = mybir.dt.float32
F16 = mybir.dt.float16
I16 = mybir.dt.int16


class Cfg:
    def __init__(self, small=False):
        if small:
            self.n_c, self.n_a, self.e_lbl = 6144, 1024, 8192
            self.sbn, self.sbn_cf = 128, 128
            self.srcb_c, self.srcb_a, self.srcb_h = 1024, 256, 384
            self.gch_a1, self.gch_cf, self.dch = 512, 256, 256
        else:
            self.n_c, self.n_a, self.e_lbl = 300000, 100000, 1000000
            self.sbn = 1250                 # superblock nodes (divides 12500)
            self.sbn_cf = 1024              # Cf superblock (2 psums, bank fit)
            self.srcb_c = 30000             # A1 src block (customers)
            self.srcb_a = 25000             # Cf src block (articles)
            self.srcb_h = 18750             # A2 src block (local customers)
            self.gch_a1 = 1024              # idx per gather, A1/A2 (256B rows)
            self.gch_cf = 1024              # idx per gather, Cf (512B rows)
            self.dch = 1024                 # decoder labels per chunk
        self.cpc, self.apc = self.n_c // NCORES, self.n_a // NCORES
        self.chalf = self.cpc // 2
        self.nblk_c = -(-self.n_c // self.srcb_c)
        self.nblk_a = -(-self.n_a // self.srcb_a)
        self.nblk_h = -(-self.cpc // self.srcb_h)
        self.nab = self.nblk_a
        assert self.apc % self.sbn == 0
        self.sb_per_blk = self.apc // self.sbn
        self.nwin_uc = -(-self.cpc // P)
        self.cpc_pad = self.nwin_uc * P


def _ru(x, m):
    return (x + m - 1) // m * m


def _wrap_idx(flat):
    """[n] int -> [128, n/16] wrap (16-partition layout, replicated x8)."""
    n = flat.shape[0]
    w = flat.astype(np.int16).reshape(n // 16, 16).T
    return np.tile(w, (8, 1))


def _pack_pcol(a):
    """[n] -> [128, n/128]: element i -> partition i%128, col i//128."""
    return np.ascontiguousarray(a.reshape(-1, P).T)


# ---------------------------------------------------------------------------
# host prep: one aggregation pass (uniform SPMD structure across cores)
# ---------------------------------------------------------------------------

class AggPass:
    def __init__(self, name, nloc, srcb, nblk, nsrc_rows, sbn):
        self.name, self.nloc, self.srcb = name, nloc, srcb
        self.nblk, self.nsrc_rows, self.sbn = nblk, nsrc_rows, sbn
        self.nsb = -(-nloc // sbn)
        self.sb_nodes = [min(sbn, nloc - s * sbn) for s in range(self.nsb)]


def prep_agg_pass(name, src, dst_loc, core_e, nloc, srcb, nblk, nsrc_rows, sbn):
    """Edges (src gathered, dst accumulated into nloc-range) per core.

    Returns AggPass with: run_L [nsb,nblk] uniform padded lengths, emits
    (per sb: list of (j, t, wlo, nw)), uncovered windows, per-core idx
    (wrapped int16) and dsc (dst-rel fp16, [128, etot/128])."""
    ap = AggPass(name, nloc, srcb, nblk, nsrc_rows, sbn)
    nsb = ap.nsb
    nruns = nsb * nblk

    per_core, counts = [], np.zeros((NCORES, nruns), np.int64)
    for k in range(NCORES):
        m = core_e == k
        s, d = src[m], dst_loc[m]
        j = s // srcb
        sb = d // sbn
        order = np.lexsort((d, j, sb))
        s, d, j, sb = s[order], d[order], j[order], sb[order]
        rid = sb * nblk + j
        counts[k] = np.bincount(rid, minlength=nruns)
        per_core.append((s, d, rid))

    run_L = _ru(counts.max(axis=0), P)
    offs = np.concatenate([[0], np.cumsum(run_L)]).astype(np.int64)
    etot = int(offs[-1])
    ap.run_L = run_L.reshape(nsb, nblk)
    ap.etot = etot

    dstrel_all = np.full((NCORES, etot), -1.0e9, np.float64)
    pos_all = []
    for k in range(NCORES):
        s, d, rid = per_core[k]
        run_start = np.concatenate([[0], np.cumsum(counts[k])])[:-1]
        pos = offs[rid] + (np.arange(len(s)) - run_start[rid])
        pos_all.append(pos)
        dstrel_all[k, pos] = (d - (d // sbn) * sbn).astype(np.float64)

    T = etot // P
    Dw = dstrel_all.reshape(NCORES, T, P)
    valid_any = Dw.max(axis=2) >= 0
    with np.errstate(invalid="ignore"):
        wlo_c = np.where(Dw >= 0, Dw, np.inf).min(axis=2) // P
        whi_c = np.where(Dw >= 0, Dw, -np.inf).max(axis=2) // P
    wlo_t = np.where(valid_any, wlo_c, np.inf).min(axis=0)
    whi_t = np.where(valid_any, whi_c, -np.inf).max(axis=0)

    emits, uncovered = [], []
    tile_wlo = np.zeros(T, np.int64)
    for s in range(nsb):
        nwin = -(-ap.sb_nodes[s] // P)
        covered, sb_emits = set(), []
        for j in range(nblk):
            r = s * nblk + j
            o = int(offs[r])
            nt = int(ap.run_L[s, j] // P)
            for t in range(nt):
                g = o // P + t
                if np.isfinite(wlo_t[g]):
                    a = max(0, min(int(wlo_t[g]), nwin - 1))
                    b = max(a, min(int(whi_t[g]), nwin - 1))
                else:
                    a, b = 0, 0
                nw = b - a + 1
                assert nw <= MAXW, f"{name}: tile spans {nw} windows"
                covered.update(range(a, b + 1))
                tile_wlo[g] = a
                sb_emits.append((j, t, a, nw))
        emits.append(sb_emits)
        uncovered.append(sorted(set(range(nwin)) - covered))
    ap.emits, ap.uncovered = emits, uncovered

    idxs, dscs = [], []
    for k in range(NCORES):
        s, d, rid = per_core[k]
        pos = pos_all[k]
        idx16 = np.zeros(etot, np.int16)
        idx16[pos] = (s - (s // srcb) * srcb).astype(np.int16)
        dstrel = np.full(etot, -1000.0, np.float32)
        dstrel[pos] = (d - (d // sbn) * sbn).astype(np.float32)
        dstrel -= 128.0 * tile_wlo[np.arange(etot) // P]
        dstrel[dstrel < -1000.0] = -1000.0
        idxs.append(_wrap_idx(idx16))
        dscs.append(_pack_pcol(dstrel).astype(np.float16))
    ap.idx, ap.dsc = idxs, dscs
    return ap


# ---------------------------------------------------------------------------
# host prep: decoder labels
# ---------------------------------------------------------------------------

def prep_decoder(cfg, l_c, l_a):
    """Labels partitioned by customer owner; per article-block (4 of 25000),
    sorted by customer.  Chunked into dch with uniform per-(core,ablk)
    padding.  Q emissions use tile_wlo + MAXW window-relative encoding."""
    nab = cfg.nab
    core_l = l_c // cfg.cpc
    ablk = l_a // cfg.srcb_a
    gcounts = np.zeros((NCORES, nab), np.int64)
    per_core = []
    for k in range(NCORES):
        m = core_l == k
        lc, la, ab, orig = l_c[m], l_a[m], ablk[m], np.nonzero(m)[0]
        order = np.lexsort((lc, ab))
        lc, la, ab, orig = lc[order], la[order], ab[order], orig[order]
        gcounts[k] = np.bincount(ab, minlength=nab)
        per_core.append((lc, la, ab, orig))
    grp_L = _ru(gcounts.max(axis=0), cfg.dch)
    goffs = np.concatenate([[0], np.cumsum(grp_L)]).astype(np.int64)
    ld_pad = int(goffs[-1])

    # window-relative structure (union over cores)
    win_all = np.full((NCORES, ld_pad), -1, np.int64)
    idx_a_list, crel_list, out_pos = [], [], []
    pos_all = []
    for k in range(NCORES):
        lc, la, ab, orig = per_core[k]
        gstart = np.concatenate([[0], np.cumsum(gcounts[k])])[:-1]
        pos = goffs[ab] + (np.arange(len(lc)) - gstart[ab])
        pos_all.append(pos)
        win_all[k, pos] = (lc % cfg.cpc) // P

    T = ld_pad // P
    Ww = win_all.reshape(NCORES, T, P)
    valid_any = Ww.max(axis=2) >= 0
    wlo_c = np.where(Ww >= 0, Ww, np.inf).min(axis=2)
    whi_c = np.where(Ww >= 0, Ww, -np.inf).max(axis=2)
    wlo_t = np.where(valid_any, wlo_c, np.inf).min(axis=0)
    whi_t = np.where(valid_any, whi_c, -np.inf).max(axis=0)
    tile_wlo = np.zeros(T, np.int64)
    tile_nw = np.ones(T, np.int64)
    for t in range(T):
        if np.isfinite(wlo_t[t]):
            a = min(int(wlo_t[t]), cfg.nwin_uc - 1)
            b = min(int(whi_t[t]), cfg.nwin_uc - 1)
            nw = b - a + 1
            assert nw <= MAXW_DEC, f"dec tile spans {nw} windows"
            tile_wlo[t], tile_nw[t] = a, nw

    for k in range(NCORES):
        lc, la, ab, orig = per_core[k]
        pos = pos_all[k]
        ia = np.zeros(ld_pad, np.int16)
        ia[pos] = (la - (la // cfg.srcb_a) * cfg.srcb_a).astype(np.int16)
        crel = np.full(ld_pad, -1000.0, np.float32)
        crel[pos] = ((lc % cfg.cpc) - tile_wlo[pos // P] * P).astype(np.float32)
        po = np.full(ld_pad, -1, np.int64)
        po[pos] = orig
        idx_a_list.append(_wrap_idx(ia))
        # replicated across partitions: [128, ld_pad] fp16
        crel_list.append(np.tile(crel.astype(np.float16)[None, :], (P, 1)))
        out_pos.append(po)

    return dict(grp_L=grp_L, goffs=goffs, ld_pad=ld_pad, tile_wlo=tile_wlo,
                tile_nw=tile_nw, idx_a=idx_a_list, crel=crel_list,
                out_pos=out_pos)


def prep_all(cfg, inputs):
    i64 = lambda a: np.asarray(a).astype(np.int64)
    e_src = i64(inputs["edge_src_customer"])
    e_dst = i64(inputs["edge_dst_article"])
    l_c = i64(inputs["label_customer"])
    l_a = i64(inputs["label_article"])

    cnt_a = np.bincount(e_dst, minlength=cfg.n_a)
    cnt_c = np.bincount(e_src, minlength=cfg.n_c)
    scl_a = (1.0 / np.maximum(cnt_a, 1.0)).astype(np.float32)
    scl_c = (1.0 / np.maximum(cnt_c, 1.0)).astype(np.float32)

    # A1: dst = local articles (owner core), src = global customers
    pa1 = prep_agg_pass("A1", e_src, e_dst % cfg.apc, e_dst // cfg.apc,
                        cfg.apc, cfg.srcb_c, cfg.nblk_c, cfg.n_c, cfg.sbn)
    # Cf halves: dst = local customers (owner core), src = global articles
    dloc = e_src % cfg.cpc
    core_c = e_src // cfg.cpc
    half = (dloc >= cfg.chalf).astype(np.int64)
    pcf = []
    for h in range(2):
        m = half == h
        pcf.append(prep_agg_pass(f"Cf{h}", e_dst[m], dloc[m] - h * cfg.chalf,
                                 core_c[m], cfg.chalf, cfg.srcb_a, cfg.nblk_a,
                                 cfg.n_a, cfg.sbn_cf))
    # A2: src = local customers (owner core), dst = ALL articles
    pa2 = prep_agg_pass("A2", e_src % cfg.cpc, e_dst, core_c,
                        cfg.n_a, cfg.srcb_h, cfg.nblk_h, cfg.cpc, cfg.sbn)
    dec = prep_decoder(cfg, l_c, l_a)
    return pa1, pcf, pa2, dec, scl_a, scl_c


# ---------------------------------------------------------------------------
# kernel builder
# ---------------------------------------------------------------------------

def build_nc(cfg, pa1, pcf, pa2, dec, dbg=False):
    import os
    stage = int(os.environ.get("K2_STAGE", "5"))
    nc = bacc.Bacc("TRN2", target_bir_lowering=False, debug=False,
                   num_devices=NCORES, num_swdge_queues=4)
    qctr = [0]
    def next_q():
        qctr[0] = (qctr[0] + 1) % 4
        return qctr[0]

    ei = lambda n, s, d: nc.dram_tensor(n, s, d, kind="ExternalInput")
    xc = ei("xc", [cfg.n_c, P], F16)              # full customer rows
    xa_own = ei("xa_own", [cfg.apc, P], F16)      # own article rows
    xaT = ei("xaT", [P, cfg.apc], F16)            # own articles colmajor
    xcT = ei("xcT", [P, cfg.cpc], F16)            # own customers colmajor
    scl_a_rep = ei("scl_a_rep", [P, cfg.apc], F16)
    scl_c_rep = ei("scl_c_rep", [P, cfg.cpc], F16)
    a1_idx = ei("a1_idx", [P, pa1.etot // 16], I16)
    a1_dsc = ei("a1_dsc", [P, pa1.etot // P], F16)
    cf_idx = [ei(f"cf{h}_idx", [P, pcf[h].etot // 16], I16) for h in range(2)]
    cf_dsc = [ei(f"cf{h}_dsc", [P, pcf[h].etot // P], F16) for h in range(2)]
    a2_idx = ei("a2_idx", [P, pa2.etot // 16], I16)
    a2_dsc = ei("a2_dsc", [P, pa2.etot // P], F16)
    dec_idx_a = ei("dec_idx_a", [P, dec["ld_pad"] // 16], I16)
    dec_crel = ei("dec_crel", [P, dec["ld_pad"]], F16)

    wnames = ["W_msg1_ca", "W_self1_a", "W_msg1_ac", "W_self1_c",
              "W_msg2_ca", "W_self2_a", "W_msg2_ac", "W_self2_c",
              "Wd1c", "Wd1a"]
    wts = {n: ei(n, [P, P], F16) for n in wnames}
    w2r = ei("w2r", [P, 1024], F16)
    bnames = ["b1_a", "b1_c", "b2_a", "b2_c",
              "bn_gamma_c", "bn_beta_c", "bn_gamma_a", "bn_beta_a",
              "b_dec1", "b_dec2c"]
    bis = {n: ei(n, [P, 1], F32) for n in bnames}

    y_out = nc.dram_tensor("y", [P, dec["ld_pad"] // P], F32,
                           kind="ExternalOutput")
    dbg_outs = {}
    if dbg:
        for n, shp in [("d_comb", [cfg.n_a, 2 * P]), ("d_hcr", [cfg.cpc, P]),
                       ("d_zc", [P, cfg.cpc_pad]), ("d_za", [cfg.n_a, P]),
                       ("d_rs", [P, cfg.apc]), ("d_st", [P, 8])]:
            dbg_outs[n] = nc.dram_tensor(n, shp, F16 if n != "d_st" else F32,
                                         kind="ExternalOutput")
    rg = [list(range(NCORES))]

    with tile.TileContext(nc) as tc:
        with (
            tc.tile_pool(name="dramp", bufs=1, space="DRAM") as dramp,
            tc.tile_pool(name="const", bufs=1) as cs,
        ):
            comb_own = dramp.tile([cfg.apc, 2 * P], F16)
            comb_full = dramp.tile([cfg.n_a, 2 * P], F16, addr_space="Shared")
            haT_d = dramp.tile([P, cfg.apc], F16)
            hcrows_d = dramp.tile([cfg.cpc, P], F16)
            zcT_d = dramp.tile([P, cfg.cpc_pad], F16)
            partial_d = dramp.tile([NCORES, P, cfg.apc], F16)
            rs_out = dramp.tile([P, cfg.apc], F16)
            zaT_d = dramp.tile([P, cfg.apc], F16)
            za_own = dramp.tile([cfg.apc, P], F16)
            za_full = dramp.tile([cfg.n_a, P], F16, addr_space="Shared")
            stc_in = dramp.tile([P, 2], F32)
            stc_out = dramp.tile([P, 2], F32, addr_space="Shared")
            sta_in = dramp.tile([P, 2], F32)
            sta_out = dramp.tile([P, 2], F32, addr_space="Shared")

            # constants
            iota8 = []
            ii = cs.tile([P, 2048], mybir.dt.int32, name="ioi")
            for wi in range(MAXW):
                nc.gpsimd.iota(ii[:], pattern=[[0, 16], [1, P]],
                               base=wi * P, channel_multiplier=0)
                ff = cs.tile([P, 2048], F16, name=f"iof{wi}")
                nc.vector.tensor_copy(ff[:], ii[:])
                iota8.append(ff)
            # decoder: const tiles with value p + wi*128 everywhere
            iotaP4 = []
            iop_i = cs.tile([P, cfg.dch], mybir.dt.int32, name="iopi")
            for wi in range(MAXW_DEC):
                nc.gpsimd.iota(iop_i[:], pattern=[[0, cfg.dch]], base=wi * P,
                               channel_multiplier=1)
                qf = cs.tile([P, cfg.dch], F16, name=f"iopf{wi}")
                nc.vector.tensor_copy(qf[:], iop_i[:])
                iotaP4.append(qf)
            ident = cs.tile([P, P], F16)
            make_identity(nc, ident[:])
            w_sb = {n: cs.tile([P, P], F16, name=f"w_{n}") for n in wnames}
            for n in wnames:
                nc.sync.dma_start(out=w_sb[n][:], in_=wts[n][:])
            w2r_sb = cs.tile([P, cfg.dch], F16)
            nc.sync.dma_start(out=w2r_sb[:], in_=w2r[:, :cfg.dch])
            b_sb = {n: cs.tile([P, 1], F32, name=f"b_{n}") for n in bnames}
            for n in bnames:
                nc.sync.dma_start(out=b_sb[n][:], in_=bis[n][:])
            stc_sb = cs.tile([P, 2], F32)
            nc.vector.memset(stc_sb[:], 0.0)
            sta_sb = cs.tile([P, 2], F32)
            nc.vector.memset(sta_sb[:], 0.0)

            # copy own x_a rows into comb (h half filled by A1 W-stage)
            nc.scalar.dma_start(out=comb_own[:, 0:P], in_=xa_own[:])

            # ----------------------------------------------------------------
            # generic aggregation superblock: gathers + one-hot matmuls
            # ----------------------------------------------------------------
            def agg_sb(ps, s, table, idx_d, dsc_d, psums, gch, esz, sbp, tag):
                """Accumulate superblock s of pass ps into psums (list of
                (psum_tile, lhsT_lo) pairs: lhsT slice [lo:lo+128] of the
                gathered row).  esz = row elements (128 or 256)."""
                offs = np.concatenate(
                    [[0], np.cumsum(ps.run_L.reshape(-1))]).astype(np.int64)
                touch = {}
                for (j, t, wlo, nw) in ps.emits[s]:
                    for wi in range(nw):
                        w = wlo + wi
                        touch.setdefault(w // 4, []).append((j, t, w))
                firsts = {b: v[0] for b, v in touch.items()}
                lasts = {b: v[-1] for b, v in touch.items()}
                by_run = {}
                for e in ps.emits[s]:
                    by_run.setdefault(e[0], []).append(e)
                tpc = gch // P
                for j in sorted(by_run):
                    r = s * ps.nblk + j
                    o = int(offs[r])
                    L = int(ps.run_L[s, j])
                    nt = L // P
                    blk_rows = min(ps.srcb, ps.nsrc_rows - j * ps.srcb)
                    idx_sb = sbp.tile([P, L // 16], I16, tag=f"{tag}gi",
                                      name=f"{tag}gi", bufs=5)
                    nc.sync.dma_start(out=idx_sb[:],
                                        in_=idx_d[:, o // 16:(o + L) // 16])
                    dsc_sb = sbp.tile([P, nt], F16, tag=f"{tag}gd",
                                      name=f"{tag}gd", bufs=5)
                    nc.sync.dma_start(out=dsc_sb[:],
                                        in_=dsc_d[:, o // P:o // P + nt])
                    run_emits = by_run[j]
                    for c0 in range(0, L, gch):
                        cl = min(gch, L - c0)
                        c = c0 // P          # first tile index of chunk
                        ctn = cl // P
                        x = sbp.tile([P, tpc, esz], F16, tag=f"{tag}gx",
                                     name=f"{tag}gx", bufs=8)
                        nc.gpsimd.dma_gather(
                            x[:, :ctn, :],
                            table[j * ps.srcb:j * ps.srcb + blk_rows, :],
                            idx_sb[:, c0 // 16:(c0 + cl) // 16],
                            cl, cl, esz, queue_num=next_q())
                        ch_emits = [e for e in run_emits
                                    if c <= e[1] < c + ctn]
                        maxnw = max(e[3] for e in ch_emits)
                        dstb = dsc_sb[:, c:c + ctn].to_broadcast([P, ctn, P])
                        p8s = {}
                        for wi in range(maxnw):
                            p8 = sbp.tile([P, gch], F16, tag=f"{tag}gp",
                                          name=f"{tag}gp", bufs=2 * MAXW)
                            p83 = p8[:, :ctn * P].rearrange(
                                "p (t w) -> p t w", w=P)
                            nc.vector.tensor_tensor(
                                out=p83,
                                in0=iota8[wi][:, :ctn * P].rearrange(
                                    "p (t w) -> p t w", w=P),
                                in1=dstb, op=mybir.AluOpType.is_equal)
                            p8s[wi] = p8
                        for (j2, t, wlo, nw) in ch_emits:
                            for wi in range(nw):
                                w = wlo + wi
                                p8 = p8s[wi]
                                first = firsts[w // 4] == (j2, t, w)
                                last = lasts[w // 4] == (j2, t, w)
                                for psum, lo in psums:
                                    nc.tensor.matmul(
                                        psum[:, w * P:(w + 1) * P],
                                        lhsT=x[:, t - c, lo:lo + P],
                                        rhs=p8[:, (t - c) * P:
                                               (t - c + 1) * P],
                                        start=first, stop=last,
                                        skip_group_check=True)

            def zero_uncovered(ps, s, dst_sb, nodes):
                for w in ps.uncovered[s]:
                    a, b = w * P, min(w * P + P, nodes)
                    nc.vector.memset(dst_sb[:, a:b], 0.0)

            def emit_rows(srcT_sb, cw, rows_dram, row_base, sbp, psp, tag,
                          col_lo=0, col_n=P):
                for b0 in range(0, cw, P):
                    bw = min(P, cw - b0)
                    tp = psp.tile([P, P], F16, tag=f"{tag}tp", name=f"{tag}tp",
                                  bufs=1)
                    nc.tensor.transpose(tp[:bw, :], srcT_sb[:, b0:b0 + bw],
                                        ident[:])
                    rows = sbp.tile([P, P], F16, tag=f"{tag}ro",
                                    name=f"{tag}ro", bufs=3)
                    nc.scalar.copy(rows[:bw, :], tp[:bw, :])
                    nc.scalar.dma_start(
                        out=rows_dram[row_base + b0:row_base + b0 + bw,
                                      col_lo:col_lo + col_n],
                        in_=rows[:bw, :])

            # ================= A1: layer-1 articles =================
            with (
                tc.tile_pool(name="a1", bufs=1) as sbp,
                tc.tile_pool(name="a1p", bufs=1, space="PSUM") as psp,
            ):
                for s in range(pa1.nsb):
                    nodes = pa1.sb_nodes[s]
                    nwin = -(-nodes // P)
                    psum = psp.tile([P, nwin * P], F32, tag="a1ps",
                                    name="a1ps", bufs=2)
                    agg_sb(pa1, s, xc, a1_idx, a1_dsc, [(psum, 0)],
                           cfg.gch_a1, P, sbp, "a1")
                    # scaled copy psum -> meanT
                    mean_sb = sbp.tile([P, cfg.sbn], F16, tag="a1mn",
                                       name="a1mn", bufs=2)
                    scl_sb = sbp.tile([P, cfg.sbn], F16, tag="a1sc",
                                      name="a1sc", bufs=2)
                    nc.sync.dma_start(
                        out=scl_sb[:, :nodes],
                        in_=scl_a_rep[:, s * cfg.sbn:s * cfg.sbn + nodes])
                    nc.vector.tensor_tensor(out=mean_sb[:, :nodes],
                                            in0=psum[:, :nodes],
                                            in1=scl_sb[:, :nodes],
                                            op=mybir.AluOpType.mult)
                    zero_uncovered(pa1, s, mean_sb, nodes)
                    # W-stage: haT = relu(Wmsg1^T meanT + Wself1^T xaT + b)
                    haT_sb = sbp.tile([P, cfg.sbn], F16, tag="a1h",
                                      name="a1h", bufs=2)
                    for c0 in range(0, nodes, 512):
                        cw = min(512, nodes - c0)
                        sT = sbp.tile([P, 512], F16, tag="a1sf", name="a1sf",
                                      bufs=3)
                        nc.sync.dma_start(
                            out=sT[:, :cw],
                            in_=xaT[:, s * cfg.sbn + c0:s * cfg.sbn + c0 + cw])
                        wps = psp.tile([P, 512], F32, tag="a1wp", name="a1wp",
                                       bufs=1)
                        nc.tensor.matmul(wps[:, :cw], lhsT=w_sb["W_msg1_ca"][:],
                                         rhs=mean_sb[:, c0:c0 + cw],
                                         start=True, stop=False,
                                         skip_group_check=True)
                        nc.tensor.matmul(wps[:, :cw], lhsT=w_sb["W_self1_a"][:],
                                         rhs=sT[:, :cw], start=False, stop=True,
                                         skip_group_check=True)
                        nc.scalar.activation(
                            haT_sb[:, c0:c0 + cw], wps[:, :cw],
                            mybir.ActivationFunctionType.Relu,
                            bias=b_sb["b1_a"][:], scale=1.0)
                    nc.scalar.dma_start(
                        out=haT_d[:, s * cfg.sbn:s * cfg.sbn + nodes],
                        in_=haT_sb[:, :nodes])
                    emit_rows(haT_sb, nodes, comb_own, s * cfg.sbn, sbp, psp,
                              "a1r", col_lo=P, col_n=P)

            # ================= AllGather comb =================
            nc.gpsimd.collective_compute(
                "AllGather", mybir.AluOpType.bypass, replica_groups=rg,
                ins=[comb_own[:]], outs=[comb_full[:]])

            # ================= C-fused: layers 1+2 customers =================
            for h in range(2 if stage >= 2 else 0):
                ps = pcf[h]
                with (
                    tc.tile_pool(name=f"cf{h}", bufs=1) as sbp,
                    tc.tile_pool(name=f"cf{h}p", bufs=1, space="PSUM") as psp,
                ):
                    for s in range(ps.nsb):
                        nodes = ps.sb_nodes[s]
                        nwin = -(-nodes // P)
                        base = h * cfg.chalf + s * cfg.sbn_cf
                        psum1 = psp.tile([P, nwin * P], F32, tag="cfp1",
                                         name="cfp1", bufs=1)
                        psum2 = psp.tile([P, nwin * P], F32, tag="cfp2",
                                         name="cfp2", bufs=1)
                        agg_sb(ps, s, comb_full, cf_idx[h], cf_dsc[h],
                               [(psum1, 0), (psum2, P)], cfg.gch_cf, 2 * P,
                               sbp, "cf")
                        scl_sb = sbp.tile([P, cfg.sbn_cf], F16, tag="cfsc",
                                          name="cfsc", bufs=2)
                        nc.sync.dma_start(
                            out=scl_sb[:, :nodes],
                            in_=scl_c_rep[:, base:base + nodes])
                        mean1 = sbp.tile([P, cfg.sbn_cf], F16, tag="cfm1",
                                         name="cfm1", bufs=2)
                        nc.vector.tensor_tensor(out=mean1[:, :nodes],
                                                in0=psum1[:, :nodes],
                                                in1=scl_sb[:, :nodes],
                                                op=mybir.AluOpType.mult)
                        zero_uncovered(ps, s, mean1, nodes)
                        mean2 = sbp.tile([P, cfg.sbn_cf], F16, tag="cfm2",
                                         name="cfm2", bufs=2)
                        nc.vector.tensor_tensor(out=mean2[:, :nodes],
                                                in0=psum2[:, :nodes],
                                                in1=scl_sb[:, :nodes],
                                                op=mybir.AluOpType.mult)
                        zero_uncovered(ps, s, mean2, nodes)
                        hcT_sb = sbp.tile([P, cfg.sbn_cf], F16, tag="cfh",
                                          name="cfh", bufs=2)
                        zcT_sb = sbp.tile([P, cfg.sbn_cf], F16, tag="cfz",
                                          name="cfz", bufs=2)
                        for c0 in range(0, nodes, 512):
                            cw = min(512, nodes - c0)
                            sT = sbp.tile([P, 512], F16, tag="cfsf",
                                          name="cfsf", bufs=3)
                            nc.sync.dma_start(
                                out=sT[:, :cw],
                                in_=xcT[:, base + c0:base + c0 + cw])
                            wps = psp.tile([P, 512], F32, tag="cfwp",
                                           name="cfwp", bufs=1)
                            nc.tensor.matmul(wps[:, :cw],
                                             lhsT=w_sb["W_msg1_ac"][:],
                                             rhs=mean1[:, c0:c0 + cw],
                                             start=True, stop=False,
                                             skip_group_check=True)
                            nc.tensor.matmul(wps[:, :cw],
                                             lhsT=w_sb["W_self1_c"][:],
                                             rhs=sT[:, :cw],
                                             start=False, stop=True,
                                             skip_group_check=True)
                            nc.scalar.activation(
                                hcT_sb[:, c0:c0 + cw], wps[:, :cw],
                                mybir.ActivationFunctionType.Relu,
                                bias=b_sb["b1_c"][:], scale=1.0)
                            wps2 = psp.tile([P, 512], F32, tag="cfw2",
                                            name="cfw2", bufs=1)
                            nc.tensor.matmul(wps2[:, :cw],
                                             lhsT=w_sb["W_msg2_ac"][:],
                                             rhs=mean2[:, c0:c0 + cw],
                                             start=True, stop=False,
                                             skip_group_check=True)
                            nc.tensor.matmul(wps2[:, :cw],
                                             lhsT=w_sb["W_self2_c"][:],
                                             rhs=hcT_sb[:, c0:c0 + cw],
                                             start=False, stop=True,
                                             skip_group_check=True)
                            nc.scalar.activation(
                                zcT_sb[:, c0:c0 + cw], wps2[:, :cw],
                                mybir.ActivationFunctionType.Identity,
                                bias=b_sb["b2_c"][:], scale=1.0)
                            # BN-c stats
                            part = sbp.tile([P, 1], F32, tag="cfs1",
                                            name="cfs1", bufs=2)
                            nc.vector.reduce_sum(part[:], zcT_sb[:, c0:c0 + cw],
                                                 mybir.AxisListType.X)
                            nc.vector.tensor_add(stc_sb[:, 0:1],
                                                 stc_sb[:, 0:1], part[:])
                            trash = sbp.tile([P, 512], F32, tag="cftr",
                                             name="cftr", bufs=2)
                            part2 = sbp.tile([P, 1], F32, tag="cfs2",
                                             name="cfs2", bufs=2)
                            nc.scalar.activation(
                                trash[:, :cw], zcT_sb[:, c0:c0 + cw],
                                mybir.ActivationFunctionType.Square,
                                accum_out=part2[:])
                            nc.vector.tensor_add(stc_sb[:, 1:2],
                                                 stc_sb[:, 1:2], part2[:])
                        nc.scalar.dma_start(out=zcT_d[:, base:base + nodes],
                                          in_=zcT_sb[:, :nodes])
                        emit_rows(hcT_sb, nodes, hcrows_d, base, sbp, psp,
                                  "cfr")

            # zero zcT_d pad tail
            if stage >= 2 and cfg.cpc_pad > cfg.cpc:
                with tc.tile_pool(name="ztp", bufs=1) as sbp:
                    zt = sbp.tile([P, cfg.cpc_pad - cfg.cpc], F16, name="ztt")
                    nc.vector.memset(zt[:], 0.0)
                    nc.scalar.dma_start(out=zcT_d[:, cfg.cpc:], in_=zt[:])

            # BN-c stats AllReduce (A2 overlaps with it)
            if stage >= 2:
                nc.scalar.dma_start(out=stc_in[:], in_=stc_sb[:])
                nc.gpsimd.collective_compute(
                    "AllReduce", mybir.AluOpType.add, replica_groups=rg,
                    ins=[stc_in[:]], outs=[stc_out[:]])

            # ================= A2: partial article aggregates =================
            with (
                tc.tile_pool(name="a2", bufs=1) as sbp,
                tc.tile_pool(name="a2p", bufs=1, space="PSUM") as psp,
            ):
                for s in range(pa2.nsb if stage >= 3 else 0):
                    nodes = pa2.sb_nodes[s]
                    nwin = -(-nodes // P)
                    psum = psp.tile([P, nwin * P], F32, tag="a2ps",
                                    name="a2ps", bufs=2)
                    agg_sb(pa2, s, hcrows_d, a2_idx, a2_dsc, [(psum, 0)],
                           cfg.gch_a1, P, sbp, "a2")
                    stg = sbp.tile([P, cfg.sbn], F16, tag="a2st",
                                   name="a2st", bufs=3)
                    nc.vector.tensor_copy(stg[:, :nodes], psum[:, :nodes])
                    zero_uncovered(pa2, s, stg, nodes)
                    blk = s // cfg.sb_per_blk
                    col = (s % cfg.sb_per_blk) * cfg.sbn
                    nc.scalar.dma_start(
                        out=partial_d[blk, :, col:col + nodes],
                        in_=stg[:, :nodes])

            # ================= ReduceScatter =================
            if stage >= 3:
                nc.gpsimd.collective_compute(
                    "ReduceScatter", mybir.AluOpType.add, replica_groups=rg,
                    ins=[partial_d[:]], outs=[rs_out[:]])

            # ---------------- BN coeff helper ----------------
            def bn_coeff(st_sb, n, gamma, beta, tagp, sbp):
                mu = sbp.tile([P, 1], F32, name=f"mu{tagp}")
                nc.vector.tensor_scalar_mul(mu[:], st_sb[:, 0:1], 1.0 / n)
                msq = sbp.tile([P, 1], F32, name=f"ms{tagp}")
                nc.vector.tensor_scalar_mul(msq[:], st_sb[:, 1:2], 1.0 / n)
                mu2 = sbp.tile([P, 1], F32, name=f"m2{tagp}")
                nc.vector.tensor_mul(mu2[:], mu[:], mu[:])
                var = sbp.tile([P, 1], F32, name=f"va{tagp}")
                nc.vector.tensor_sub(var[:], msq[:], mu2[:])
                nc.vector.tensor_scalar_add(var[:], var[:], BN_EPS)
                sd = sbp.tile([P, 1], F32, name=f"sd{tagp}")
                nc.scalar.activation(sd[:], var[:],
                                     mybir.ActivationFunctionType.Sqrt)
                rstd = sbp.tile([P, 1], F32, name=f"rs{tagp}")
                nc.vector.reciprocal(rstd[:], sd[:])
                scl = sbp.tile([P, 1], F32, name=f"sc{tagp}")
                nc.vector.tensor_mul(scl[:], b_sb[gamma][:], rstd[:])
                mg = sbp.tile([P, 1], F32, name=f"mg{tagp}")
                nc.vector.tensor_mul(mg[:], mu[:], scl[:])
                bia = sbp.tile([P, 1], F32, name=f"bi{tagp}")
                nc.vector.tensor_sub(bia[:], b_sb[beta][:], mg[:])
                return scl, bia

            with tc.tile_pool(name="tail", bufs=1) as keep:
                ucrows = keep.tile([P, cfg.cpc_pad], F16, name="ucrows")

                # ============ U_c build (overlaps RS wait) ============
                with (
                    tc.tile_pool(name="uc", bufs=1) as sbp,
                    tc.tile_pool(name="ucp", bufs=1, space="PSUM") as psp,
                ):
                  if stage >= 4:
                    st = sbp.tile([P, 2], F32, name="ucst")
                    nc.sync.dma_start(out=st[:], in_=stc_out[:])
                    scl_c_col, bia_c_col = bn_coeff(
                        st, cfg.n_c, "bn_gamma_c", "bn_beta_c", "c", sbp)
                    for c0 in range(0, cfg.cpc_pad, 512):
                        cw = min(512, cfg.cpc_pad - c0)
                        zT = sbp.tile([P, 512], F16, tag="ucz", name="ucz",
                                      bufs=3)
                        nc.sync.dma_start(out=zT[:, :cw],
                                            in_=zcT_d[:, c0:c0 + cw])
                        bnT = sbp.tile([P, 512], F16, tag="ucb", name="ucb",
                                       bufs=3)
                        nc.scalar.activation(
                            bnT[:, :cw], zT[:, :cw],
                            mybir.ActivationFunctionType.Identity,
                            bias=bia_c_col[:], scale=scl_c_col[:])
                        ups = psp.tile([P, 512], F32, tag="ucp", name="ucp",
                                       bufs=2)
                        nc.tensor.matmul(ups[:, :cw], lhsT=w_sb["Wd1c"][:],
                                         rhs=bnT[:, :cw], start=True,
                                         stop=True, skip_group_check=True)
                        uT = sbp.tile([P, 512], F16, tag="ucu", name="ucu",
                                      bufs=3)
                        nc.scalar.activation(
                            uT[:, :cw], ups[:, :cw],
                            mybir.ActivationFunctionType.Identity,
                            bias=b_sb["b_dec1"][:], scale=1.0)
                        for b0 in range(0, cw, P):
                            tp = psp.tile([P, P], F16, tag="uctp",
                                          name="uctp", bufs=2)
                            nc.tensor.transpose(tp[:], uT[:, b0:b0 + P],
                                                ident[:])
                            nc.scalar.copy(ucrows[:, c0 + b0:c0 + b0 + P],
                                           tp[:])

                # ============== z_a stage (after RS) ==============
                with (
                    tc.tile_pool(name="za", bufs=1) as sbp,
                    tc.tile_pool(name="zap", bufs=1, space="PSUM") as psp,
                ):
                    for c0 in range(0, cfg.apc if stage >= 4 else 0, 512):
                        cw = min(512, cfg.apc - c0)
                        rsT = sbp.tile([P, 512], F16, tag="zar", name="zar",
                                       bufs=3)
                        nc.sync.dma_start(out=rsT[:, :cw],
                                            in_=rs_out[:, c0:c0 + cw])
                        sclT = sbp.tile([P, 512], F16, tag="zas", name="zas",
                                        bufs=3)
                        nc.sync.dma_start(out=sclT[:, :cw],
                                            in_=scl_a_rep[:, c0:c0 + cw])
                        m2 = sbp.tile([P, 512], F16, tag="zam", name="zam",
                                      bufs=3)
                        nc.vector.tensor_tensor(out=m2[:, :cw],
                                                in0=rsT[:, :cw],
                                                in1=sclT[:, :cw],
                                                op=mybir.AluOpType.mult)
                        hT = sbp.tile([P, 512], F16, tag="zah", name="zah",
                                      bufs=3)
                        nc.sync.dma_start(out=hT[:, :cw],
                                            in_=haT_d[:, c0:c0 + cw])
                        wps = psp.tile([P, 512], F32, tag="zap", name="zap",
                                       bufs=2)
                        nc.tensor.matmul(wps[:, :cw],
                                         lhsT=w_sb["W_msg2_ca"][:],
                                         rhs=m2[:, :cw], start=True,
                                         stop=False, skip_group_check=True)
                        nc.tensor.matmul(wps[:, :cw],
                                         lhsT=w_sb["W_self2_a"][:],
                                         rhs=hT[:, :cw], start=False,
                                         stop=True, skip_group_check=True)
                        zaT = sbp.tile([P, 512], F16, tag="zaz", name="zaz",
                                       bufs=3)
                        nc.scalar.activation(
                            zaT[:, :cw], wps[:, :cw],
                            mybir.ActivationFunctionType.Identity,
                            bias=b_sb["b2_a"][:], scale=1.0)
                        part = sbp.tile([P, 1], F32, tag="zs1", name="zs1",
                                        bufs=2)
                        nc.vector.reduce_sum(part[:], zaT[:, :cw],
                                             mybir.AxisListType.X)
                        nc.vector.tensor_add(sta_sb[:, 0:1], sta_sb[:, 0:1],
                                             part[:])
                        trash = sbp.tile([P, 512], F32, tag="ztr", name="ztr",
                                         bufs=2)
                        part2 = sbp.tile([P, 1], F32, tag="zs2", name="zs2",
                                         bufs=2)
                        nc.scalar.activation(
                            trash[:, :cw], zaT[:, :cw],
                            mybir.ActivationFunctionType.Square,
                            accum_out=part2[:])
                        nc.vector.tensor_add(sta_sb[:, 1:2], sta_sb[:, 1:2],
                                             part2[:])
                        nc.scalar.dma_start(out=zaT_d[:, c0:c0 + cw],
                                          in_=zaT[:, :cw])

                if stage >= 4:
                    nc.scalar.dma_start(out=sta_in[:], in_=sta_sb[:])
                    nc.gpsimd.collective_compute(
                        "AllReduce", mybir.AluOpType.add, replica_groups=rg,
                        ins=[sta_in[:]], outs=[sta_out[:]])
                    # V_a = bn_a(z_a) @ Wd1a, as rows -> AllGather
                    with (
                        tc.tile_pool(name="va", bufs=1) as sbp,
                        tc.tile_pool(name="vap", bufs=1, space="PSUM") as psp,
                    ):
                        sta2 = sbp.tile([P, 2], F32, name="vast")
                        nc.sync.dma_start(out=sta2[:], in_=sta_out[:])
                        scl_a_col, bia_a_col = bn_coeff(
                            sta2, cfg.n_a, "bn_gamma_a", "bn_beta_a", "a",
                            sbp)
                        for c0 in range(0, cfg.apc, 512):
                            cw = min(512, cfg.apc - c0)
                            zT = sbp.tile([P, 512], F16, tag="vaz",
                                          name="vaz", bufs=3)
                            nc.sync.dma_start(out=zT[:, :cw],
                                                in_=zaT_d[:, c0:c0 + cw])
                            bnT = sbp.tile([P, 512], F16, tag="vab",
                                           name="vab", bufs=3)
                            nc.scalar.activation(
                                bnT[:, :cw], zT[:, :cw],
                                mybir.ActivationFunctionType.Identity,
                                bias=bia_a_col[:], scale=scl_a_col[:])
                            vps = psp.tile([P, 512], F32, tag="vap",
                                           name="vap", bufs=2)
                            nc.tensor.matmul(vps[:, :cw],
                                             lhsT=w_sb["Wd1a"][:],
                                             rhs=bnT[:, :cw], start=True,
                                             stop=True,
                                             skip_group_check=True)
                            vaT = sbp.tile([P, 512], F16, tag="vav",
                                           name="vav", bufs=3)
                            nc.scalar.copy(vaT[:, :cw], vps[:, :cw])
                            emit_rows(vaT, cw, za_own, c0, sbp, psp, "var")
                    nc.gpsimd.collective_compute(
                        "AllGather", mybir.AluOpType.bypass, replica_groups=rg,
                        ins=[za_own[:]], outs=[za_full[:]])

                if dbg:
                    nc.sync.dma_start(out=dbg_outs["d_comb"][:],
                                      in_=comb_full[:])
                    nc.sync.dma_start(out=dbg_outs["d_hcr"][:],
                                      in_=hcrows_d[:])
                    nc.sync.dma_start(out=dbg_outs["d_zc"][:], in_=zcT_d[:])
                    nc.sync.dma_start(out=dbg_outs["d_za"][:], in_=za_full[:])
                    nc.sync.dma_start(out=dbg_outs["d_rs"][:], in_=rs_out[:])
                    nc.sync.dma_start(out=dbg_outs["d_st"][:, 0:2],
                                      in_=stc_out[:])
                    nc.sync.dma_start(out=dbg_outs["d_st"][:, 2:4],
                                      in_=sta_out[:])

                # ================= decoder =================
                with (
                    tc.tile_pool(name="dc", bufs=1) as sbp,
                    tc.tile_pool(name="dcp", bufs=1, space="PSUM") as psp,
                ):
                  if stage >= 5:
                    sta_sb2 = sbp.tile([P, 2], F32, name="dsta")
                    nc.sync.dma_start(out=sta_sb2[:], in_=sta_out[:])
                    scl_a_col, bia_a_col = bn_coeff(
                        sta_sb2, cfg.n_a, "bn_gamma_a", "bn_beta_a", "a", sbp)
                    goffs, grp_L = dec["goffs"], dec["grp_L"]
                    tile_wlo, tile_nw = dec["tile_wlo"], dec["tile_nw"]
                    for ab in range(cfg.nab):
                        o0, L = int(goffs[ab]), int(grp_L[ab])
                        blk_rows = min(cfg.srcb_a, cfg.n_a - ab * cfg.srcb_a)
                        for c0 in range(o0, o0 + L, cfg.dch):
                            cl = min(cfg.dch, o0 + L - c0)
                            ctn = cl // P
                            ixa = sbp.tile([P, cfg.dch // 16], I16, tag="dia",
                                           name="dia", bufs=4)
                            nc.sync.dma_start(
                                out=ixa[:, :cl // 16],
                                in_=dec_idx_a[:, c0 // 16:(c0 + cl) // 16])
                            zg = sbp.tile([P, 1, cfg.dch], F16, tag="dzg",
                                          name="dzg", bufs=4)
                            nc.gpsimd.dma_gather(
                                zg[:, :, :cl],
                                za_full[ab * cfg.srcb_a:
                                        ab * cfg.srcb_a + blk_rows, :],
                                ixa[:, :cl // 16], cl, cl, P,
                                transpose=True, queue_num=next_q())
                            bnz = sbp.tile([P, cfg.dch], F16, tag="dbn",
                                           name="dbn", bufs=4)
                            nc.scalar.activation(
                                bnz[:, :cl], zg[:, 0, :cl],
                                mybir.ActivationFunctionType.Identity,
                                bias=bia_a_col[:], scale=scl_a_col[:])
                            crel = sbp.tile([P, cfg.dch], F16, tag="dcr",
                                            name="dcr", bufs=4)
                            nc.sync.dma_start(out=crel[:, :cl],
                                                in_=dec_crel[:, c0:c0 + cl])
                            spsum = psp.tile([P, cfg.dch], F32, tag="dsp",
                                             name="dsp", bufs=2)
                            emlist = []
                            for cc in range(0, cl, 512):
                                emlist.append(("w", cc, min(512, cl - cc)))
                            for t in range(ctn):
                                g = c0 // P + t
                                for wi in range(int(tile_nw[g])):
                                    emlist.append(("q", t, wi))
                            banks = {}
                            for em in emlist:
                                if em[0] == "w":
                                    bset = set(range(
                                        em[1] // 512,
                                        (em[1] + em[2] - 1) // 512 + 1))
                                else:
                                    bset = {em[1] * P // 512}
                                for b in bset:
                                    banks.setdefault(b, []).append(em)
                            firsts = {b: v[0] for b, v in banks.items()}
                            lasts = {b: v[-1] for b, v in banks.items()}
                            for em in emlist:
                                if em[0] == "w":
                                    _, cc, cww = em
                                    b = cc // 512
                                    nc.tensor.matmul(
                                        spsum[:, cc:cc + cww],
                                        lhsT=w_sb["Wd1a"][:],
                                        rhs=bnz[:, cc:cc + cww],
                                        start=firsts[b] == em,
                                        stop=lasts[b] == em,
                                        skip_group_check=True)
                            qts = {}
                            for t in range(ctn):
                                g = c0 // P + t
                                for wi in range(int(tile_nw[g])):
                                    if wi not in qts:
                                        q = sbp.tile([P, cfg.dch], F16,
                                                     tag=f"dq{wi}",
                                                     name=f"dq{wi}", bufs=2)
                                        nc.vector.tensor_tensor(
                                            out=q[:, :cl],
                                            in0=iotaP4[wi][:, :cl],
                                            in1=crel[:, :cl],
                                            op=mybir.AluOpType.is_equal)
                                        qts[wi] = q
                            for em in emlist:
                                if em[0] == "q":
                                    _, t, wi = em
                                    g = c0 // P + t
                                    w = int(tile_wlo[g]) + wi
                                    b = t * P // 512
                                    nc.tensor.matmul(
                                        spsum[:, t * P:(t + 1) * P],
                                        lhsT=ucrows[:, w * P:(w + 1) * P],
                                        rhs=qts[wi][:, t * P:(t + 1) * P],
                                        start=firsts[b] == em,
                                        stop=lasts[b] == em,
                                        skip_group_check=True)
                            relu_sb = sbp.tile([P, cfg.dch], F16, tag="drl",
                                               name="drl", bufs=3)
                            nc.scalar.activation(
                                relu_sb[:, :cl], spsum[:, :cl],
                                mybir.ActivationFunctionType.Relu)
                            yp = psp.tile([P, cfg.dch], F32, tag="dyp",
                                          name="dyp", bufs=1)
                            for cc in range(0, cl, 512):
                                cww = min(512, cl - cc)
                                nc.tensor.matmul(yp[0:1, cc:cc + cww],
                                                 lhsT=w2_sb[:],
                                                 rhs=relu_sb[:, cc:cc + cww],
                                                 start=True, stop=True,
                                                 skip_group_check=True)
                            ysb = sbp.tile([1, cfg.dch], F32, tag="dys",
                                           name="dys", bufs=3)
                            nc.scalar.copy(ysb[:, :cl], yp[0:1, :cl])
                            nc.sync.dma_start(out=y_out[:, c0:c0 + cl],
                                              in_=ysb[:, :cl])

    nc.compile()
    return nc


# ---------------------------------------------------------------------------
# entry point
# ---------------------------------------------------------------------------

def make_in_maps(cfg, inputs, pa1, pcf, pa2, dec, scl_a, scl_c):
    f = lambda a: np.ascontiguousarray(np.asarray(a), dtype=np.float32)
    xc16 = f(inputs["x_customer"]).astype(np.float16)
    xa16 = f(inputs["x_article"]).astype(np.float16)
    wd1 = f(inputs["W_dec1"])
    base = dict(
        xc=xc16,
        W_msg1_ca=f(inputs["W_msg1_ca"]).astype(np.float16),
        W_self1_a=f(inputs["W_self1_a"]).astype(np.float16),
        W_msg1_ac=f(inputs["W_msg1_ac"]).astype(np.float16),
        W_self1_c=f(inputs["W_self1_c"]).astype(np.float16),
        W_msg2_ca=f(inputs["W_msg2_ca"]).astype(np.float16),
        W_self2_a=f(inputs["W_self2_a"]).astype(np.float16),
        W_msg2_ac=f(inputs["W_msg2_ac"]).astype(np.float16),
        W_self2_c=f(inputs["W_self2_c"]).astype(np.float16),
        Wd1c=wd1[:P].astype(np.float16), Wd1a=wd1[P:].astype(np.float16),
        w2r=np.tile(f(inputs["W_dec2"]).reshape(1, P),
                    (P, 8)).astype(np.float16),
        b1_a=f(inputs["b1_a"]).reshape(P, 1),
        b1_c=f(inputs["b1_c"]).reshape(P, 1),
        b2_a=f(inputs["b2_a"]).reshape(P, 1),
        b2_c=f(inputs["b2_c"]).reshape(P, 1),
        bn_gamma_c=f(inputs["bn_gamma_c"]).reshape(P, 1),
        bn_beta_c=f(inputs["bn_beta_c"]).reshape(P, 1),
        bn_gamma_a=f(inputs["bn_gamma_a"]).reshape(P, 1),
        bn_beta_a=f(inputs["bn_beta_a"]).reshape(P, 1),
        b_dec1=f(inputs["b_dec1"]).reshape(P, 1),
        b_dec2c=np.full((P, 1), float(np.asarray(inputs["b_dec2"]).item()),
                        np.float32),
    )
    in_maps = []
    for k in range(NCORES):
        m = dict(base)
        m["xa_own"] = np.ascontiguousarray(xa16[k * cfg.apc:(k + 1) * cfg.apc])
        m["xaT"] = np.ascontiguousarray(
            xa16[k * cfg.apc:(k + 1) * cfg.apc].T)
        m["xcT"] = np.ascontiguousarray(
            xc16[k * cfg.cpc:(k + 1) * cfg.cpc].T)
        m["scl_a_rep"] = np.tile(
            scl_a[k * cfg.apc:(k + 1) * cfg.apc].astype(np.float16)[None, :],
            (P, 1))
        m["scl_c_rep"] = np.tile(
            scl_c[k * cfg.cpc:(k + 1) * cfg.cpc].astype(np.float16)[None, :],
            (P, 1))
        m["a1_idx"], m["a1_dsc"] = pa1.idx[k], pa1.dsc[k]
        for h in range(2):
            m[f"cf{h}_idx"], m[f"cf{h}_dsc"] = pcf[h].idx[k], pcf[h].dsc[k]
        m["a2_idx"], m["a2_dsc"] = pa2.idx[k], pa2.dsc[k]
        m["dec_idx_a"], m["dec_crel"] = dec["idx_a"][k], dec["crel"][k]
        in_maps.append(m)
    return in_maps


def run(cfg, inputs, trace=False, dbg=False):
    pa1, pcf, pa2, dec, scl_a, scl_c = prep_all(cfg, inputs)
    in_maps = make_in_maps(cfg, inputs, pa1, pcf, pa2, dec, scl_a, scl_c)
    nc = build_nc(cfg, pa1, pcf, pa2, dec, dbg=dbg)
    res = run_bass_kernel_spmd(nc, in_maps, core_ids=list(range(NCORES)),
                               trace=trace)
    y = np.empty(cfg.e_lbl, np.float32)
    b2 = float(np.asarray(inputs["b_dec2"]).item())
    for k in range(NCORES):
        yl = res.results[k]["y"].T.reshape(-1) + b2
        po = dec["out_pos"][k]
        vm = po >= 0
        y[po[vm]] = yl[vm]
    return y, res


def kernel(**inputs):
    cfg = Cfg()
    y, _ = run(cfg, inputs, trace=False)
    return y
